# revision 4
# baseline (speedup 1.0000x reference)
"""Trainium2 Bass kernel for nn_Network_61658550501610 (Mamba block + MLP head).

Reference computation (per batch element b, sequence length L=2048):
  xz = x @ W_in.T; xi, z = split(xz)
  xc = silu(causal_depthwise_conv(xi, conv_w) + conv_b)
  x_dbl = xc @ W_xproj.T -> (dt, B, C)
  delta = softplus(dt @ W_dt.T + b_dt)
  h_t = exp(delta*A)*h_{t-1} + delta*B*xc   (selective scan, state [82,16])
  y = (h @ C) + D*xc; y *= silu(z)
  out = y @ W_out.T;  logits = relu(out@W_c1.T+b_c1)@W_c2.T + b_c2

Sharding: data-parallel over batch (B=16 -> 2 per core across 8 cores).

Engine assignment (v2 redesign):
  - x is pre-transposed and bf16-cast on the host -> no on-chip transposes.
  - Activation-table discipline: phase A0 uses the silu set, phases A1+B use
    the natural_log_exp set -> 2 table loads total instead of ~16.
  - The packed scan layout (rows = (n, dsub), 11 groups of 8 d's) as before,
    but: delta-broadcast on TensorE -> dA=exp on Act straight into PSUM; the
    u-broadcast is done by SBUF->SBUF DMA replication; dBx = u*B runs on
    GPSIMD (SBUF-only engine) so it can overlap the DVE scans, whose operands
    are PSUM(dA) + SBUF(dBx) and thus leave the shared DVE/GPSIMD SBUF port
    free.
"""
import ml_dtypes
import numpy as np

import concourse.bacc as bacc
import concourse.tile as tile
import concourse.mybir as mybir
from concourse.bass_utils import run_bass_kernel_spmd

F32 = mybir.dt.float32
BF16 = mybir.dt.bfloat16
OP = mybir.AluOpType
ACTF = mybir.ActivationFunctionType

# problem dims (hardcoded per contract)
B, L, DM = 16, 2048, 41
DIN, N, K = 82, 16, 4          # d_inner, d_state, d_conv
DTR, HID, NL = 3, 64, 10
NCORES = 8
BLOC = B // NCORES             # batch per core

DMP = 48                       # padded d_model
DP = 88                        # padded d_inner
DG = 11                        # d-groups of 8 for the packed scan
C = 512                        # time-chunk length
NCH = L // C                   # chunks per batch element
Q = C // 128                   # 128-row subtiles per chunk

# tuning knobs
DBX_GP = 11                    # how many of the 11 dBx groups run on gpsimd
HC_GP = 0                      # how many of the 11 hC groups run on gpsimd

_cache = {}


def _build(cfg):
    nc = bacc.Bacc("TRN2", target_bir_lowering=False, debug=False,
                   enable_asserts=False)

    def din(name, shape, dt=F32):
        return nc.dram_tensor(name, list(shape), dt, kind="ExternalInput").ap()

    xT_d = din("xT", (BLOC, DMP, L), BF16)
    w_inT_d = din("w_inT", (DMP, 2 * DP), BF16)
    conv_diag_d = din("conv_diag", (DP, K * DP), BF16)
    conv_b_d = din("conv_b", (DP, 1))
    w_effT_d = din("w_effT", (DP, DP), BF16)
    w_bcT_d = din("w_bcT", (DP, 2 * N), BF16)
    b_dt_d = din("b_dt", (DP, 1))
    d_col_d = din("d_col", (DP, 1))
    w1T_d = din("w1T", (DP, HID), BF16)
    b_c1_d = din("b_c1", (HID, 1))
    w2T_d = din("w2T", (HID, NL), BF16)
    p_sel_d = din("p_sel", (DP, DG * 128), BF16)
    ed_sel_d = din("ed_sel", (128, DG * DP), BF16)
    qb_sel_d = din("qb_sel", (2 * N, 128), BF16)
    qc_sel_d = din("qc_sel", (2 * N, 128), BF16)
    a_pack_d = din("a_pack", (128, DG))
    out_d = nc.dram_tensor("out", [BLOC, L, NL], F32, kind="ExternalOutput").ap()
    u_scr = nc.dram_tensor("u_scr", [BLOC, DP, L], BF16, kind="Internal").ap()

    with tile.TileContext(nc) as tc, tc.tile_pool(name="wts", bufs=1) as wp, \
         tc.tile_pool(name="xtp", bufs=3) as xp, \
         tc.tile_pool(name="rep", bufs=2) as rp, \
         tc.tile_pool(name="dbx", bufs=2) as bp, \
         tc.tile_pool(name="hbf", bufs=2) as hp, \
         tc.tile_pool(name="sml", bufs=4) as sp, \
         tc.tile_pool(name="yws", bufs=2) as yp, \
         tc.tile_pool(name="ps_f", bufs=3, space="PSUM") as pf, \
         tc.tile_pool(name="ps_d", bufs=2, space="PSUM") as pd, \
         tc.tile_pool(name="ps_a", bufs=2, space="PSUM") as pa, \
         tc.tile_pool(name="ps_y", bufs=1, space="PSUM") as py:

        # ---- constant weights ----
        w_inT = wp.tile([DMP, 2 * DP], BF16)
        conv_diag = wp.tile([DP, K * DP], BF16)
        conv_b = wp.tile([DP, 1], F32)
        w_effT = wp.tile([DP, DP], BF16)
        w_bcT = wp.tile([DP, 2 * N], BF16)
        b_dt = wp.tile([DP, 1], F32)
        d_col = wp.tile([DP, 1], F32)
        w1T = wp.tile([DP, HID], BF16)
        b_c1 = wp.tile([HID, 1], F32)
        w2T = wp.tile([HID, NL], BF16)
        p_sel = wp.tile([DP, DG * 128], BF16)
        ed_sel = wp.tile([128, DG * DP], BF16)
        qb_sel = wp.tile([2 * N, 128], BF16)
        qc_sel = wp.tile([2 * N, 128], BF16)
        a_pack = wp.tile([128, DG], F32)
        for t_, d_ in [(w_inT, w_inT_d), (conv_diag, conv_diag_d),
                       (conv_b, conv_b_d), (w_effT, w_effT_d),
                       (w_bcT, w_bcT_d), (b_dt, b_dt_d), (d_col, d_col_d),
                       (w1T, w1T_d), (b_c1, b_c1_d), (w2T, w2T_d),
                       (p_sel, p_sel_d), (ed_sel, ed_sel_d),
                       (qb_sel, qb_sel_d), (qc_sel, qc_sel_d),
                       (a_pack, a_pack_d)]:
            nc.sync.dma_start(t_[:], d_[:])

        # ---- per-batch-element persistent state ----
        xi_b = [wp.tile([DP, K - 1 + L], BF16, name=f"xi{i}", tag=f"xi{i}")
                for i in range(BLOC)]
        xc_b = [wp.tile([DP, L], BF16, name=f"xc{i}", tag=f"xc{i}")
                for i in range(BLOC)]
        zs_b = [wp.tile([DP, L], BF16, name=f"zs{i}", tag=f"zs{i}")
                for i in range(BLOC)]
        dl_b = [wp.tile([DP, L], BF16, name=f"dl{i}", tag=f"dl{i}")
                for i in range(BLOC)]
        u_b = [wp.tile([DP, L], BF16, name=f"u{i}", tag=f"u{i}")
               for i in range(BLOC)]
        br_b = [wp.tile([128, L], BF16, name=f"br{i}", tag=f"br{i}")
                for i in range(BLOC)]
        cr_b = [wp.tile([128, L], BF16, name=f"cr{i}", tag=f"cr{i}")
                for i in range(BLOC)]
        hcar_b = [wp.tile([128, DG], F32, name=f"hc{i}", tag=f"hc{i}")
                  for i in range(BLOC)]
        for t_ in xi_b:
            nc.vector.memset(t_[:, 0:K - 1], 0.0)

        iters = [(ch, b) for ch in range(NCH) for b in range(BLOC)]

        # ======== phase A0: input proj + conv + silus (silu table) ========
        for ch, b in iters:
            t0 = ch * C
            xT = xp.tile([DMP, C], BF16)
            nc.sync.dma_start(xT[:], xT_d[b, :, t0:t0 + C])

            xi_ps = pf.tile([DP, C], F32, tag="f")
            z_ps = pf.tile([DP, C], F32, tag="f")
            nc.tensor.matmul(xi_ps[:], w_inT[:, 0:DP], xT[:],
                             start=True, stop=True)
            nc.tensor.matmul(z_ps[:], w_inT[:, DP:2 * DP], xT[:],
                             start=True, stop=True)
            # z gate: silu in one op
            nc.scalar.activation(zs_b[b][:, t0:t0 + C], z_ps[:], ACTF.Silu)
            # xi to SBUF (DVE, PSUM-side read)
            nc.vector.tensor_copy(xi_b[b][:, K - 1 + t0:K - 1 + t0 + C],
                                  xi_ps[:])
            # causal depthwise conv as 4 accumulating diag matmuls
            xc_ps = pf.tile([DP, C], F32, tag="f")
            for k in range(K):
                nc.tensor.matmul(xc_ps[:], conv_diag[:, k * DP:(k + 1) * DP],
                                 xi_b[b][:, t0 + k:t0 + k + C],
                                 start=(k == 0), stop=(k == K - 1))
            nc.scalar.activation(xc_b[b][:, t0:t0 + C], xc_ps[:], ACTF.Silu,
                                 bias=conv_b[:])

        # ======== phases A1 + B interleaved (natural_log_exp table) ========
        for ch, b in iters:
            t0 = ch * C
            xc_s = xc_b[b][:, t0:t0 + C]
            # ---- A1: x_proj -> delta / B / C ----
            dpre_ps = pf.tile([DP, C], F32, tag="f")
            nc.tensor.matmul(dpre_ps[:], w_effT[:], xc_s, start=True, stop=True)
            esp_ps = pf.tile([DP, C], F32, tag="f")
            nc.scalar.activation(esp_ps[:], dpre_ps[:], ACTF.Exp, bias=b_dt[:])
            nc.scalar.activation(dl_b[b][:, t0:t0 + C], esp_ps[:], ACTF.Ln,
                                 bias=1.0)
            bc_ps = pf.tile([2 * N, C], F32, tag="f")
            nc.tensor.matmul(bc_ps[:], w_bcT[:], xc_s, start=True, stop=True)
            bc_sb = sp.tile([2 * N, C], BF16, tag="bc")
            nc.vector.tensor_copy(bc_sb[:], bc_ps[:])
            bq_ps = pd.tile([128, C], F32, tag="d")
            nc.tensor.matmul(bq_ps[:], qb_sel[:], bc_sb[:], start=True,
                             stop=True)
            nc.vector.tensor_copy(br_b[b][:, t0:t0 + C], bq_ps[:])
            cq_ps = pd.tile([128, C], F32, tag="d")
            nc.tensor.matmul(cq_ps[:], qc_sel[:], bc_sb[:], start=True,
                             stop=True)
            nc.vector.tensor_copy(cr_b[b][:, t0:t0 + C], cq_ps[:])
            # u = delta * xc
            nc.vector.tensor_tensor(u_b[b][:, t0:t0 + C],
                                    dl_b[b][:, t0:t0 + C], xc_s, op=OP.mult)
            nc.sync.dma_start(u_scr[b, :, t0:t0 + C], u_b[b][:, t0:t0 + C])

            # ---- B: packed selective scan ----
            # u replicated to the (n, dsub) packed layout via a DRAM
            # round-trip (DRAM APs are linear, so the partition-crossing
            # read pattern is legal)
            urep = rp.tile([128, DG * C], BF16, tag="ur")
            usrc = u_scr[b, :, t0:t0 + C].rearrange("(g p) t -> p g t", p=8)
            for n in range(N):
                nc.sync.dma_start(
                    urep[n * 8:(n + 1) * 8, :].rearrange(
                        "p (g t) -> p g t", g=DG), usrc)

            dbx = bp.tile([128, DG * C], BF16, tag="dbx")
            h = hp.tile([128, DG * C], BF16, tag="h")
            b_s = br_b[b][:, t0:t0 + C]
            c_s = cr_b[b][:, t0:t0 + C]
            dl_s = dl_b[b][:, t0:t0 + C]
            y_ps = py.tile([DP, C], F32, tag="y")
            for g in range(DG):
                # dBx = u_rep * B  (gpsimd; overlaps the DVE scans)
                eng = nc.gpsimd if g < DBX_GP else nc.vector
                eng.tensor_tensor(dbx[:, g * C:(g + 1) * C],
                                  urep[:, g * C:(g + 1) * C], b_s, op=OP.mult)
                # delta broadcast -> PSUM
                drep_ps = pd.tile([128, C], F32, tag="d")
                nc.tensor.matmul(drep_ps[:], p_sel[:, g * 128:(g + 1) * 128],
                                 dl_s, start=True, stop=True)
                # dA = exp(A * delta) -> PSUM (scan in0 reads PSUM)
                dA_ps = pa.tile([128, C], F32, tag="a")
                nc.scalar.activation(dA_ps[:], drep_ps[:], ACTF.Exp,
                                     scale=a_pack[:, g:g + 1])
                hs = h[:, g * C:(g + 1) * C]
                init = 0.0 if ch == 0 else hcar_b[b][:, g:g + 1]
                nc.vector.tensor_tensor_scan(hs, dA_ps[:],
                                             dbx[:, g * C:(g + 1) * C], init,
                                             op0=OP.mult, op1=OP.add)
                hC = sp.tile([128, C], BF16, tag="hC")
                heng = nc.gpsimd if g < HC_GP else nc.vector
                heng.tensor_tensor(hC[:], hs, c_s, op=OP.mult)
                nc.tensor.matmul(y_ps[:], ed_sel[:, g * DP:(g + 1) * DP],
                                 hC[:], start=(g == 0), stop=(g == DG - 1))
            if ch < NCH - 1:
                nc.vector.tensor_copy(
                    hcar_b[b][:].rearrange("p (g c) -> p g c", c=1),
                    h[:].rearrange("p (g c) -> p g c", g=DG)[:, :, C - 1:C])

            # ---- gate + head ----
            y1 = yp.tile([DP, C], BF16, tag="y1")
            nc.vector.scalar_tensor_tensor(y1[:], xc_s, d_col[:], y_ps[:],
                                           op0=OP.mult, op1=OP.add)
            yg = yp.tile([DP, C], BF16, tag="yg")
            nc.vector.tensor_tensor(yg[:], y1[:], zs_b[b][:, t0:t0 + C],
                                    op=OP.mult)
            g_ps = pf.tile([HID, C], F32, tag="f")
            nc.tensor.matmul(g_ps[:], w1T[:], yg[:], start=True, stop=True)
            g_sb = sp.tile([HID, C], BF16, tag="g")
            nc.vector.tensor_scalar(g_sb[:], g_ps[:], b_c1[:], 0.0,
                                    op0=OP.add, op1=OP.max)
            lg_ps = pf.tile([128, Q * NL], F32, tag="f")
            for q in range(Q):
                nc.tensor.matmul(lg_ps[:, q * NL:(q + 1) * NL],
                                 g_sb[:, q * 128:(q + 1) * 128], w2T[:],
                                 start=True, stop=True)
            out_sb = sp.tile([128, Q * NL], F32, tag="o")
            nc.vector.tensor_copy(out_sb[:], lg_ps[:])
            dst = out_d[b, t0:t0 + C, :].rearrange("(q p) c -> p q c", p=128)
            nc.sync.dma_start(dst,
                              out_sb[:].rearrange("p (q c) -> p q c", q=Q))

    nc.compile()
    return nc


def _packed_consts(A):
    p_sel = np.zeros((DP, DG * 128), np.float32)
    ed = np.zeros((128, DG * DP), np.float32)
    qb = np.zeros((2 * N, 128), np.float32)
    qc = np.zeros((2 * N, 128), np.float32)
    a_pack = np.zeros((128, DG), np.float32)
    for n in range(N):
        for ds in range(8):
            r = n * 8 + ds
            qb[n, r] = 1.0
            qc[N + n, r] = 1.0
            for g in range(DG):
                d = g * 8 + ds
                if d < DIN:
                    p_sel[d, g * 128 + r] = 1.0
                    ed[r, g * DP + d] = 1.0
                    a_pack[r, g] = A[d, n]
    bf = ml_dtypes.bfloat16
    return {"p_sel": p_sel.astype(bf), "ed_sel": ed.astype(bf),
            "qb_sel": qb.astype(bf), "qc_sel": qc.astype(bf),
            "a_pack": a_pack}


def _prep_inputs(inputs):
    bf = ml_dtypes.bfloat16
    x = np.asarray(inputs["x"], np.float32)
    W_in = np.asarray(inputs["W_in"], np.float64)
    conv_w = np.asarray(inputs["conv_w"], np.float64)
    conv_b = np.asarray(inputs["conv_b"], np.float64)
    W_xproj = np.asarray(inputs["W_xproj"], np.float64)
    W_dt = np.asarray(inputs["W_dt"], np.float64)
    b_dt = np.asarray(inputs["b_dt"], np.float64)
    A_log = np.asarray(inputs["A_log"], np.float64)
    D = np.asarray(inputs["D"], np.float64)
    W_out = np.asarray(inputs["W_out"], np.float64)
    W_c1 = np.asarray(inputs["W_c1"], np.float64)
    b_c1 = np.asarray(inputs["b_c1"], np.float64)
    W_c2 = np.asarray(inputs["W_c2"], np.float64)

    def padrc(a, rows, cols):
        out = np.zeros((rows, cols), np.float64)
        out[:a.shape[0], :a.shape[1]] = a
        return out

    # x: pad d_model 41->48, cast bf16, transpose to [b, d, t]
    xp = np.zeros((B, L, DMP), np.float32)
    xp[:, :, :DM] = x
    xT = np.ascontiguousarray(xp.transpose(0, 2, 1)).astype(bf)

    w_inT = np.zeros((DMP, 2 * DP), np.float64)
    w_inT[:DM, 0:DIN] = W_in[:DIN].T
    w_inT[:DM, DP:DP + DIN] = W_in[DIN:].T

    conv_diag = np.zeros((DP, K * DP), np.float64)
    for k in range(K):
        conv_diag[:DIN, k * DP:k * DP + DIN] = np.diag(conv_w[:, k])

    f32c = lambda a: np.ascontiguousarray(a, dtype=np.float32)
    bfc = lambda a: np.ascontiguousarray(a.astype(np.float32)).astype(bf)
    shared = {
        "w_inT": bfc(w_inT),
        "conv_diag": bfc(conv_diag),
        "conv_b": f32c(padrc(conv_b[:, None], DP, 1)),
        "w_effT": bfc(padrc((W_dt @ W_xproj[:DTR]).T, DP, DP)),
        "w_bcT": bfc(padrc(W_xproj[DTR:].T, DP, 2 * N)),
        "b_dt": f32c(padrc(b_dt[:, None], DP, 1)),
        "d_col": f32c(padrc(D[:, None], DP, 1)),
        "w1T": bfc(padrc((W_c1 @ W_out).T, DP, HID)),
        "b_c1": f32c(b_c1[:, None]),
        "w2T": bfc(W_c2.T),
        **_packed_consts((-np.exp(A_log)).astype(np.float32)),
    }
    in_maps = []
    for c in range(NCORES):
        m = dict(shared)
        m["xT"] = xT[c * BLOC:(c + 1) * BLOC]
        in_maps.append(m)
    return in_maps


def kernel(**inputs):
    return _run(inputs, trace=False)[0]


def kernel_traced(**inputs):
    return _run(inputs, trace=True)


def _run(inputs, trace=False):
    key = "nc"
    if key not in _cache:
        _cache[key] = _build({})
    nc = _cache[key]
    in_maps = _prep_inputs(inputs)
    res = run_bass_kernel_spmd(nc, in_maps, core_ids=list(range(NCORES)),
                               trace=trace)
    b_c2 = np.asarray(inputs["b_c2"], np.float32)
    out = np.concatenate([r["out"] for r in res.results], axis=0)
    out = out + b_c2[None, None, :]
    return out, res


# revision 6
# speedup vs baseline: 1.1430x; 1.1430x over previous
"""Trainium2 Bass kernel for nn_Network_61658550501610 (Mamba block + MLP head).

Reference computation (per batch element b, sequence length L=2048):
  xz = x @ W_in.T; xi, z = split(xz)
  xc = silu(causal_depthwise_conv(xi, conv_w) + conv_b)
  x_dbl = xc @ W_xproj.T -> (dt, B, C)
  delta = softplus(dt @ W_dt.T + b_dt)
  h_t = exp(delta*A)*h_{t-1} + delta*B*xc   (selective scan, state [82,16])
  y = (h @ C) + D*xc; y *= silu(z)
  out = y @ W_out.T;  logits = relu(out@W_c1.T+b_c1)@W_c2.T + b_c2

Sharding: data-parallel over batch (B=16 -> 2 per core across 8 cores).

Engine assignment (v2 redesign):
  - x is pre-transposed and bf16-cast on the host -> no on-chip transposes.
  - Activation-table discipline: phase A0 uses the silu set, phases A1+B use
    the natural_log_exp set -> 2 table loads total instead of ~16.
  - The packed scan layout (rows = (n, dsub), 11 groups of 8 d's) as before,
    but: delta-broadcast on TensorE -> dA=exp on Act straight into PSUM; the
    u-broadcast is done by SBUF->SBUF DMA replication; dBx = u*B runs on
    GPSIMD (SBUF-only engine) so it can overlap the DVE scans, whose operands
    are PSUM(dA) + SBUF(dBx) and thus leave the shared DVE/GPSIMD SBUF port
    free.
"""
import ml_dtypes
import numpy as np

import concourse.bacc as bacc
import concourse.tile as tile
import concourse.mybir as mybir
from concourse.bass_utils import run_bass_kernel_spmd

F32 = mybir.dt.float32
BF16 = mybir.dt.bfloat16
OP = mybir.AluOpType
ACTF = mybir.ActivationFunctionType

# problem dims (hardcoded per contract)
B, L, DM = 16, 2048, 41
DIN, N, K = 82, 16, 4          # d_inner, d_state, d_conv
DTR, HID, NL = 3, 64, 10
NCORES = 8
BLOC = B // NCORES             # batch per core

DMP = 48                       # padded d_model
DP = 88                        # padded d_inner
DG = 11                        # d-groups of 8 for the packed scan
C = 512                        # time-chunk length
NCH = L // C                   # chunks per batch element
Q = C // 128                   # 128-row subtiles per chunk

# tuning knobs
DBX_GP = 11                    # how many of the 11 dBx groups run on gpsimd
HC_GP = 0                      # how many of the 11 hC groups run on gpsimd

_cache = {}


def _build(cfg):
    nc = bacc.Bacc("TRN2", target_bir_lowering=False, debug=False,
                   enable_asserts=False)

    def din(name, shape, dt=F32):
        return nc.dram_tensor(name, list(shape), dt, kind="ExternalInput").ap()

    xT_d = din("xT", (BLOC, DMP, L), BF16)
    w_inT_d = din("w_inT", (DMP, 2 * DP), BF16)
    conv_diag_d = din("conv_diag", (DP, K * DP), BF16)
    conv_b_d = din("conv_b", (DP, 1))
    w_effT_d = din("w_effT", (DP, DP), BF16)
    w_bcT_d = din("w_bcT", (DP, 2 * N), BF16)
    b_dt_d = din("b_dt", (DP, 1))
    d_col_d = din("d_col", (DP, 1))
    w1T_d = din("w1T", (DP, HID), BF16)
    b_c1_d = din("b_c1", (HID, 1))
    w2T_d = din("w2T", (HID, NL), BF16)
    p_sel_d = din("p_sel", (DP, DG * 128), BF16)
    ed_sel_d = din("ed_sel", (128, DG * DP), BF16)
    qb_sel_d = din("qb_sel", (2 * N, 128), BF16)
    qc_sel_d = din("qc_sel", (2 * N, 128), BF16)
    a_pack_d = din("a_pack", (128, DG))
    out_d = nc.dram_tensor("out", [BLOC, L, NL], F32, kind="ExternalOutput").ap()
    u_scr = nc.dram_tensor("u_scr", [BLOC, DP, L], BF16, kind="Internal").ap()

    with tile.TileContext(nc) as tc, tc.tile_pool(name="wts", bufs=1) as wp, \
         tc.tile_pool(name="xtp", bufs=3) as xp, \
         tc.tile_pool(name="rep", bufs=3) as rp, \
         tc.tile_pool(name="dbx", bufs=2) as bp, \
         tc.tile_pool(name="hbf", bufs=2) as hp, \
         tc.tile_pool(name="sml", bufs=4) as sp, \
         tc.tile_pool(name="yws", bufs=2) as yp, \
         tc.tile_pool(name="ps_f", bufs=3, space="PSUM") as pf, \
         tc.tile_pool(name="ps_d", bufs=2, space="PSUM") as pd, \
         tc.tile_pool(name="ps_a", bufs=2, space="PSUM") as pa, \
         tc.tile_pool(name="ps_y", bufs=1, space="PSUM") as py:

        # ---- constant weights ----
        w_inT = wp.tile([DMP, 2 * DP], BF16)
        conv_diag = wp.tile([DP, K * DP], BF16)
        conv_b = wp.tile([DP, 1], F32)
        w_effT = wp.tile([DP, DP], BF16)
        w_bcT = wp.tile([DP, 2 * N], BF16)
        b_dt = wp.tile([DP, 1], F32)
        d_col = wp.tile([DP, 1], F32)
        w1T = wp.tile([DP, HID], BF16)
        b_c1 = wp.tile([HID, 1], F32)
        w2T = wp.tile([HID, NL], BF16)
        p_sel = wp.tile([DP, DG * 128], BF16)
        ed_sel = wp.tile([128, DG * DP], BF16)
        qb_sel = wp.tile([2 * N, 128], BF16)
        qc_sel = wp.tile([2 * N, 128], BF16)
        a_pack = wp.tile([128, DG], F32)
        for t_, d_ in [(w_inT, w_inT_d), (conv_diag, conv_diag_d),
                       (conv_b, conv_b_d), (w_effT, w_effT_d),
                       (w_bcT, w_bcT_d), (b_dt, b_dt_d), (d_col, d_col_d),
                       (w1T, w1T_d), (b_c1, b_c1_d), (w2T, w2T_d),
                       (p_sel, p_sel_d), (ed_sel, ed_sel_d),
                       (qb_sel, qb_sel_d), (qc_sel, qc_sel_d),
                       (a_pack, a_pack_d)]:
            nc.sync.dma_start(t_[:], d_[:])

        # ---- per-batch-element persistent state ----
        xi_b = [wp.tile([DP, K - 1 + L], BF16, name=f"xi{i}", tag=f"xi{i}")
                for i in range(BLOC)]
        xc_b = [wp.tile([DP, L], BF16, name=f"xc{i}", tag=f"xc{i}")
                for i in range(BLOC)]
        zs_b = [wp.tile([DP, L], BF16, name=f"zs{i}", tag=f"zs{i}")
                for i in range(BLOC)]
        dl_b = [wp.tile([DP, L], BF16, name=f"dl{i}", tag=f"dl{i}")
                for i in range(BLOC)]
        u_b = [wp.tile([DP, L], BF16, name=f"u{i}", tag=f"u{i}")
               for i in range(BLOC)]
        br_b = [wp.tile([128, L], BF16, name=f"br{i}", tag=f"br{i}")
                for i in range(BLOC)]
        cr_b = [wp.tile([128, L], BF16, name=f"cr{i}", tag=f"cr{i}")
                for i in range(BLOC)]
        hcar_b = [wp.tile([128, DG], F32, name=f"hc{i}", tag=f"hc{i}")
                  for i in range(BLOC)]
        for t_ in xi_b:
            nc.vector.memset(t_[:, 0:K - 1], 0.0)

        iters = [(ch, b) for ch in range(NCH) for b in range(BLOC)]

        # ======== phase A0: input proj + conv + silus (silu table) ========
        for ch, b in iters:
            t0 = ch * C
            xT = xp.tile([DMP, C], BF16)
            nc.sync.dma_start(xT[:], xT_d[b, :, t0:t0 + C])

            xi_ps = pf.tile([DP, C], F32, tag="f")
            z_ps = pf.tile([DP, C], F32, tag="f")
            nc.tensor.matmul(xi_ps[:], w_inT[:, 0:DP], xT[:],
                             start=True, stop=True)
            nc.tensor.matmul(z_ps[:], w_inT[:, DP:2 * DP], xT[:],
                             start=True, stop=True)
            # z gate: silu in one op
            nc.scalar.activation(zs_b[b][:, t0:t0 + C], z_ps[:], ACTF.Silu)
            # xi to SBUF (DVE, PSUM-side read)
            nc.vector.tensor_copy(xi_b[b][:, K - 1 + t0:K - 1 + t0 + C],
                                  xi_ps[:])
            # causal depthwise conv as 4 accumulating diag matmuls
            xc_ps = pf.tile([DP, C], F32, tag="f")
            for k in range(K):
                nc.tensor.matmul(xc_ps[:], conv_diag[:, k * DP:(k + 1) * DP],
                                 xi_b[b][:, t0 + k:t0 + k + C],
                                 start=(k == 0), stop=(k == K - 1))
            nc.scalar.activation(xc_b[b][:, t0:t0 + C], xc_ps[:], ACTF.Silu,
                                 bias=conv_b[:])

        # ======== phase A1: x_proj -> delta / B / C (lnexp table) ========
        for ch, b in iters:
            t0 = ch * C
            xc_s = xc_b[b][:, t0:t0 + C]
            dpre_ps = pf.tile([DP, C], F32, tag="f")
            nc.tensor.matmul(dpre_ps[:], w_effT[:], xc_s, start=True, stop=True)
            esp_ps = pf.tile([DP, C], F32, tag="f")
            nc.scalar.activation(esp_ps[:], dpre_ps[:], ACTF.Exp, bias=b_dt[:])
            nc.scalar.activation(dl_b[b][:, t0:t0 + C], esp_ps[:], ACTF.Ln,
                                 bias=1.0)
            bc_ps = pf.tile([2 * N, C], F32, tag="f")
            nc.tensor.matmul(bc_ps[:], w_bcT[:], xc_s, start=True, stop=True)
            bc_sb = sp.tile([2 * N, C], BF16, tag="bc")
            nc.vector.tensor_copy(bc_sb[:], bc_ps[:])
            bq_ps = pd.tile([128, C], F32, tag="d")
            nc.tensor.matmul(bq_ps[:], qb_sel[:], bc_sb[:], start=True,
                             stop=True)
            nc.vector.tensor_copy(br_b[b][:, t0:t0 + C], bq_ps[:])
            cq_ps = pd.tile([128, C], F32, tag="d")
            nc.tensor.matmul(cq_ps[:], qc_sel[:], bc_sb[:], start=True,
                             stop=True)
            nc.vector.tensor_copy(cr_b[b][:, t0:t0 + C], cq_ps[:])
            # u = delta * xc
            nc.vector.tensor_tensor(u_b[b][:, t0:t0 + C],
                                    dl_b[b][:, t0:t0 + C], xc_s, op=OP.mult)
            nc.sync.dma_start(u_scr[b, :, t0:t0 + C], u_b[b][:, t0:t0 + C])

        # ======== phase B: packed selective scan + head ========
        for ch, b in iters:
            t0 = ch * C
            xc_s = xc_b[b][:, t0:t0 + C]
            # u replicated to the (n, dsub) packed layout via a DRAM
            # round-trip (DRAM APs are linear, so the partition-crossing
            # read pattern is legal)
            urep = rp.tile([128, DG * C], BF16, tag="ur")
            usrc = u_scr[b, :, t0:t0 + C].rearrange("(g p) t -> p g t", p=8)
            for n in range(N):
                nc.sync.dma_start(
                    urep[n * 8:(n + 1) * 8, :].rearrange(
                        "p (g t) -> p g t", g=DG), usrc)

            dbx = bp.tile([128, DG * C], BF16, tag="dbx")
            h = hp.tile([128, DG * C], BF16, tag="h")
            b_s = br_b[b][:, t0:t0 + C]
            c_s = cr_b[b][:, t0:t0 + C]
            dl_s = dl_b[b][:, t0:t0 + C]
            y_ps = py.tile([DP, C], F32, tag="y")
            for g in range(DG):
                # dBx = u_rep * B  (gpsimd; overlaps the DVE scans)
                eng = nc.gpsimd if g < DBX_GP else nc.vector
                eng.tensor_tensor(dbx[:, g * C:(g + 1) * C],
                                  urep[:, g * C:(g + 1) * C], b_s, op=OP.mult)
                # delta broadcast -> PSUM
                drep_ps = pd.tile([128, C], F32, tag="d")
                nc.tensor.matmul(drep_ps[:], p_sel[:, g * 128:(g + 1) * 128],
                                 dl_s, start=True, stop=True)
                # dA = exp(A * delta) -> PSUM (scan in0 reads PSUM)
                dA_ps = pa.tile([128, C], F32, tag="a")
                nc.scalar.activation(dA_ps[:], drep_ps[:], ACTF.Exp,
                                     scale=a_pack[:, g:g + 1])
                hs = h[:, g * C:(g + 1) * C]
                init = 0.0 if ch == 0 else hcar_b[b][:, g:g + 1]
                nc.vector.tensor_tensor_scan(hs, dA_ps[:],
                                             dbx[:, g * C:(g + 1) * C], init,
                                             op0=OP.mult, op1=OP.add)
                hC = sp.tile([128, C], BF16, tag="hC")
                heng = nc.gpsimd if g < HC_GP else nc.vector
                heng.tensor_tensor(hC[:], hs, c_s, op=OP.mult)
                nc.tensor.matmul(y_ps[:], ed_sel[:, g * DP:(g + 1) * DP],
                                 hC[:], start=(g == 0), stop=(g == DG - 1))
            if ch < NCH - 1:
                nc.vector.tensor_copy(
                    hcar_b[b][:].rearrange("p (g c) -> p g c", c=1),
                    h[:].rearrange("p (g c) -> p g c", g=DG)[:, :, C - 1:C])

            # ---- gate + head ----
            y1 = yp.tile([DP, C], BF16, tag="y1")
            nc.vector.scalar_tensor_tensor(y1[:], xc_s, d_col[:], y_ps[:],
                                           op0=OP.mult, op1=OP.add)
            yg = yp.tile([DP, C], BF16, tag="yg")
            nc.vector.tensor_tensor(yg[:], y1[:], zs_b[b][:, t0:t0 + C],
                                    op=OP.mult)
            g_ps = pf.tile([HID, C], F32, tag="f")
            nc.tensor.matmul(g_ps[:], w1T[:], yg[:], start=True, stop=True)
            g_sb = sp.tile([HID, C], BF16, tag="g")
            nc.vector.tensor_scalar(g_sb[:], g_ps[:], b_c1[:], 0.0,
                                    op0=OP.add, op1=OP.max)
            lg_ps = pf.tile([128, Q * NL], F32, tag="f")
            for q in range(Q):
                nc.tensor.matmul(lg_ps[:, q * NL:(q + 1) * NL],
                                 g_sb[:, q * 128:(q + 1) * 128], w2T[:],
                                 start=True, stop=True)
            out_sb = sp.tile([128, Q * NL], F32, tag="o")
            nc.vector.tensor_copy(out_sb[:], lg_ps[:])
            dst = out_d[b, t0:t0 + C, :].rearrange("(q p) c -> p q c", p=128)
            nc.sync.dma_start(dst,
                              out_sb[:].rearrange("p (q c) -> p q c", q=Q))

    nc.compile()
    return nc


def _packed_consts(A):
    p_sel = np.zeros((DP, DG * 128), np.float32)
    ed = np.zeros((128, DG * DP), np.float32)
    qb = np.zeros((2 * N, 128), np.float32)
    qc = np.zeros((2 * N, 128), np.float32)
    a_pack = np.zeros((128, DG), np.float32)
    for n in range(N):
        for ds in range(8):
            r = n * 8 + ds
            qb[n, r] = 1.0
            qc[N + n, r] = 1.0
            for g in range(DG):
                d = g * 8 + ds
                if d < DIN:
                    p_sel[d, g * 128 + r] = 1.0
                    ed[r, g * DP + d] = 1.0
                    a_pack[r, g] = A[d, n]
    bf = ml_dtypes.bfloat16
    return {"p_sel": p_sel.astype(bf), "ed_sel": ed.astype(bf),
            "qb_sel": qb.astype(bf), "qc_sel": qc.astype(bf),
            "a_pack": a_pack}


def _prep_inputs(inputs):
    bf = ml_dtypes.bfloat16
    x = np.asarray(inputs["x"], np.float32)
    W_in = np.asarray(inputs["W_in"], np.float64)
    conv_w = np.asarray(inputs["conv_w"], np.float64)
    conv_b = np.asarray(inputs["conv_b"], np.float64)
    W_xproj = np.asarray(inputs["W_xproj"], np.float64)
    W_dt = np.asarray(inputs["W_dt"], np.float64)
    b_dt = np.asarray(inputs["b_dt"], np.float64)
    A_log = np.asarray(inputs["A_log"], np.float64)
    D = np.asarray(inputs["D"], np.float64)
    W_out = np.asarray(inputs["W_out"], np.float64)
    W_c1 = np.asarray(inputs["W_c1"], np.float64)
    b_c1 = np.asarray(inputs["b_c1"], np.float64)
    W_c2 = np.asarray(inputs["W_c2"], np.float64)

    def padrc(a, rows, cols):
        out = np.zeros((rows, cols), np.float64)
        out[:a.shape[0], :a.shape[1]] = a
        return out

    # x: pad d_model 41->48, cast bf16, transpose to [b, d, t]
    xp = np.zeros((B, L, DMP), np.float32)
    xp[:, :, :DM] = x
    xT = np.ascontiguousarray(xp.transpose(0, 2, 1)).astype(bf)

    w_inT = np.zeros((DMP, 2 * DP), np.float64)
    w_inT[:DM, 0:DIN] = W_in[:DIN].T
    w_inT[:DM, DP:DP + DIN] = W_in[DIN:].T

    conv_diag = np.zeros((DP, K * DP), np.float64)
    for k in range(K):
        conv_diag[:DIN, k * DP:k * DP + DIN] = np.diag(conv_w[:, k])

    f32c = lambda a: np.ascontiguousarray(a, dtype=np.float32)
    bfc = lambda a: np.ascontiguousarray(a.astype(np.float32)).astype(bf)
    shared = {
        "w_inT": bfc(w_inT),
        "conv_diag": bfc(conv_diag),
        "conv_b": f32c(padrc(conv_b[:, None], DP, 1)),
        "w_effT": bfc(padrc((W_dt @ W_xproj[:DTR]).T, DP, DP)),
        "w_bcT": bfc(padrc(W_xproj[DTR:].T, DP, 2 * N)),
        "b_dt": f32c(padrc(b_dt[:, None], DP, 1)),
        "d_col": f32c(padrc(D[:, None], DP, 1)),
        "w1T": bfc(padrc((W_c1 @ W_out).T, DP, HID)),
        "b_c1": f32c(b_c1[:, None]),
        "w2T": bfc(W_c2.T),
        **_packed_consts((-np.exp(A_log)).astype(np.float32)),
    }
    in_maps = []
    for c in range(NCORES):
        m = dict(shared)
        m["xT"] = xT[c * BLOC:(c + 1) * BLOC]
        in_maps.append(m)
    return in_maps


def kernel(**inputs):
    return _run(inputs, trace=False)[0]


def kernel_traced(**inputs):
    return _run(inputs, trace=True)


def _run(inputs, trace=False):
    key = "nc"
    if key not in _cache:
        _cache[key] = _build({})
    nc = _cache[key]
    in_maps = _prep_inputs(inputs)
    res = run_bass_kernel_spmd(nc, in_maps, core_ids=list(range(NCORES)),
                               trace=trace)
    b_c2 = np.asarray(inputs["b_c2"], np.float32)
    out = np.concatenate([r["out"] for r in res.results], axis=0)
    out = out + b_c2[None, None, :]
    return out, res


# revision 14
# speedup vs baseline: 1.3458x; 1.1774x over previous
"""Trainium2 Bass kernel for nn_Network_61658550501610 (Mamba block + MLP head).

Reference computation (per batch element b, sequence length L=2048):
  xz = x @ W_in.T; xi, z = split(xz)
  xc = silu(causal_depthwise_conv(xi, conv_w) + conv_b)
  x_dbl = xc @ W_xproj.T -> (dt, B, C)
  delta = softplus(dt @ W_dt.T + b_dt)
  h_t = exp(delta*A)*h_{t-1} + delta*B*xc   (selective scan, state [82,16])
  y = (h @ C) + D*xc; y *= silu(z)
  out = y @ W_out.T;  logits = relu(out@W_c1.T+b_c1)@W_c2.T + b_c2

Sharding: data-parallel over batch (B=16 -> 2 per core across 8 cores).

Engine assignment (v2 redesign):
  - x is pre-transposed and bf16-cast on the host -> no on-chip transposes.
  - Activation-table discipline: phase A0 uses the silu set, phases A1+B use
    the natural_log_exp set -> 2 table loads total instead of ~16.
  - The packed scan layout (rows = (n, dsub), 11 groups of 8 d's) as before,
    but: delta-broadcast on TensorE -> dA=exp on Act straight into PSUM; the
    u-broadcast is done by SBUF->SBUF DMA replication; dBx = u*B runs on
    GPSIMD (SBUF-only engine) so it can overlap the DVE scans, whose operands
    are PSUM(dA) + SBUF(dBx) and thus leave the shared DVE/GPSIMD SBUF port
    free.
"""
import ml_dtypes
import numpy as np

import concourse.bacc as bacc
import concourse.tile as tile
import concourse.mybir as mybir
from concourse.bass_utils import run_bass_kernel_spmd

F32 = mybir.dt.float32
BF16 = mybir.dt.bfloat16
OP = mybir.AluOpType
ACTF = mybir.ActivationFunctionType

# problem dims (hardcoded per contract)
B, L, DM = 16, 2048, 41
DIN, N, K = 82, 16, 4          # d_inner, d_state, d_conv
DTR, HID, NL = 3, 64, 10
NCORES = 8
BLOC = B // NCORES             # batch per core

DMP = 48                       # padded d_model
DP = 88                        # padded d_inner
DG = 11                        # d-groups of 8 for the packed scan
C = 512                        # time-chunk length
NCH = L // C                   # chunks per batch element
Q = C // 128                   # 128-row subtiles per chunk

# tuning knobs
DBX_GP = 11                    # how many of the 11 dBx groups run on gpsimd
HC_GP = 0                      # how many of the 11 hC groups run on gpsimd

_cache = {}


def _build(cfg):
    nc = bacc.Bacc("TRN2", target_bir_lowering=False, debug=False,
                   enable_asserts=False)

    def din(name, shape, dt=F32):
        return nc.dram_tensor(name, list(shape), dt, kind="ExternalInput").ap()

    xT_d = din("xT", (BLOC, DMP, L), BF16)
    w_inT_d = din("w_inT", (DMP, 2 * DP), BF16)
    conv_w_d = din("conv_w", (DP, K))
    conv_b_d = din("conv_b", (DP, 1))
    w_effT_d = din("w_effT", (DP, DP), BF16)
    w_bcT_d = din("w_bcT", (DP, 2 * N), BF16)
    b_dt_d = din("b_dt", (DP, 1))
    d_col_d = din("d_col", (DP, 1))
    w1T_d = din("w1T", (DP, HID), BF16)
    b_c1_d = din("b_c1", (HID, 1))
    w2T_d = din("w2T", (HID, NL), BF16)
    p_sel_d = din("p_sel", (DP, DG * 128), BF16)
    ed_sel_d = din("ed_sel", (128, DG * DP), BF16)
    qb_sel_d = din("qb_sel", (2 * N, 128), BF16)
    qc_sel_d = din("qc_sel", (2 * N, 128), BF16)
    a_pack_d = din("a_pack", (128, DG))
    out_d = nc.dram_tensor("out", [BLOC, L, NL], F32, kind="ExternalOutput").ap()
    u_scr = nc.dram_tensor("u_scr", [BLOC, DP, L], BF16, kind="Internal").ap()

    with tile.TileContext(nc) as tc, tc.tile_pool(name="wts", bufs=1) as wp, \
         tc.tile_pool(name="xtp", bufs=3) as xp, \
         tc.tile_pool(name="rep", bufs=3) as rp, \
         tc.tile_pool(name="dbx", bufs=2) as bp, \
         tc.tile_pool(name="hbf", bufs=2) as hp, \
         tc.tile_pool(name="sml", bufs=4) as sp, \
         tc.tile_pool(name="yws", bufs=2) as yp, \
         tc.tile_pool(name="ps_f", bufs=2, space="PSUM") as pf, \
         tc.tile_pool(name="ps_d", bufs=1, space="PSUM") as pd, \
         tc.tile_pool(name="ps_a", bufs=2, space="PSUM") as pa, \
         tc.tile_pool(name="ps_c", bufs=2, space="PSUM") as pc, \
         tc.tile_pool(name="ps_y", bufs=1, space="PSUM") as py:

        # ---- constant weights ----
        w_inT = wp.tile([DMP, 2 * DP], BF16)
        conv_w = wp.tile([DP, K], F32)
        conv_b = wp.tile([DP, 1], F32)
        w_effT = wp.tile([DP, DP], BF16)
        w_bcT = wp.tile([DP, 2 * N], BF16)
        b_dt = wp.tile([DP, 1], F32)
        d_col = wp.tile([DP, 1], F32)
        w1T = wp.tile([DP, HID], BF16)
        b_c1 = wp.tile([HID, 1], F32)
        w2T = wp.tile([HID, NL], BF16)
        p_sel = wp.tile([DP, DG * 128], BF16)
        ed_sel = wp.tile([128, DG * DP], BF16)
        qb_sel = wp.tile([2 * N, 128], BF16)
        qc_sel = wp.tile([2 * N, 128], BF16)
        a_pack = wp.tile([128, DG], F32)
        for t_, d_ in [(w_inT, w_inT_d), (conv_w, conv_w_d),
                       (conv_b, conv_b_d), (w_effT, w_effT_d),
                       (w_bcT, w_bcT_d), (b_dt, b_dt_d), (d_col, d_col_d),
                       (w1T, w1T_d), (b_c1, b_c1_d), (w2T, w2T_d),
                       (p_sel, p_sel_d), (ed_sel, ed_sel_d),
                       (qb_sel, qb_sel_d), (qc_sel, qc_sel_d),
                       (a_pack, a_pack_d)]:
            nc.sync.dma_start(t_[:], d_[:])

        # ---- per-batch-element persistent state ----
        xi_b = [wp.tile([DP, K - 1 + L], BF16, name=f"xi{i}", tag=f"xi{i}")
                for i in range(BLOC)]
        xc_b = [wp.tile([DP, L], BF16, name=f"xc{i}", tag=f"xc{i}")
                for i in range(BLOC)]
        zs_b = [wp.tile([DP, L], BF16, name=f"zs{i}", tag=f"zs{i}")
                for i in range(BLOC)]
        dl_b = [wp.tile([DP, L], BF16, name=f"dl{i}", tag=f"dl{i}")
                for i in range(BLOC)]
        u_b = [wp.tile([DP, L], BF16, name=f"u{i}", tag=f"u{i}")
               for i in range(BLOC)]
        br_b = [wp.tile([128, L], BF16, name=f"br{i}", tag=f"br{i}")
                for i in range(BLOC)]
        hcar_b = [wp.tile([128, DG], F32, name=f"hc{i}", tag=f"hc{i}")
                  for i in range(BLOC)]
        for t_ in xi_b:
            nc.vector.memset(t_[:, 0:K - 1], 0.0)

        iters = [(ch, b) for ch in range(NCH) for b in range(BLOC)]

        # ======== phase A0: input proj + conv + silus (silu table) ========
        for ch, b in iters:
            t0 = ch * C
            xT = xp.tile([DMP, C], BF16)
            nc.sync.dma_start(xT[:], xT_d[b, :, t0:t0 + C])

            xi_ps = pf.tile([DP, C], F32, tag="f")
            z_ps = pf.tile([DP, C], F32, tag="f")
            nc.tensor.matmul(xi_ps[:], w_inT[:, 0:DP], xT[:],
                             start=True, stop=True)
            nc.tensor.matmul(z_ps[:], w_inT[:, DP:2 * DP], xT[:],
                             start=True, stop=True)
            # z gate: silu in one op
            nc.scalar.activation(zs_b[b][:, t0:t0 + C], z_ps[:], ACTF.Silu)
            # xi to SBUF (DVE, PSUM-side read)
            nc.vector.tensor_copy(xi_b[b][:, K - 1 + t0:K - 1 + t0 + C],
                                  xi_ps[:])
            # causal depthwise conv: 4-tap FIR on DVE (shifted reads)
            xi = xi_b[b]
            acc = sp.tile([DP, C], F32, tag="cv")
            nc.vector.tensor_scalar(acc[:], xi[:, t0:t0 + C],
                                    conv_w[:, 0:1], None, op0=OP.mult)
            for k in range(1, K):
                nxt = sp.tile([DP, C], F32, tag="cv")
                nc.vector.scalar_tensor_tensor(nxt[:], xi[:, t0 + k:t0 + k + C],
                                               conv_w[:, k:k + 1], acc[:],
                                               op0=OP.mult, op1=OP.add)
                acc = nxt
            nc.scalar.activation(xc_b[b][:, t0:t0 + C], acc[:], ACTF.Silu,
                                 bias=conv_b[:])

        # ======== unified phase A1+B (natural_log_exp table) ========
        # Per outer iteration i: first the x_proj/softplus work for i+1
        # (pipelined one ahead), then the u-replication DMAs for i+1, then
        # the packed scan + head for i. The u replication goes through a
        # DRAM round-trip (DRAM APs are linear -> the partition-crossing
        # read is legal); the read-triggers sit on the sync queue before
        # out-DMA(i) so transfers overlap iteration i's compute.
        def a1(j):
            chj, bj = iters[j]
            tj = chj * C
            xc_j = xc_b[bj][:, tj:tj + C]
            dpre_ps = pf.tile([DP, C], F32, tag="f")
            nc.tensor.matmul(dpre_ps[:], w_effT[:], xc_j, start=True,
                             stop=True)
            esp_ps = pf.tile([DP, C], F32, tag="f")
            nc.scalar.activation(esp_ps[:], dpre_ps[:], ACTF.Exp, bias=b_dt[:])
            nc.scalar.activation(dl_b[bj][:, tj:tj + C], esp_ps[:], ACTF.Ln,
                                 bias=1.0)
            bc_ps = pf.tile([2 * N, C], F32, tag="f")
            nc.tensor.matmul(bc_ps[:], w_bcT[:], xc_j, start=True, stop=True)
            bc_sb = sp.tile([2 * N, C], BF16, tag="bc")
            nc.scalar.copy(bc_sb[:], bc_ps[:])
            bq_ps = pd.tile([128, C], F32, tag="d")
            nc.tensor.matmul(bq_ps[:], qb_sel[:], bc_sb[:], start=True,
                             stop=True)
            nc.scalar.copy(br_b[bj][:, tj:tj + C], bq_ps[:])
            # u = delta * xc -> DRAM scratch (write on the Act HWDGE queue)
            nc.vector.tensor_tensor(u_b[bj][:, tj:tj + C],
                                    dl_b[bj][:, tj:tj + C], xc_j, op=OP.mult)
            nc.scalar.dma_start(u_scr[bj, :, tj:tj + C], u_b[bj][:, tj:tj + C])
            return bc_sb

        def issue_urep(i):
            chi, bi = iters[i]
            ti = chi * C
            ur = rp.tile([128, DG * C], BF16, tag="ur")
            usrc = u_scr[bi, :, ti:ti + C].rearrange("(g p) t -> p g t", p=8)
            for n in range(N):
                nc.sync.dma_start(
                    ur[n * 8:(n + 1) * 8, :].rearrange(
                        "p (g t) -> p g t", g=DG), usrc)
            return ur

        bcs = {0: a1(0)}
        ureps = {0: issue_urep(0)}
        for it, (ch, b) in enumerate(iters):
            t0 = ch * C
            xc_s = xc_b[b][:, t0:t0 + C]
            if it + 1 < len(iters):
                bcs[it + 1] = a1(it + 1)
                ureps[it + 1] = issue_urep(it + 1)
            urep = ureps.pop(it)
            bc_sb = bcs.pop(it)

            # C in PSUM for the whole g-loop: hC becomes a single-SBUF-port
            # op, so gpsimd dBx is never locked out by it
            c_ps = pc.tile([128, C], F32, tag="c")
            nc.tensor.matmul(c_ps[:], qc_sel[:], bc_sb[:], start=True,
                             stop=True)

            dbx = bp.tile([128, DG * C], BF16, tag="dbx")
            h = hp.tile([128, DG * C], BF16, tag="h")
            b_s = br_b[b][:, t0:t0 + C]
            dl_s = dl_b[b][:, t0:t0 + C]
            y_ps = py.tile([DP, C], F32, tag="y")
            for g in range(DG):
                # dBx = u_rep * B  (gpsimd; overlaps the DVE scans)
                eng = nc.gpsimd if g < DBX_GP else nc.vector
                eng.tensor_tensor(dbx[:, g * C:(g + 1) * C],
                                  urep[:, g * C:(g + 1) * C], b_s, op=OP.mult)
                # delta broadcast -> PSUM
                drep_ps = pd.tile([128, C], F32, tag="d")
                nc.tensor.matmul(drep_ps[:], p_sel[:, g * 128:(g + 1) * 128],
                                 dl_s, start=True, stop=True)
                # dA = exp(A * delta) -> PSUM (scan in0 reads PSUM)
                dA_ps = pa.tile([128, C], F32, tag="a")
                nc.scalar.activation(dA_ps[:], drep_ps[:], ACTF.Exp,
                                     scale=a_pack[:, g:g + 1])
                hs = h[:, g * C:(g + 1) * C]
                init = 0.0 if ch == 0 else hcar_b[b][:, g:g + 1]
                nc.vector.tensor_tensor_scan(hs, dA_ps[:],
                                             dbx[:, g * C:(g + 1) * C], init,
                                             op0=OP.mult, op1=OP.add)
                hC = sp.tile([128, C], BF16, tag="hC")
                heng = nc.gpsimd if g < HC_GP else nc.vector
                heng.tensor_tensor(hC[:], hs, c_ps[:], op=OP.mult)
                nc.tensor.matmul(y_ps[:], ed_sel[:, g * DP:(g + 1) * DP],
                                 hC[:], start=(g == 0), stop=(g == DG - 1))
            if ch < NCH - 1:
                nc.vector.tensor_copy(
                    hcar_b[b][:].rearrange("p (g c) -> p g c", c=1),
                    h[:].rearrange("p (g c) -> p g c", g=DG)[:, :, C - 1:C])

            # ---- gate + head ----
            y1 = yp.tile([DP, C], BF16, tag="y1")
            nc.vector.scalar_tensor_tensor(y1[:], xc_s, d_col[:], y_ps[:],
                                           op0=OP.mult, op1=OP.add)
            yg = yp.tile([DP, C], BF16, tag="yg")
            nc.vector.tensor_tensor(yg[:], y1[:], zs_b[b][:, t0:t0 + C],
                                    op=OP.mult)
            g_ps = pf.tile([HID, C], F32, tag="f")
            nc.tensor.matmul(g_ps[:], w1T[:], yg[:], start=True, stop=True)
            g_sb = sp.tile([HID, C], BF16, tag="g")
            nc.vector.tensor_scalar(g_sb[:], g_ps[:], b_c1[:], 0.0,
                                    op0=OP.add, op1=OP.max)
            lg_ps = pf.tile([128, Q * NL], F32, tag="f")
            for q in range(Q):
                nc.tensor.matmul(lg_ps[:, q * NL:(q + 1) * NL],
                                 g_sb[:, q * 128:(q + 1) * 128], w2T[:],
                                 start=True, stop=True)
            out_sb = sp.tile([128, Q * NL], F32, tag="o")
            nc.vector.tensor_copy(out_sb[:], lg_ps[:])
            dst = out_d[b, t0:t0 + C, :].rearrange("(q p) c -> p q c", p=128)
            nc.sync.dma_start(dst,
                              out_sb[:].rearrange("p (q c) -> p q c", q=Q))

    nc.compile()
    return nc


def _packed_consts(A):
    p_sel = np.zeros((DP, DG * 128), np.float32)
    ed = np.zeros((128, DG * DP), np.float32)
    qb = np.zeros((2 * N, 128), np.float32)
    qc = np.zeros((2 * N, 128), np.float32)
    a_pack = np.zeros((128, DG), np.float32)
    for n in range(N):
        for ds in range(8):
            r = n * 8 + ds
            qb[n, r] = 1.0
            qc[N + n, r] = 1.0
            for g in range(DG):
                d = g * 8 + ds
                if d < DIN:
                    p_sel[d, g * 128 + r] = 1.0
                    ed[r, g * DP + d] = 1.0
                    a_pack[r, g] = A[d, n]
    bf = ml_dtypes.bfloat16
    return {"p_sel": p_sel.astype(bf), "ed_sel": ed.astype(bf),
            "qb_sel": qb.astype(bf), "qc_sel": qc.astype(bf),
            "a_pack": a_pack}


def _prep_inputs(inputs):
    bf = ml_dtypes.bfloat16
    x = np.asarray(inputs["x"], np.float32)
    W_in = np.asarray(inputs["W_in"], np.float64)
    conv_w = np.asarray(inputs["conv_w"], np.float64)
    conv_b = np.asarray(inputs["conv_b"], np.float64)
    W_xproj = np.asarray(inputs["W_xproj"], np.float64)
    W_dt = np.asarray(inputs["W_dt"], np.float64)
    b_dt = np.asarray(inputs["b_dt"], np.float64)
    A_log = np.asarray(inputs["A_log"], np.float64)
    D = np.asarray(inputs["D"], np.float64)
    W_out = np.asarray(inputs["W_out"], np.float64)
    W_c1 = np.asarray(inputs["W_c1"], np.float64)
    b_c1 = np.asarray(inputs["b_c1"], np.float64)
    W_c2 = np.asarray(inputs["W_c2"], np.float64)

    def padrc(a, rows, cols):
        out = np.zeros((rows, cols), np.float64)
        out[:a.shape[0], :a.shape[1]] = a
        return out

    # x: pad d_model 41->48, cast bf16, transpose to [b, d, t]
    xp = np.zeros((B, L, DMP), np.float32)
    xp[:, :, :DM] = x
    xT = np.ascontiguousarray(xp.transpose(0, 2, 1)).astype(bf)

    w_inT = np.zeros((DMP, 2 * DP), np.float64)
    w_inT[:DM, 0:DIN] = W_in[:DIN].T
    w_inT[:DM, DP:DP + DIN] = W_in[DIN:].T

    f32c = lambda a: np.ascontiguousarray(a, dtype=np.float32)
    bfc = lambda a: np.ascontiguousarray(a.astype(np.float32)).astype(bf)
    shared = {
        "w_inT": bfc(w_inT),
        "conv_w": f32c(padrc(conv_w, DP, K)),
        "conv_b": f32c(padrc(conv_b[:, None], DP, 1)),
        "w_effT": bfc(padrc((W_dt @ W_xproj[:DTR]).T, DP, DP)),
        "w_bcT": bfc(padrc(W_xproj[DTR:].T, DP, 2 * N)),
        "b_dt": f32c(padrc(b_dt[:, None], DP, 1)),
        "d_col": f32c(padrc(D[:, None], DP, 1)),
        "w1T": bfc(padrc((W_c1 @ W_out).T, DP, HID)),
        "b_c1": f32c(b_c1[:, None]),
        "w2T": bfc(W_c2.T),
        **_packed_consts((-np.exp(A_log)).astype(np.float32)),
    }
    in_maps = []
    for c in range(NCORES):
        m = dict(shared)
        m["xT"] = xT[c * BLOC:(c + 1) * BLOC]
        in_maps.append(m)
    return in_maps


def kernel(**inputs):
    return _run(inputs, trace=False)[0]


def kernel_traced(**inputs):
    return _run(inputs, trace=True)


def _run(inputs, trace=False):
    key = "nc"
    if key not in _cache:
        _cache[key] = _build({})
    nc = _cache[key]
    in_maps = _prep_inputs(inputs)
    res = run_bass_kernel_spmd(nc, in_maps, core_ids=list(range(NCORES)),
                               trace=trace)
    b_c2 = np.asarray(inputs["b_c2"], np.float32)
    out = np.concatenate([r["out"] for r in res.results], axis=0)
    out = out + b_c2[None, None, :]
    return out, res


# revision 15
# speedup vs baseline: 1.3745x; 1.0213x over previous
"""Trainium2 Bass kernel for nn_Network_61658550501610 (Mamba block + MLP head).

Reference computation (per batch element b, sequence length L=2048):
  xz = x @ W_in.T; xi, z = split(xz)
  xc = silu(causal_depthwise_conv(xi, conv_w) + conv_b)
  x_dbl = xc @ W_xproj.T -> (dt, B, C)
  delta = softplus(dt @ W_dt.T + b_dt)
  h_t = exp(delta*A)*h_{t-1} + delta*B*xc   (selective scan, state [82,16])
  y = (h @ C) + D*xc; y *= silu(z)
  out = y @ W_out.T;  logits = relu(out@W_c1.T+b_c1)@W_c2.T + b_c2

Sharding: data-parallel over batch (B=16 -> 2 per core across 8 cores).

Engine assignment (v2 redesign):
  - x is pre-transposed and bf16-cast on the host -> no on-chip transposes.
  - Activation-table discipline: phase A0 uses the silu set, phases A1+B use
    the natural_log_exp set -> 2 table loads total instead of ~16.
  - The packed scan layout (rows = (n, dsub), 11 groups of 8 d's) as before,
    but: delta-broadcast on TensorE -> dA=exp on Act straight into PSUM; the
    u-broadcast is done by SBUF->SBUF DMA replication; dBx = u*B runs on
    GPSIMD (SBUF-only engine) so it can overlap the DVE scans, whose operands
    are PSUM(dA) + SBUF(dBx) and thus leave the shared DVE/GPSIMD SBUF port
    free.
"""
import ml_dtypes
import numpy as np

import concourse.bacc as bacc
import concourse.tile as tile
import concourse.mybir as mybir
from concourse.bass_utils import run_bass_kernel_spmd

F32 = mybir.dt.float32
BF16 = mybir.dt.bfloat16
OP = mybir.AluOpType
ACTF = mybir.ActivationFunctionType

# problem dims (hardcoded per contract)
B, L, DM = 16, 2048, 41
DIN, N, K = 82, 16, 4          # d_inner, d_state, d_conv
DTR, HID, NL = 3, 64, 10
NCORES = 8
BLOC = B // NCORES             # batch per core

DMP = 48                       # padded d_model
DP = 88                        # padded d_inner
DG = 11                        # d-groups of 8 for the packed scan
C = 512                        # time-chunk length
NCH = L // C                   # chunks per batch element
Q = C // 128                   # 128-row subtiles per chunk

# tuning knobs
DBX_GP = 11                    # how many of the 11 dBx groups run on gpsimd
HC_GP = 0                      # how many of the 11 hC groups run on gpsimd

_cache = {}


def _build(cfg):
    nc = bacc.Bacc("TRN2", target_bir_lowering=False, debug=False,
                   enable_asserts=False)

    def din(name, shape, dt=F32):
        return nc.dram_tensor(name, list(shape), dt, kind="ExternalInput").ap()

    xT_d = din("xT", (BLOC, DMP, L), BF16)
    w_inT_d = din("w_inT", (DMP, 2 * DP), BF16)
    conv_w_d = din("conv_w", (DP, K))
    conv_b_d = din("conv_b", (DP, 1))
    w_effT_d = din("w_effT", (DP, DP), BF16)
    w_bcT_d = din("w_bcT", (DP, 2 * N), BF16)
    b_dt_d = din("b_dt", (DP, 1))
    d_col_d = din("d_col", (DP, 1))
    w1T_d = din("w1T", (DP, HID), BF16)
    b_c1_d = din("b_c1", (HID, 1))
    w2T_d = din("w2T", (HID, NL), BF16)
    p_sel_d = din("p_sel", (DP, DG * 128), BF16)
    ed_sel_d = din("ed_sel", (128, DG * DP), BF16)
    qb_sel_d = din("qb_sel", (2 * N, 128), BF16)
    qc_sel_d = din("qc_sel", (2 * N, 128), BF16)
    a_pack_d = din("a_pack", (128, DG))
    out_d = nc.dram_tensor("out", [BLOC, L, NL], F32, kind="ExternalOutput").ap()
    u_scr = nc.dram_tensor("u_scr", [BLOC, DP, L], BF16, kind="Internal").ap()

    with tile.TileContext(nc) as tc, tc.tile_pool(name="wts", bufs=1) as wp, \
         tc.tile_pool(name="xtp", bufs=3) as xp, \
         tc.tile_pool(name="rep", bufs=3) as rp, \
         tc.tile_pool(name="dbx", bufs=2) as bp, \
         tc.tile_pool(name="hbf", bufs=2) as hp, \
         tc.tile_pool(name="sml", bufs=4) as sp, \
         tc.tile_pool(name="yws", bufs=2) as yp, \
         tc.tile_pool(name="ps_f", bufs=3, space="PSUM") as pf, \
         tc.tile_pool(name="ps_d", bufs=1, space="PSUM") as pd, \
         tc.tile_pool(name="ps_a", bufs=2, space="PSUM") as pa, \
         tc.tile_pool(name="ps_c", bufs=1, space="PSUM") as pc, \
         tc.tile_pool(name="ps_y", bufs=1, space="PSUM") as py:

        # ---- constant weights ----
        w_inT = wp.tile([DMP, 2 * DP], BF16)
        conv_w = wp.tile([DP, K], F32)
        conv_b = wp.tile([DP, 1], F32)
        w_effT = wp.tile([DP, DP], BF16)
        w_bcT = wp.tile([DP, 2 * N], BF16)
        b_dt = wp.tile([DP, 1], F32)
        d_col = wp.tile([DP, 1], F32)
        w1T = wp.tile([DP, HID], BF16)
        b_c1 = wp.tile([HID, 1], F32)
        w2T = wp.tile([HID, NL], BF16)
        p_sel = wp.tile([DP, DG * 128], BF16)
        ed_sel = wp.tile([128, DG * DP], BF16)
        qb_sel = wp.tile([2 * N, 128], BF16)
        qc_sel = wp.tile([2 * N, 128], BF16)
        a_pack = wp.tile([128, DG], F32)
        for t_, d_ in [(w_inT, w_inT_d), (conv_w, conv_w_d),
                       (conv_b, conv_b_d), (w_effT, w_effT_d),
                       (w_bcT, w_bcT_d), (b_dt, b_dt_d), (d_col, d_col_d),
                       (w1T, w1T_d), (b_c1, b_c1_d), (w2T, w2T_d),
                       (p_sel, p_sel_d), (ed_sel, ed_sel_d),
                       (qb_sel, qb_sel_d), (qc_sel, qc_sel_d),
                       (a_pack, a_pack_d)]:
            nc.sync.dma_start(t_[:], d_[:])

        # ---- per-batch-element persistent state ----
        xi_b = [wp.tile([DP, K - 1 + L], BF16, name=f"xi{i}", tag=f"xi{i}")
                for i in range(BLOC)]
        xc_b = [wp.tile([DP, L], BF16, name=f"xc{i}", tag=f"xc{i}")
                for i in range(BLOC)]
        zs_b = [wp.tile([DP, L], BF16, name=f"zs{i}", tag=f"zs{i}")
                for i in range(BLOC)]
        dl_b = [wp.tile([DP, L], BF16, name=f"dl{i}", tag=f"dl{i}")
                for i in range(BLOC)]
        u_b = [wp.tile([DP, L], BF16, name=f"u{i}", tag=f"u{i}")
               for i in range(BLOC)]
        br_b = [wp.tile([128, L], BF16, name=f"br{i}", tag=f"br{i}")
                for i in range(BLOC)]
        hcar_b = [wp.tile([128, DG], F32, name=f"hc{i}", tag=f"hc{i}")
                  for i in range(BLOC)]
        for t_ in xi_b:
            nc.vector.memset(t_[:, 0:K - 1], 0.0)

        iters = [(ch, b) for ch in range(NCH) for b in range(BLOC)]

        # ======== phase A0: input proj + conv + silus (silu table) ========
        for ch, b in iters:
            t0 = ch * C
            xT = xp.tile([DMP, C], BF16)
            nc.sync.dma_start(xT[:], xT_d[b, :, t0:t0 + C])

            xi_ps = pf.tile([DP, C], F32, tag="f")
            z_ps = pf.tile([DP, C], F32, tag="f")
            nc.tensor.matmul(xi_ps[:], w_inT[:, 0:DP], xT[:],
                             start=True, stop=True)
            nc.tensor.matmul(z_ps[:], w_inT[:, DP:2 * DP], xT[:],
                             start=True, stop=True)
            # z gate: silu in one op
            nc.scalar.activation(zs_b[b][:, t0:t0 + C], z_ps[:], ACTF.Silu)
            # xi to SBUF (Act copy; DVE is busy with the conv FIR)
            nc.scalar.copy(xi_b[b][:, K - 1 + t0:K - 1 + t0 + C], xi_ps[:])
            # causal depthwise conv: 4-tap FIR on DVE (shifted reads)
            xi = xi_b[b]
            acc = sp.tile([DP, C], F32, tag="cv")
            nc.vector.tensor_scalar(acc[:], xi[:, t0:t0 + C],
                                    conv_w[:, 0:1], None, op0=OP.mult)
            for k in range(1, K):
                nxt = sp.tile([DP, C], F32, tag="cv")
                nc.vector.scalar_tensor_tensor(nxt[:], xi[:, t0 + k:t0 + k + C],
                                               conv_w[:, k:k + 1], acc[:],
                                               op0=OP.mult, op1=OP.add)
                acc = nxt
            nc.scalar.activation(xc_b[b][:, t0:t0 + C], acc[:], ACTF.Silu,
                                 bias=conv_b[:])

        # ======== unified phase A1+B (natural_log_exp table) ========
        # Per outer iteration i: first the x_proj/softplus work for i+1
        # (pipelined one ahead), then the u-replication DMAs for i+1, then
        # the packed scan + head for i. The u replication goes through a
        # DRAM round-trip (DRAM APs are linear -> the partition-crossing
        # read is legal); the read-triggers sit on the sync queue before
        # out-DMA(i) so transfers overlap iteration i's compute.
        def a1(j):
            chj, bj = iters[j]
            tj = chj * C
            xc_j = xc_b[bj][:, tj:tj + C]
            dpre_ps = pf.tile([DP, C], F32, tag="f")
            nc.tensor.matmul(dpre_ps[:], w_effT[:], xc_j, start=True,
                             stop=True)
            esp_sb = sp.tile([DP, C], BF16, tag="es")
            nc.scalar.activation(esp_sb[:], dpre_ps[:], ACTF.Exp, bias=b_dt[:])
            nc.scalar.activation(dl_b[bj][:, tj:tj + C], esp_sb[:], ACTF.Ln,
                                 bias=1.0)
            bc_ps = pf.tile([2 * N, C], F32, tag="f")
            nc.tensor.matmul(bc_ps[:], w_bcT[:], xc_j, start=True, stop=True)
            bc_sb = sp.tile([2 * N, C], BF16, tag="bc")
            nc.scalar.copy(bc_sb[:], bc_ps[:])
            bq_ps = pd.tile([128, C], F32, tag="d")
            nc.tensor.matmul(bq_ps[:], qb_sel[:], bc_sb[:], start=True,
                             stop=True)
            nc.scalar.copy(br_b[bj][:, tj:tj + C], bq_ps[:])
            # u = delta * xc -> DRAM scratch (write on the Act HWDGE queue)
            nc.vector.tensor_tensor(u_b[bj][:, tj:tj + C],
                                    dl_b[bj][:, tj:tj + C], xc_j, op=OP.mult)
            nc.scalar.dma_start(u_scr[bj, :, tj:tj + C], u_b[bj][:, tj:tj + C])
            return bc_sb

        def issue_urep(i):
            chi, bi = iters[i]
            ti = chi * C
            ur = rp.tile([128, DG * C], BF16, tag="ur")
            usrc = u_scr[bi, :, ti:ti + C].rearrange("(g p) t -> p g t", p=8)
            for n in range(N):
                nc.sync.dma_start(
                    ur[n * 8:(n + 1) * 8, :].rearrange(
                        "p (g t) -> p g t", g=DG), usrc)
            return ur

        bcs = {0: a1(0)}
        ureps = {0: issue_urep(0)}
        for it, (ch, b) in enumerate(iters):
            t0 = ch * C
            xc_s = xc_b[b][:, t0:t0 + C]
            if it + 1 < len(iters):
                bcs[it + 1] = a1(it + 1)
                ureps[it + 1] = issue_urep(it + 1)
            urep = ureps.pop(it)
            bc_sb = bcs.pop(it)

            # C in PSUM for the whole g-loop: hC becomes a single-SBUF-port
            # op, so gpsimd dBx is never locked out by it
            c_ps = pc.tile([128, C], F32, tag="c")
            nc.tensor.matmul(c_ps[:], qc_sel[:], bc_sb[:], start=True,
                             stop=True)

            dbx = bp.tile([128, DG * C], BF16, tag="dbx")
            h = hp.tile([128, DG * C], BF16, tag="h")
            b_s = br_b[b][:, t0:t0 + C]
            dl_s = dl_b[b][:, t0:t0 + C]
            y_ps = py.tile([DP, C], F32, tag="y")
            for g in range(DG):
                # dBx = u_rep * B  (gpsimd; overlaps the DVE scans)
                eng = nc.gpsimd if g < DBX_GP else nc.vector
                eng.tensor_tensor(dbx[:, g * C:(g + 1) * C],
                                  urep[:, g * C:(g + 1) * C], b_s, op=OP.mult)
                # delta broadcast -> PSUM
                drep_ps = pd.tile([128, C], F32, tag="d")
                nc.tensor.matmul(drep_ps[:], p_sel[:, g * 128:(g + 1) * 128],
                                 dl_s, start=True, stop=True)
                # dA = exp(A * delta) -> PSUM (scan in0 reads PSUM)
                dA_ps = pa.tile([128, C], F32, tag="a")
                nc.scalar.activation(dA_ps[:], drep_ps[:], ACTF.Exp,
                                     scale=a_pack[:, g:g + 1])
                hs = h[:, g * C:(g + 1) * C]
                init = 0.0 if ch == 0 else hcar_b[b][:, g:g + 1]
                nc.vector.tensor_tensor_scan(hs, dA_ps[:],
                                             dbx[:, g * C:(g + 1) * C], init,
                                             op0=OP.mult, op1=OP.add)
                hC = sp.tile([128, C], BF16, tag="hC")
                heng = nc.gpsimd if g < HC_GP else nc.vector
                heng.tensor_tensor(hC[:], hs, c_ps[:], op=OP.mult)
                nc.tensor.matmul(y_ps[:], ed_sel[:, g * DP:(g + 1) * DP],
                                 hC[:], start=(g == 0), stop=(g == DG - 1))
            if ch < NCH - 1:
                nc.vector.tensor_copy(
                    hcar_b[b][:].rearrange("p (g c) -> p g c", c=1),
                    h[:].rearrange("p (g c) -> p g c", g=DG)[:, :, C - 1:C])

            # ---- gate + head ----
            y1 = yp.tile([DP, C], BF16, tag="y1")
            nc.vector.scalar_tensor_tensor(y1[:], xc_s, d_col[:], y_ps[:],
                                           op0=OP.mult, op1=OP.add)
            yg = yp.tile([DP, C], BF16, tag="yg")
            nc.vector.tensor_tensor(yg[:], y1[:], zs_b[b][:, t0:t0 + C],
                                    op=OP.mult)
            g_ps = pf.tile([HID, C], F32, tag="f")
            nc.tensor.matmul(g_ps[:], w1T[:], yg[:], start=True, stop=True)
            g_sb = sp.tile([HID, C], BF16, tag="g")
            nc.scalar.activation(g_sb[:], g_ps[:], ACTF.Relu, bias=b_c1[:])
            lg_ps = pf.tile([128, Q * NL], F32, tag="f")
            for q in range(Q):
                nc.tensor.matmul(lg_ps[:, q * NL:(q + 1) * NL],
                                 g_sb[:, q * 128:(q + 1) * 128], w2T[:],
                                 start=True, stop=True)
            out_sb = sp.tile([128, Q * NL], F32, tag="o")
            nc.vector.tensor_copy(out_sb[:], lg_ps[:])
            dst = out_d[b, t0:t0 + C, :].rearrange("(q p) c -> p q c", p=128)
            nc.sync.dma_start(dst,
                              out_sb[:].rearrange("p (q c) -> p q c", q=Q))

    nc.compile()
    return nc


def _packed_consts(A):
    p_sel = np.zeros((DP, DG * 128), np.float32)
    ed = np.zeros((128, DG * DP), np.float32)
    qb = np.zeros((2 * N, 128), np.float32)
    qc = np.zeros((2 * N, 128), np.float32)
    a_pack = np.zeros((128, DG), np.float32)
    for n in range(N):
        for ds in range(8):
            r = n * 8 + ds
            qb[n, r] = 1.0
            qc[N + n, r] = 1.0
            for g in range(DG):
                d = g * 8 + ds
                if d < DIN:
                    p_sel[d, g * 128 + r] = 1.0
                    ed[r, g * DP + d] = 1.0
                    a_pack[r, g] = A[d, n]
    bf = ml_dtypes.bfloat16
    return {"p_sel": p_sel.astype(bf), "ed_sel": ed.astype(bf),
            "qb_sel": qb.astype(bf), "qc_sel": qc.astype(bf),
            "a_pack": a_pack}


def _prep_inputs(inputs):
    bf = ml_dtypes.bfloat16
    x = np.asarray(inputs["x"], np.float32)
    W_in = np.asarray(inputs["W_in"], np.float64)
    conv_w = np.asarray(inputs["conv_w"], np.float64)
    conv_b = np.asarray(inputs["conv_b"], np.float64)
    W_xproj = np.asarray(inputs["W_xproj"], np.float64)
    W_dt = np.asarray(inputs["W_dt"], np.float64)
    b_dt = np.asarray(inputs["b_dt"], np.float64)
    A_log = np.asarray(inputs["A_log"], np.float64)
    D = np.asarray(inputs["D"], np.float64)
    W_out = np.asarray(inputs["W_out"], np.float64)
    W_c1 = np.asarray(inputs["W_c1"], np.float64)
    b_c1 = np.asarray(inputs["b_c1"], np.float64)
    W_c2 = np.asarray(inputs["W_c2"], np.float64)

    def padrc(a, rows, cols):
        out = np.zeros((rows, cols), np.float64)
        out[:a.shape[0], :a.shape[1]] = a
        return out

    # x: pad d_model 41->48, cast bf16, transpose to [b, d, t]
    xp = np.zeros((B, L, DMP), np.float32)
    xp[:, :, :DM] = x
    xT = np.ascontiguousarray(xp.transpose(0, 2, 1)).astype(bf)

    w_inT = np.zeros((DMP, 2 * DP), np.float64)
    w_inT[:DM, 0:DIN] = W_in[:DIN].T
    w_inT[:DM, DP:DP + DIN] = W_in[DIN:].T

    f32c = lambda a: np.ascontiguousarray(a, dtype=np.float32)
    bfc = lambda a: np.ascontiguousarray(a.astype(np.float32)).astype(bf)
    shared = {
        "w_inT": bfc(w_inT),
        "conv_w": f32c(padrc(conv_w, DP, K)),
        "conv_b": f32c(padrc(conv_b[:, None], DP, 1)),
        "w_effT": bfc(padrc((W_dt @ W_xproj[:DTR]).T, DP, DP)),
        "w_bcT": bfc(padrc(W_xproj[DTR:].T, DP, 2 * N)),
        "b_dt": f32c(padrc(b_dt[:, None], DP, 1)),
        "d_col": f32c(padrc(D[:, None], DP, 1)),
        "w1T": bfc(padrc((W_c1 @ W_out).T, DP, HID)),
        "b_c1": f32c(b_c1[:, None]),
        "w2T": bfc(W_c2.T),
        **_packed_consts((-np.exp(A_log)).astype(np.float32)),
    }
    in_maps = []
    for c in range(NCORES):
        m = dict(shared)
        m["xT"] = xT[c * BLOC:(c + 1) * BLOC]
        in_maps.append(m)
    return in_maps


def kernel(**inputs):
    return _run(inputs, trace=False)[0]


def kernel_traced(**inputs):
    return _run(inputs, trace=True)


def _run(inputs, trace=False):
    key = "nc"
    if key not in _cache:
        _cache[key] = _build({})
    nc = _cache[key]
    in_maps = _prep_inputs(inputs)
    res = run_bass_kernel_spmd(nc, in_maps, core_ids=list(range(NCORES)),
                               trace=trace)
    b_c2 = np.asarray(inputs["b_c2"], np.float32)
    out = np.concatenate([r["out"] for r in res.results], axis=0)
    out = out + b_c2[None, None, :]
    return out, res


# revision 19
# speedup vs baseline: 1.3975x; 1.0168x over previous
"""Trainium2 Bass kernel for nn_Network_61658550501610 (Mamba block + MLP head).

Reference computation (per batch element b, sequence length L=2048):
  xz = x @ W_in.T; xi, z = split(xz)
  xc = silu(causal_depthwise_conv(xi, conv_w) + conv_b)
  x_dbl = xc @ W_xproj.T -> (dt, B, C)
  delta = softplus(dt @ W_dt.T + b_dt)
  h_t = exp(delta*A)*h_{t-1} + delta*B*xc   (selective scan, state [82,16])
  y = (h @ C) + D*xc; y *= silu(z)
  out = y @ W_out.T;  logits = relu(out@W_c1.T+b_c1)@W_c2.T + b_c2

Sharding: data-parallel over batch (B=16 -> 2 per core across 8 cores).

Engine assignment (v2 redesign):
  - x is pre-transposed and bf16-cast on the host -> no on-chip transposes.
  - Activation-table discipline: phase A0 uses the silu set, phases A1+B use
    the natural_log_exp set -> 2 table loads total instead of ~16.
  - The packed scan layout (rows = (n, dsub), 11 groups of 8 d's) as before,
    but: delta-broadcast on TensorE -> dA=exp on Act straight into PSUM; the
    u-broadcast is done by SBUF->SBUF DMA replication; dBx = u*B runs on
    GPSIMD (SBUF-only engine) so it can overlap the DVE scans, whose operands
    are PSUM(dA) + SBUF(dBx) and thus leave the shared DVE/GPSIMD SBUF port
    free.
"""
import ml_dtypes
import numpy as np

import bass_rust as _bass_rust
import concourse.bacc as bacc
import concourse.tile as tile
import concourse.mybir as mybir
from concourse.bass_utils import run_bass_kernel_spmd
from concourse.hw_specs import get_activation_tables


class _Bacc(bacc.Bacc):
    """Bacc with activation-table preference for the combined ln+exp set.

    The stock table-placement pass picks the first set containing each
    function, which sends Ln to ``natural_log`` and Exp to
    ``exp_and_others`` — reloading tables twice per iteration. Putting the
    combined set first makes both resolve to one resident table."""

    _KEEP = ("natural_log_exp_and_others", "silu_and_others")

    def insert_act_table_loads(self):
        has_activation = any(
            isinstance(i, mybir.InstActivation)
            for b in self.main_func.blocks
            for i in b.instructions
        )
        if not has_activation:
            return
        # Keep list order (set ids are positional) but blank out every set
        # except the two we want, so all activations resolve to them.
        tables = [(name, fns if name in self._KEEP else set())
                  for name, fns in get_activation_tables(self.m.arch).items()]
        _bass_rust.insert_act_table_loads(self, tables)

F32 = mybir.dt.float32
BF16 = mybir.dt.bfloat16
OP = mybir.AluOpType
ACTF = mybir.ActivationFunctionType

# problem dims (hardcoded per contract)
B, L, DM = 16, 2048, 41
DIN, N, K = 82, 16, 4          # d_inner, d_state, d_conv
DTR, HID, NL = 3, 64, 10
NCORES = 8
BLOC = B // NCORES             # batch per core

DMP = 48                       # padded d_model
DP = 88                        # padded d_inner
DG = 11                        # d-groups of 8 for the packed scan
C = 512                        # time-chunk length
NCH = L // C                   # chunks per batch element
Q = C // 128                   # 128-row subtiles per chunk

# tuning knobs
DBX_GP = 11                    # how many of the 11 dBx groups run on gpsimd
HC_GP = 0                      # how many of the 11 hC groups run on gpsimd

_cache = {}


def _build(cfg):
    nc = _Bacc("TRN2", target_bir_lowering=False, debug=False,
               enable_asserts=False)

    def din(name, shape, dt=F32):
        return nc.dram_tensor(name, list(shape), dt, kind="ExternalInput").ap()

    xT_d = din("xT", (BLOC, DMP, L), BF16)
    w_inT_d = din("w_inT", (DMP, 2 * DP), BF16)
    conv_w_d = din("conv_w", (DP, K))
    conv_b_d = din("conv_b", (DP, 1))
    w_effT_d = din("w_effT", (DP, DP), BF16)
    w_bcT_d = din("w_bcT", (DP, 2 * N), BF16)
    b_dt_d = din("b_dt", (DP, 1))
    d_col_d = din("d_col", (DP, 1))
    w1T_d = din("w1T", (DP, HID), BF16)
    b_c1_d = din("b_c1", (HID, 1))
    w2T_d = din("w2T", (HID, NL), BF16)
    p_sel_d = din("p_sel", (DP, DG * 128), BF16)
    ed_sel_d = din("ed_sel", (128, DG * DP), BF16)
    qb_sel_d = din("qb_sel", (2 * N, 128), BF16)
    qc_sel_d = din("qc_sel", (2 * N, 128), BF16)
    a_pack_d = din("a_pack", (128, DG))
    out_d = nc.dram_tensor("out", [BLOC, L, NL], F32, kind="ExternalOutput").ap()
    u_scr = nc.dram_tensor("u_scr", [BLOC, DP, L], BF16, kind="Internal").ap()

    with tile.TileContext(nc) as tc, tc.tile_pool(name="wts", bufs=1) as wp, \
         tc.tile_pool(name="xtp", bufs=3) as xp, \
         tc.tile_pool(name="rep", bufs=3) as rp, \
         tc.tile_pool(name="dbx", bufs=2) as bp, \
         tc.tile_pool(name="hbf", bufs=2) as hp, \
         tc.tile_pool(name="sml", bufs=4) as sp, \
         tc.tile_pool(name="yws", bufs=2) as yp, \
         tc.tile_pool(name="ps_f", bufs=3, space="PSUM") as pf, \
         tc.tile_pool(name="ps_d", bufs=1, space="PSUM") as pd, \
         tc.tile_pool(name="ps_a", bufs=2, space="PSUM") as pa, \
         tc.tile_pool(name="ps_c", bufs=1, space="PSUM") as pc, \
         tc.tile_pool(name="ps_y", bufs=1, space="PSUM") as py:

        # ---- constant weights ----
        w_inT = wp.tile([DMP, 2 * DP], BF16)
        conv_w = wp.tile([DP, K], F32)
        conv_b = wp.tile([DP, 1], F32)
        w_effT = wp.tile([DP, DP], BF16)
        w_bcT = wp.tile([DP, 2 * N], BF16)
        b_dt = wp.tile([DP, 1], F32)
        d_col = wp.tile([DP, 1], F32)
        w1T = wp.tile([DP, HID], BF16)
        b_c1 = wp.tile([HID, 1], F32)
        w2T = wp.tile([HID, NL], BF16)
        p_sel = wp.tile([DP, DG * 128], BF16)
        ed_sel = wp.tile([128, DG * DP], BF16)
        qb_sel = wp.tile([2 * N, 128], BF16)
        qc_sel = wp.tile([2 * N, 128], BF16)
        a_pack = wp.tile([128, DG], F32)
        for t_, d_ in [(w_inT, w_inT_d), (conv_w, conv_w_d),
                       (conv_b, conv_b_d), (w_effT, w_effT_d),
                       (w_bcT, w_bcT_d), (b_dt, b_dt_d), (d_col, d_col_d),
                       (w1T, w1T_d), (b_c1, b_c1_d), (w2T, w2T_d),
                       (p_sel, p_sel_d), (ed_sel, ed_sel_d),
                       (qb_sel, qb_sel_d), (qc_sel, qc_sel_d),
                       (a_pack, a_pack_d)]:
            nc.sync.dma_start(t_[:], d_[:])

        # ---- per-batch-element persistent state ----
        xi_b = [wp.tile([DP, K - 1 + L], BF16, name=f"xi{i}", tag=f"xi{i}")
                for i in range(BLOC)]
        xc_b = [wp.tile([DP, L], BF16, name=f"xc{i}", tag=f"xc{i}")
                for i in range(BLOC)]
        zs_b = [wp.tile([DP, L], BF16, name=f"zs{i}", tag=f"zs{i}")
                for i in range(BLOC)]
        dl_b = [wp.tile([DP, L], BF16, name=f"dl{i}", tag=f"dl{i}")
                for i in range(BLOC)]
        u_b = [wp.tile([DP, L], BF16, name=f"u{i}", tag=f"u{i}")
               for i in range(BLOC)]
        br_b = [wp.tile([128, L], BF16, name=f"br{i}", tag=f"br{i}")
                for i in range(BLOC)]
        hcar_b = [wp.tile([128, DG], F32, name=f"hc{i}", tag=f"hc{i}")
                  for i in range(BLOC)]
        for t_ in xi_b:
            nc.vector.memset(t_[:, 0:K - 1], 0.0)

        iters = [(ch, b) for ch in range(NCH) for b in range(BLOC)]

        # ======== phase A0: input proj + conv + silus (silu table) ========
        for ch, b in iters:
            t0 = ch * C
            xT = xp.tile([DMP, C], BF16)
            nc.sync.dma_start(xT[:], xT_d[b, :, t0:t0 + C])

            xi_ps = pf.tile([DP, C], F32, tag="f")
            z_ps = pf.tile([DP, C], F32, tag="f")
            nc.tensor.matmul(xi_ps[:], w_inT[:, 0:DP], xT[:],
                             start=True, stop=True)
            nc.tensor.matmul(z_ps[:], w_inT[:, DP:2 * DP], xT[:],
                             start=True, stop=True)
            # z gate: silu in one op
            nc.scalar.activation(zs_b[b][:, t0:t0 + C], z_ps[:], ACTF.Silu)
            # xi to SBUF (Act copy; DVE is busy with the conv FIR)
            nc.scalar.copy(xi_b[b][:, K - 1 + t0:K - 1 + t0 + C], xi_ps[:])
            # causal depthwise conv: 4-tap FIR on DVE (shifted reads)
            xi = xi_b[b]
            acc = sp.tile([DP, C], F32, tag="cv")
            nc.vector.tensor_scalar(acc[:], xi[:, t0:t0 + C],
                                    conv_w[:, 0:1], None, op0=OP.mult)
            for k in range(1, K):
                nxt = sp.tile([DP, C], F32, tag="cv")
                nc.vector.scalar_tensor_tensor(nxt[:], xi[:, t0 + k:t0 + k + C],
                                               conv_w[:, k:k + 1], acc[:],
                                               op0=OP.mult, op1=OP.add)
                acc = nxt
            nc.scalar.activation(xc_b[b][:, t0:t0 + C], acc[:], ACTF.Silu,
                                 bias=conv_b[:])

        # ======== unified phase A1+B (natural_log_exp table) ========
        # Per outer iteration i: first the x_proj/softplus work for i+1
        # (pipelined one ahead), then the u-replication DMAs for i+1, then
        # the packed scan + head for i. The u replication goes through a
        # DRAM round-trip (DRAM APs are linear -> the partition-crossing
        # read is legal); the read-triggers sit on the sync queue before
        # out-DMA(i) so transfers overlap iteration i's compute.
        def a1(j):
            chj, bj = iters[j]
            tj = chj * C
            xc_j = xc_b[bj][:, tj:tj + C]
            dpre_ps = pf.tile([DP, C], F32, tag="f")
            nc.tensor.matmul(dpre_ps[:], w_effT[:], xc_j, start=True,
                             stop=True)
            esp_sb = sp.tile([DP, C], BF16, tag="es")
            nc.scalar.activation(esp_sb[:], dpre_ps[:], ACTF.Exp, bias=b_dt[:])
            nc.scalar.activation(dl_b[bj][:, tj:tj + C], esp_sb[:], ACTF.Ln,
                                 bias=1.0)
            bc_ps = pf.tile([2 * N, C], F32, tag="f")
            nc.tensor.matmul(bc_ps[:], w_bcT[:], xc_j, start=True, stop=True)
            bc_sb = sp.tile([2 * N, C], BF16, tag="bc")
            nc.scalar.copy(bc_sb[:], bc_ps[:])
            bq_ps = pd.tile([128, C], F32, tag="d")
            nc.tensor.matmul(bq_ps[:], qb_sel[:], bc_sb[:], start=True,
                             stop=True)
            nc.scalar.copy(br_b[bj][:, tj:tj + C], bq_ps[:])
            # u = delta * xc -> DRAM scratch (write on the Act HWDGE queue)
            nc.vector.tensor_tensor(u_b[bj][:, tj:tj + C],
                                    dl_b[bj][:, tj:tj + C], xc_j, op=OP.mult)
            nc.scalar.dma_start(u_scr[bj, :, tj:tj + C], u_b[bj][:, tj:tj + C])
            return bc_sb

        def issue_urep(i):
            chi, bi = iters[i]
            ti = chi * C
            ur = rp.tile([128, DG * C], BF16, tag="ur")
            usrc = u_scr[bi, :, ti:ti + C].rearrange("(g p) t -> p g t", p=8)
            for n in range(N):
                nc.sync.dma_start(
                    ur[n * 8:(n + 1) * 8, :].rearrange(
                        "p (g t) -> p g t", g=DG), usrc)
            return ur

        bcs = {0: a1(0)}
        ureps = {0: issue_urep(0)}
        for it, (ch, b) in enumerate(iters):
            t0 = ch * C
            xc_s = xc_b[b][:, t0:t0 + C]
            if it + 1 < len(iters):
                bcs[it + 1] = a1(it + 1)
                ureps[it + 1] = issue_urep(it + 1)
            urep = ureps.pop(it)
            bc_sb = bcs.pop(it)

            # C in PSUM for the whole g-loop: hC becomes a single-SBUF-port
            # op, so gpsimd dBx is never locked out by it
            c_ps = pc.tile([128, C], F32, tag="c")
            nc.tensor.matmul(c_ps[:], qc_sel[:], bc_sb[:], start=True,
                             stop=True)

            dbx = bp.tile([128, DG * C], BF16, tag="dbx")
            h = hp.tile([128, DG * C], BF16, tag="h")
            b_s = br_b[b][:, t0:t0 + C]
            dl_s = dl_b[b][:, t0:t0 + C]
            y_ps = py.tile([DP, C], F32, tag="y")
            for g in range(DG):
                # dBx = u_rep * B  (gpsimd; overlaps the DVE scans)
                eng = nc.gpsimd if g < DBX_GP else nc.vector
                eng.tensor_tensor(dbx[:, g * C:(g + 1) * C],
                                  urep[:, g * C:(g + 1) * C], b_s, op=OP.mult)
                # delta broadcast -> PSUM
                drep_ps = pd.tile([128, C], F32, tag="d")
                nc.tensor.matmul(drep_ps[:], p_sel[:, g * 128:(g + 1) * 128],
                                 dl_s, start=True, stop=True)
                # dA = exp(A * delta) -> PSUM (scan in0 reads PSUM)
                dA_ps = pa.tile([128, C], F32, tag="a")
                nc.scalar.activation(dA_ps[:], drep_ps[:], ACTF.Exp,
                                     scale=a_pack[:, g:g + 1])
                hs = h[:, g * C:(g + 1) * C]
                init = 0.0 if ch == 0 else hcar_b[b][:, g:g + 1]
                nc.vector.tensor_tensor_scan(hs, dA_ps[:],
                                             dbx[:, g * C:(g + 1) * C], init,
                                             op0=OP.mult, op1=OP.add)
                hC = sp.tile([128, C], BF16, tag="hC")
                heng = nc.gpsimd if g < HC_GP else nc.vector
                heng.tensor_tensor(hC[:], hs, c_ps[:], op=OP.mult)
                nc.tensor.matmul(y_ps[:], ed_sel[:, g * DP:(g + 1) * DP],
                                 hC[:], start=(g == 0), stop=(g == DG - 1))
            if ch < NCH - 1:
                nc.vector.tensor_copy(
                    hcar_b[b][:].rearrange("p (g c) -> p g c", c=1),
                    h[:].rearrange("p (g c) -> p g c", g=DG)[:, :, C - 1:C])

            # ---- gate + head ----
            y1 = yp.tile([DP, C], BF16, tag="y1")
            nc.vector.scalar_tensor_tensor(y1[:], xc_s, d_col[:], y_ps[:],
                                           op0=OP.mult, op1=OP.add)
            yg = yp.tile([DP, C], BF16, tag="yg")
            nc.vector.tensor_tensor(yg[:], y1[:], zs_b[b][:, t0:t0 + C],
                                    op=OP.mult)
            g_ps = pf.tile([HID, C], F32, tag="f")
            nc.tensor.matmul(g_ps[:], w1T[:], yg[:], start=True, stop=True)
            g_sb = sp.tile([HID, C], BF16, tag="g")
            nc.scalar.activation(g_sb[:], g_ps[:], ACTF.Relu, bias=b_c1[:])
            lg_ps = pf.tile([128, Q * NL], F32, tag="f")
            for q in range(Q):
                nc.tensor.matmul(lg_ps[:, q * NL:(q + 1) * NL],
                                 g_sb[:, q * 128:(q + 1) * 128], w2T[:],
                                 start=True, stop=True)
            out_sb = sp.tile([128, Q * NL], F32, tag="o")
            nc.vector.tensor_copy(out_sb[:], lg_ps[:])
            dst = out_d[b, t0:t0 + C, :].rearrange("(q p) c -> p q c", p=128)
            nc.sync.dma_start(dst,
                              out_sb[:].rearrange("p (q c) -> p q c", q=Q))

    nc.compile()
    return nc


def _packed_consts(A):
    p_sel = np.zeros((DP, DG * 128), np.float32)
    ed = np.zeros((128, DG * DP), np.float32)
    qb = np.zeros((2 * N, 128), np.float32)
    qc = np.zeros((2 * N, 128), np.float32)
    a_pack = np.zeros((128, DG), np.float32)
    for n in range(N):
        for ds in range(8):
            r = n * 8 + ds
            qb[n, r] = 1.0
            qc[N + n, r] = 1.0
            for g in range(DG):
                d = g * 8 + ds
                if d < DIN:
                    p_sel[d, g * 128 + r] = 1.0
                    ed[r, g * DP + d] = 1.0
                    a_pack[r, g] = A[d, n]
    bf = ml_dtypes.bfloat16
    return {"p_sel": p_sel.astype(bf), "ed_sel": ed.astype(bf),
            "qb_sel": qb.astype(bf), "qc_sel": qc.astype(bf),
            "a_pack": a_pack}


def _prep_inputs(inputs):
    bf = ml_dtypes.bfloat16
    x = np.asarray(inputs["x"], np.float32)
    W_in = np.asarray(inputs["W_in"], np.float64)
    conv_w = np.asarray(inputs["conv_w"], np.float64)
    conv_b = np.asarray(inputs["conv_b"], np.float64)
    W_xproj = np.asarray(inputs["W_xproj"], np.float64)
    W_dt = np.asarray(inputs["W_dt"], np.float64)
    b_dt = np.asarray(inputs["b_dt"], np.float64)
    A_log = np.asarray(inputs["A_log"], np.float64)
    D = np.asarray(inputs["D"], np.float64)
    W_out = np.asarray(inputs["W_out"], np.float64)
    W_c1 = np.asarray(inputs["W_c1"], np.float64)
    b_c1 = np.asarray(inputs["b_c1"], np.float64)
    W_c2 = np.asarray(inputs["W_c2"], np.float64)

    def padrc(a, rows, cols):
        out = np.zeros((rows, cols), np.float64)
        out[:a.shape[0], :a.shape[1]] = a
        return out

    # x: pad d_model 41->48, cast bf16, transpose to [b, d, t]
    xp = np.zeros((B, L, DMP), np.float32)
    xp[:, :, :DM] = x
    xT = np.ascontiguousarray(xp.transpose(0, 2, 1)).astype(bf)

    w_inT = np.zeros((DMP, 2 * DP), np.float64)
    w_inT[:DM, 0:DIN] = W_in[:DIN].T
    w_inT[:DM, DP:DP + DIN] = W_in[DIN:].T

    f32c = lambda a: np.ascontiguousarray(a, dtype=np.float32)
    bfc = lambda a: np.ascontiguousarray(a.astype(np.float32)).astype(bf)
    shared = {
        "w_inT": bfc(w_inT),
        "conv_w": f32c(padrc(conv_w, DP, K)),
        "conv_b": f32c(padrc(conv_b[:, None], DP, 1)),
        "w_effT": bfc(padrc((W_dt @ W_xproj[:DTR]).T, DP, DP)),
        "w_bcT": bfc(padrc(W_xproj[DTR:].T, DP, 2 * N)),
        "b_dt": f32c(padrc(b_dt[:, None], DP, 1)),
        "d_col": f32c(padrc(D[:, None], DP, 1)),
        "w1T": bfc(padrc((W_c1 @ W_out).T, DP, HID)),
        "b_c1": f32c(b_c1[:, None]),
        "w2T": bfc(W_c2.T),
        **_packed_consts((-np.exp(A_log)).astype(np.float32)),
    }
    in_maps = []
    for c in range(NCORES):
        m = dict(shared)
        m["xT"] = xT[c * BLOC:(c + 1) * BLOC]
        in_maps.append(m)
    return in_maps


def kernel(**inputs):
    return _run(inputs, trace=False)[0]


def kernel_traced(**inputs):
    return _run(inputs, trace=True)


def _run(inputs, trace=False):
    key = "nc"
    if key not in _cache:
        _cache[key] = _build({})
    nc = _cache[key]
    in_maps = _prep_inputs(inputs)
    res = run_bass_kernel_spmd(nc, in_maps, core_ids=list(range(NCORES)),
                               trace=trace)
    b_c2 = np.asarray(inputs["b_c2"], np.float32)
    out = np.concatenate([r["out"] for r in res.results], axis=0)
    out = out + b_c2[None, None, :]
    return out, res


# revision 20
# speedup vs baseline: 1.4513x; 1.0385x over previous
"""Trainium2 Bass kernel for nn_Network_61658550501610 (Mamba block + MLP head).

Reference computation (per batch element b, sequence length L=2048):
  xz = x @ W_in.T; xi, z = split(xz)
  xc = silu(causal_depthwise_conv(xi, conv_w) + conv_b)
  x_dbl = xc @ W_xproj.T -> (dt, B, C)
  delta = softplus(dt @ W_dt.T + b_dt)
  h_t = exp(delta*A)*h_{t-1} + delta*B*xc   (selective scan, state [82,16])
  y = (h @ C) + D*xc; y *= silu(z)
  out = y @ W_out.T;  logits = relu(out@W_c1.T+b_c1)@W_c2.T + b_c2

Sharding: data-parallel over batch (B=16 -> 2 per core across 8 cores).

Engine assignment (v2 redesign):
  - x is pre-transposed and bf16-cast on the host -> no on-chip transposes.
  - Activation-table discipline: phase A0 uses the silu set, phases A1+B use
    the natural_log_exp set -> 2 table loads total instead of ~16.
  - The packed scan layout (rows = (n, dsub), 11 groups of 8 d's) as before,
    but: delta-broadcast on TensorE -> dA=exp on Act straight into PSUM; the
    u-broadcast is done by SBUF->SBUF DMA replication; dBx = u*B runs on
    GPSIMD (SBUF-only engine) so it can overlap the DVE scans, whose operands
    are PSUM(dA) + SBUF(dBx) and thus leave the shared DVE/GPSIMD SBUF port
    free.
"""
import ml_dtypes
import numpy as np

import bass_rust as _bass_rust
import concourse.bacc as bacc
import concourse.tile as tile
import concourse.mybir as mybir
from concourse.bass_utils import run_bass_kernel_spmd
from concourse.hw_specs import get_activation_tables


class _Bacc(bacc.Bacc):
    """Bacc with activation-table preference for the combined ln+exp set.

    The stock table-placement pass picks the first set containing each
    function, which sends Ln to ``natural_log`` and Exp to
    ``exp_and_others`` — reloading tables twice per iteration. Putting the
    combined set first makes both resolve to one resident table."""

    _KEEP = ("natural_log_exp_and_others", "silu_and_others")

    def insert_act_table_loads(self):
        has_activation = any(
            isinstance(i, mybir.InstActivation)
            for b in self.main_func.blocks
            for i in b.instructions
        )
        if not has_activation:
            return
        # Keep list order (set ids are positional) but blank out every set
        # except the two we want, so all activations resolve to them.
        tables = [(name, fns if name in self._KEEP else set())
                  for name, fns in get_activation_tables(self.m.arch).items()]
        _bass_rust.insert_act_table_loads(self, tables)

F32 = mybir.dt.float32
BF16 = mybir.dt.bfloat16
OP = mybir.AluOpType
ACTF = mybir.ActivationFunctionType

# problem dims (hardcoded per contract)
B, L, DM = 16, 2048, 41
DIN, N, K = 82, 16, 4          # d_inner, d_state, d_conv
DTR, HID, NL = 3, 64, 10
NCORES = 8
BLOC = B // NCORES             # batch per core

DMP = 48                       # padded d_model
DP = 88                        # padded d_inner
DG = 11                        # d-groups of 8 for the packed scan
C = 512                        # time-chunk length
NCH = L // C                   # chunks per batch element
Q = C // 128                   # 128-row subtiles per chunk

# tuning knobs
DBX_GP = 11                    # how many of the 11 dBx groups run on gpsimd
HC_GP = 0                      # how many of the 11 hC groups run on gpsimd

_cache = {}


def _build(cfg):
    nc = _Bacc("TRN2", target_bir_lowering=False, debug=False,
               enable_asserts=False)

    def din(name, shape, dt=F32):
        return nc.dram_tensor(name, list(shape), dt, kind="ExternalInput").ap()

    xT_d = din("xT", (BLOC, DMP, L), BF16)
    w_inT_d = din("w_inT", (DMP, 2 * DP), BF16)
    conv_w_d = din("conv_w", (DP, K))
    conv_b_d = din("conv_b", (DP, 1))
    w_effT_d = din("w_effT", (DP, DP), BF16)
    w_bcT_d = din("w_bcT", (DP, 2 * N), BF16)
    b_dt_d = din("b_dt", (DP, 1))
    d_col_d = din("d_col", (DP, 1))
    w1T_d = din("w1T", (DP, HID), BF16)
    b_c1_d = din("b_c1", (HID, 1))
    w2T_d = din("w2T", (HID, NL), BF16)
    p_sel_d = din("p_sel", (DP, DG * 128), BF16)
    ed_sel_d = din("ed_sel", (128, DG * DP), BF16)
    qb_sel_d = din("qb_sel", (2 * N, 128), BF16)
    qc_sel_d = din("qc_sel", (2 * N, 128), BF16)
    a_pack_d = din("a_pack", (128, DG))
    out_d = nc.dram_tensor("out", [BLOC, L, NL], F32, kind="ExternalOutput").ap()
    u_scr = nc.dram_tensor("u_scr", [BLOC, DP, L], BF16, kind="Internal").ap()

    with tile.TileContext(nc) as tc, tc.tile_pool(name="wts", bufs=1) as wp, \
         tc.tile_pool(name="xtp", bufs=3) as xp, \
         tc.tile_pool(name="rep", bufs=3) as rp, \
         tc.tile_pool(name="dbx", bufs=2) as bp, \
         tc.tile_pool(name="hbf", bufs=2) as hp, \
         tc.tile_pool(name="sml", bufs=4) as sp, \
         tc.tile_pool(name="yws", bufs=2) as yp, \
         tc.tile_pool(name="ps_f", bufs=3, space="PSUM") as pf, \
         tc.tile_pool(name="ps_d", bufs=1, space="PSUM") as pd, \
         tc.tile_pool(name="ps_a", bufs=2, space="PSUM") as pa, \
         tc.tile_pool(name="ps_c", bufs=1, space="PSUM") as pc, \
         tc.tile_pool(name="ps_y", bufs=1, space="PSUM") as py:

        # ---- constant weights ----
        w_inT = wp.tile([DMP, 2 * DP], BF16)
        conv_w = wp.tile([DP, K], F32)
        conv_b = wp.tile([DP, 1], F32)
        w_effT = wp.tile([DP, DP], BF16)
        w_bcT = wp.tile([DP, 2 * N], BF16)
        b_dt = wp.tile([DP, 1], F32)
        d_col = wp.tile([DP, 1], F32)
        w1T = wp.tile([DP, HID], BF16)
        b_c1 = wp.tile([HID, 1], F32)
        w2T = wp.tile([HID, NL], BF16)
        p_sel = wp.tile([DP, DG * 128], BF16)
        ed_sel = wp.tile([128, DG * DP], BF16)
        qb_sel = wp.tile([2 * N, 128], BF16)
        qc_sel = wp.tile([2 * N, 128], BF16)
        a_pack = wp.tile([128, DG], F32)
        for t_, d_ in [(w_inT, w_inT_d), (conv_w, conv_w_d),
                       (conv_b, conv_b_d), (w_effT, w_effT_d),
                       (w_bcT, w_bcT_d), (b_dt, b_dt_d), (d_col, d_col_d),
                       (w1T, w1T_d), (b_c1, b_c1_d), (w2T, w2T_d),
                       (p_sel, p_sel_d), (ed_sel, ed_sel_d),
                       (qb_sel, qb_sel_d), (qc_sel, qc_sel_d),
                       (a_pack, a_pack_d)]:
            nc.sync.dma_start(t_[:], d_[:])

        # ---- per-batch-element persistent state ----
        xi_b = [wp.tile([DP, K - 1 + L], BF16, name=f"xi{i}", tag=f"xi{i}")
                for i in range(BLOC)]
        xc_b = [wp.tile([DP, L], BF16, name=f"xc{i}", tag=f"xc{i}")
                for i in range(BLOC)]
        zs_b = [wp.tile([DP, L], BF16, name=f"zs{i}", tag=f"zs{i}")
                for i in range(BLOC)]
        dl_b = [wp.tile([DP, L], BF16, name=f"dl{i}", tag=f"dl{i}")
                for i in range(BLOC)]
        u_b = [wp.tile([DP, L], BF16, name=f"u{i}", tag=f"u{i}")
               for i in range(BLOC)]
        br_b = [wp.tile([128, L], BF16, name=f"br{i}", tag=f"br{i}")
                for i in range(BLOC)]
        hcar_b = [wp.tile([128, DG], F32, name=f"hc{i}", tag=f"hc{i}")
                  for i in range(BLOC)]
        for t_ in xi_b:
            nc.vector.memset(t_[:, 0:K - 1], 0.0)

        iters = [(ch, b) for ch in range(NCH) for b in range(BLOC)]

        # ======== phase A0: input proj + conv + silus (silu table) ========
        for ch, b in iters:
            t0 = ch * C
            xT = xp.tile([DMP, C], BF16)
            nc.sync.dma_start(xT[:], xT_d[b, :, t0:t0 + C])

            xi_ps = pf.tile([DP, C], F32, tag="f")
            z_ps = pf.tile([DP, C], F32, tag="f")
            nc.tensor.matmul(xi_ps[:], w_inT[:, 0:DP], xT[:],
                             start=True, stop=True)
            nc.tensor.matmul(z_ps[:], w_inT[:, DP:2 * DP], xT[:],
                             start=True, stop=True)
            # z gate: silu in one op
            nc.scalar.activation(zs_b[b][:, t0:t0 + C], z_ps[:], ACTF.Silu)
            # xi to SBUF (Act copy; DVE is busy with the conv FIR)
            nc.scalar.copy(xi_b[b][:, K - 1 + t0:K - 1 + t0 + C], xi_ps[:])
            # causal depthwise conv: 4-tap FIR on DVE (shifted reads)
            xi = xi_b[b]
            acc = sp.tile([DP, C], F32, tag="cv")
            nc.vector.tensor_scalar(acc[:], xi[:, t0:t0 + C],
                                    conv_w[:, 0:1], None, op0=OP.mult)
            for k in range(1, K):
                nxt = sp.tile([DP, C], F32, tag="cv")
                nc.vector.scalar_tensor_tensor(nxt[:], xi[:, t0 + k:t0 + k + C],
                                               conv_w[:, k:k + 1], acc[:],
                                               op0=OP.mult, op1=OP.add)
                acc = nxt
            nc.scalar.activation(xc_b[b][:, t0:t0 + C], acc[:], ACTF.Silu,
                                 bias=conv_b[:])

        # ======== unified phase A1+B (natural_log_exp table) ========
        # Per outer iteration i: first the x_proj/softplus work for i+1
        # (pipelined one ahead), then the u-replication DMAs for i+1, then
        # the packed scan + head for i. The u replication goes through a
        # DRAM round-trip (DRAM APs are linear -> the partition-crossing
        # read is legal); the read-triggers sit on the sync queue before
        # out-DMA(i) so transfers overlap iteration i's compute.
        def a1(j):
            chj, bj = iters[j]
            tj = chj * C
            xc_j = xc_b[bj][:, tj:tj + C]
            dpre_ps = pf.tile([DP, C], F32, tag="f")
            nc.tensor.matmul(dpre_ps[:], w_effT[:], xc_j, start=True,
                             stop=True)
            esp_sb = sp.tile([DP, C], BF16, tag="es")
            nc.scalar.activation(esp_sb[:], dpre_ps[:], ACTF.Exp, bias=b_dt[:])
            nc.scalar.activation(dl_b[bj][:, tj:tj + C], esp_sb[:], ACTF.Ln,
                                 bias=1.0)
            bc_ps = pf.tile([2 * N, C], F32, tag="f")
            nc.tensor.matmul(bc_ps[:], w_bcT[:], xc_j, start=True, stop=True)
            bc_sb = sp.tile([2 * N, C], BF16, tag="bc")
            nc.scalar.copy(bc_sb[:], bc_ps[:])
            bq_ps = pd.tile([128, C], F32, tag="d")
            nc.tensor.matmul(bq_ps[:], qb_sel[:], bc_sb[:], start=True,
                             stop=True)
            nc.scalar.copy(br_b[bj][:, tj:tj + C], bq_ps[:])
            # u = delta * xc -> DRAM scratch (write on the Act HWDGE queue)
            nc.vector.tensor_tensor(u_b[bj][:, tj:tj + C],
                                    dl_b[bj][:, tj:tj + C], xc_j, op=OP.mult)
            nc.scalar.dma_start(u_scr[bj, :, tj:tj + C], u_b[bj][:, tj:tj + C])
            return bc_sb

        def issue_urep(i):
            chi, bi = iters[i]
            ti = chi * C
            ur = rp.tile([128, DG * C], BF16, tag="ur")
            usrc = u_scr[bi, :, ti:ti + C].rearrange("(g p) t -> p g t", p=8)
            for n in range(N):
                nc.sync.dma_start(
                    ur[n * 8:(n + 1) * 8, :].rearrange(
                        "p (g t) -> p g t", g=DG), usrc)
            return ur

        bcs = {0: a1(0), 1: a1(1)}
        ureps = {0: issue_urep(0)}
        for it, (ch, b) in enumerate(iters):
            t0 = ch * C
            xc_s = xc_b[b][:, t0:t0 + C]
            if it + 2 < len(iters):
                bcs[it + 2] = a1(it + 2)
            if it + 1 < len(iters):
                ureps[it + 1] = issue_urep(it + 1)
            urep = ureps.pop(it)
            bc_sb = bcs.pop(it)

            # C in PSUM for the whole g-loop: hC becomes a single-SBUF-port
            # op, so gpsimd dBx is never locked out by it
            c_ps = pc.tile([128, C], F32, tag="c")
            nc.tensor.matmul(c_ps[:], qc_sel[:], bc_sb[:], start=True,
                             stop=True)

            dbx = bp.tile([128, DG * C], BF16, tag="dbx")
            h = hp.tile([128, DG * C], BF16, tag="h")
            b_s = br_b[b][:, t0:t0 + C]
            dl_s = dl_b[b][:, t0:t0 + C]
            y_ps = py.tile([DP, C], F32, tag="y")
            for g in range(DG):
                # dBx = u_rep * B  (gpsimd; overlaps the DVE scans)
                eng = nc.gpsimd if g < DBX_GP else nc.vector
                eng.tensor_tensor(dbx[:, g * C:(g + 1) * C],
                                  urep[:, g * C:(g + 1) * C], b_s, op=OP.mult)
                # delta broadcast -> PSUM
                drep_ps = pd.tile([128, C], F32, tag="d")
                nc.tensor.matmul(drep_ps[:], p_sel[:, g * 128:(g + 1) * 128],
                                 dl_s, start=True, stop=True)
                # dA = exp(A * delta) -> PSUM (scan in0 reads PSUM)
                dA_ps = pa.tile([128, C], F32, tag="a")
                nc.scalar.activation(dA_ps[:], drep_ps[:], ACTF.Exp,
                                     scale=a_pack[:, g:g + 1])
                hs = h[:, g * C:(g + 1) * C]
                init = 0.0 if ch == 0 else hcar_b[b][:, g:g + 1]
                nc.vector.tensor_tensor_scan(hs, dA_ps[:],
                                             dbx[:, g * C:(g + 1) * C], init,
                                             op0=OP.mult, op1=OP.add)
                hC = sp.tile([128, C], BF16, tag="hC")
                heng = nc.gpsimd if g < HC_GP else nc.vector
                heng.tensor_tensor(hC[:], hs, c_ps[:], op=OP.mult)
                nc.tensor.matmul(y_ps[:], ed_sel[:, g * DP:(g + 1) * DP],
                                 hC[:], start=(g == 0), stop=(g == DG - 1))
            if ch < NCH - 1:
                nc.vector.tensor_copy(
                    hcar_b[b][:].rearrange("p (g c) -> p g c", c=1),
                    h[:].rearrange("p (g c) -> p g c", g=DG)[:, :, C - 1:C])

            # ---- gate + head ----
            y1 = yp.tile([DP, C], BF16, tag="y1")
            nc.vector.scalar_tensor_tensor(y1[:], xc_s, d_col[:], y_ps[:],
                                           op0=OP.mult, op1=OP.add)
            yg = yp.tile([DP, C], BF16, tag="yg")
            nc.vector.tensor_tensor(yg[:], y1[:], zs_b[b][:, t0:t0 + C],
                                    op=OP.mult)
            g_ps = pf.tile([HID, C], F32, tag="f")
            nc.tensor.matmul(g_ps[:], w1T[:], yg[:], start=True, stop=True)
            g_sb = sp.tile([HID, C], BF16, tag="g")
            nc.scalar.activation(g_sb[:], g_ps[:], ACTF.Relu, bias=b_c1[:])
            lg_ps = pf.tile([128, Q * NL], F32, tag="f")
            for q in range(Q):
                nc.tensor.matmul(lg_ps[:, q * NL:(q + 1) * NL],
                                 g_sb[:, q * 128:(q + 1) * 128], w2T[:],
                                 start=True, stop=True)
            out_sb = sp.tile([128, Q * NL], F32, tag="o")
            nc.vector.tensor_copy(out_sb[:], lg_ps[:])
            dst = out_d[b, t0:t0 + C, :].rearrange("(q p) c -> p q c", p=128)
            nc.sync.dma_start(dst,
                              out_sb[:].rearrange("p (q c) -> p q c", q=Q))

    nc.compile()
    return nc


def _packed_consts(A):
    p_sel = np.zeros((DP, DG * 128), np.float32)
    ed = np.zeros((128, DG * DP), np.float32)
    qb = np.zeros((2 * N, 128), np.float32)
    qc = np.zeros((2 * N, 128), np.float32)
    a_pack = np.zeros((128, DG), np.float32)
    for n in range(N):
        for ds in range(8):
            r = n * 8 + ds
            qb[n, r] = 1.0
            qc[N + n, r] = 1.0
            for g in range(DG):
                d = g * 8 + ds
                if d < DIN:
                    p_sel[d, g * 128 + r] = 1.0
                    ed[r, g * DP + d] = 1.0
                    a_pack[r, g] = A[d, n]
    bf = ml_dtypes.bfloat16
    return {"p_sel": p_sel.astype(bf), "ed_sel": ed.astype(bf),
            "qb_sel": qb.astype(bf), "qc_sel": qc.astype(bf),
            "a_pack": a_pack}


def _prep_inputs(inputs):
    bf = ml_dtypes.bfloat16
    x = np.asarray(inputs["x"], np.float32)
    W_in = np.asarray(inputs["W_in"], np.float64)
    conv_w = np.asarray(inputs["conv_w"], np.float64)
    conv_b = np.asarray(inputs["conv_b"], np.float64)
    W_xproj = np.asarray(inputs["W_xproj"], np.float64)
    W_dt = np.asarray(inputs["W_dt"], np.float64)
    b_dt = np.asarray(inputs["b_dt"], np.float64)
    A_log = np.asarray(inputs["A_log"], np.float64)
    D = np.asarray(inputs["D"], np.float64)
    W_out = np.asarray(inputs["W_out"], np.float64)
    W_c1 = np.asarray(inputs["W_c1"], np.float64)
    b_c1 = np.asarray(inputs["b_c1"], np.float64)
    W_c2 = np.asarray(inputs["W_c2"], np.float64)

    def padrc(a, rows, cols):
        out = np.zeros((rows, cols), np.float64)
        out[:a.shape[0], :a.shape[1]] = a
        return out

    # x: pad d_model 41->48, cast bf16, transpose to [b, d, t]
    xp = np.zeros((B, L, DMP), np.float32)
    xp[:, :, :DM] = x
    xT = np.ascontiguousarray(xp.transpose(0, 2, 1)).astype(bf)

    w_inT = np.zeros((DMP, 2 * DP), np.float64)
    w_inT[:DM, 0:DIN] = W_in[:DIN].T
    w_inT[:DM, DP:DP + DIN] = W_in[DIN:].T

    f32c = lambda a: np.ascontiguousarray(a, dtype=np.float32)
    bfc = lambda a: np.ascontiguousarray(a.astype(np.float32)).astype(bf)
    shared = {
        "w_inT": bfc(w_inT),
        "conv_w": f32c(padrc(conv_w, DP, K)),
        "conv_b": f32c(padrc(conv_b[:, None], DP, 1)),
        "w_effT": bfc(padrc((W_dt @ W_xproj[:DTR]).T, DP, DP)),
        "w_bcT": bfc(padrc(W_xproj[DTR:].T, DP, 2 * N)),
        "b_dt": f32c(padrc(b_dt[:, None], DP, 1)),
        "d_col": f32c(padrc(D[:, None], DP, 1)),
        "w1T": bfc(padrc((W_c1 @ W_out).T, DP, HID)),
        "b_c1": f32c(b_c1[:, None]),
        "w2T": bfc(W_c2.T),
        **_packed_consts((-np.exp(A_log)).astype(np.float32)),
    }
    in_maps = []
    for c in range(NCORES):
        m = dict(shared)
        m["xT"] = xT[c * BLOC:(c + 1) * BLOC]
        in_maps.append(m)
    return in_maps


def kernel(**inputs):
    return _run(inputs, trace=False)[0]


def kernel_traced(**inputs):
    return _run(inputs, trace=True)


def _run(inputs, trace=False):
    key = "nc"
    if key not in _cache:
        _cache[key] = _build({})
    nc = _cache[key]
    in_maps = _prep_inputs(inputs)
    res = run_bass_kernel_spmd(nc, in_maps, core_ids=list(range(NCORES)),
                               trace=trace)
    b_c2 = np.asarray(inputs["b_c2"], np.float32)
    out = np.concatenate([r["out"] for r in res.results], axis=0)
    out = out + b_c2[None, None, :]
    return out, res


# revision 22
# speedup vs baseline: 1.4531x; 1.0012x over previous
"""Trainium2 Bass kernel for nn_Network_61658550501610 (Mamba block + MLP head).

Reference computation (per batch element b, sequence length L=2048):
  xz = x @ W_in.T; xi, z = split(xz)
  xc = silu(causal_depthwise_conv(xi, conv_w) + conv_b)
  x_dbl = xc @ W_xproj.T -> (dt, B, C)
  delta = softplus(dt @ W_dt.T + b_dt)
  h_t = exp(delta*A)*h_{t-1} + delta*B*xc   (selective scan, state [82,16])
  y = (h @ C) + D*xc; y *= silu(z)
  out = y @ W_out.T;  logits = relu(out@W_c1.T+b_c1)@W_c2.T + b_c2

Sharding: data-parallel over batch (B=16 -> 2 per core across 8 cores).

Engine assignment (v2 redesign):
  - x is pre-transposed and bf16-cast on the host -> no on-chip transposes.
  - Activation-table discipline: phase A0 uses the silu set, phases A1+B use
    the natural_log_exp set -> 2 table loads total instead of ~16.
  - The packed scan layout (rows = (n, dsub), 11 groups of 8 d's) as before,
    but: delta-broadcast on TensorE -> dA=exp on Act straight into PSUM; the
    u-broadcast is done by SBUF->SBUF DMA replication; dBx = u*B runs on
    GPSIMD (SBUF-only engine) so it can overlap the DVE scans, whose operands
    are PSUM(dA) + SBUF(dBx) and thus leave the shared DVE/GPSIMD SBUF port
    free.
"""
import ml_dtypes
import numpy as np

import bass_rust as _bass_rust
import concourse.bacc as bacc
import concourse.tile as tile
import concourse.mybir as mybir
from concourse.bass_utils import run_bass_kernel_spmd
from concourse.hw_specs import get_activation_tables


class _Bacc(bacc.Bacc):
    """Bacc with activation-table preference for the combined ln+exp set.

    The stock table-placement pass picks the first set containing each
    function, which sends Ln to ``natural_log`` and Exp to
    ``exp_and_others`` — reloading tables twice per iteration. Putting the
    combined set first makes both resolve to one resident table."""

    _KEEP = ("natural_log_exp_and_others", "silu_and_others")

    def insert_act_table_loads(self):
        has_activation = any(
            isinstance(i, mybir.InstActivation)
            for b in self.main_func.blocks
            for i in b.instructions
        )
        if not has_activation:
            return
        # Keep list order (set ids are positional) but blank out every set
        # except the two we want, so all activations resolve to them.
        tables = [(name, fns if name in self._KEEP else set())
                  for name, fns in get_activation_tables(self.m.arch).items()]
        _bass_rust.insert_act_table_loads(self, tables)

F32 = mybir.dt.float32
BF16 = mybir.dt.bfloat16
OP = mybir.AluOpType
ACTF = mybir.ActivationFunctionType

# problem dims (hardcoded per contract)
B, L, DM = 16, 2048, 41
DIN, N, K = 82, 16, 4          # d_inner, d_state, d_conv
DTR, HID, NL = 3, 64, 10
NCORES = 8
BLOC = B // NCORES             # batch per core

DMP = 48                       # padded d_model
DP = 88                        # padded d_inner
DG = 11                        # d-groups of 8 for the packed scan
C = 512                        # time-chunk length
NCH = L // C                   # chunks per batch element
Q = C // 128                   # 128-row subtiles per chunk

# tuning knobs
DBX_GP = 11                    # how many of the 11 dBx groups run on gpsimd
HC_GP = 0                      # how many of the 11 hC groups run on gpsimd

_cache = {}


def _build(cfg):
    nc = _Bacc("TRN2", target_bir_lowering=False, debug=False,
               enable_asserts=False)

    def din(name, shape, dt=F32):
        return nc.dram_tensor(name, list(shape), dt, kind="ExternalInput").ap()

    xT_d = din("xT", (BLOC, DMP, L), BF16)
    w_inT_d = din("w_inT", (DMP, 2 * DP), BF16)
    conv_w_d = din("conv_w", (DP, K))
    conv_b_d = din("conv_b", (DP, 1))
    w_effT_d = din("w_effT", (DP, DP), BF16)
    w_bcT_d = din("w_bcT", (DP, 2 * N), BF16)
    b_dt_d = din("b_dt", (DP, 1))
    d_col_d = din("d_col", (DP, 1))
    w1T_d = din("w1T", (DP, HID), BF16)
    b_c1_d = din("b_c1", (HID, 1))
    w2T_d = din("w2T", (HID, NL), BF16)
    p_sel_d = din("p_sel", (DP, DG * 128), BF16)
    ed_sel_d = din("ed_sel", (128, DG * DP), BF16)
    qb_sel_d = din("qb_sel", (2 * N, 128), BF16)
    qc_sel_d = din("qc_sel", (2 * N, 128), BF16)
    a_pack_d = din("a_pack", (128, DG))
    out_d = nc.dram_tensor("out", [BLOC, NCH, 128, Q, NL], F32,
                           kind="ExternalOutput").ap()
    u_scr = nc.dram_tensor("u_scr", [BLOC, NCH, 8, DG, C], BF16,
                            kind="Internal").ap()

    with tile.TileContext(nc) as tc, tc.tile_pool(name="wts", bufs=1) as wp, \
         tc.tile_pool(name="xtp", bufs=3) as xp, \
         tc.tile_pool(name="rep", bufs=3) as rp, \
         tc.tile_pool(name="dbx", bufs=2) as bp, \
         tc.tile_pool(name="hbf", bufs=2) as hp, \
         tc.tile_pool(name="sml", bufs=4) as sp, \
         tc.tile_pool(name="yws", bufs=2) as yp, \
         tc.tile_pool(name="ps_f", bufs=3, space="PSUM") as pf, \
         tc.tile_pool(name="ps_d", bufs=1, space="PSUM") as pd, \
         tc.tile_pool(name="ps_a", bufs=2, space="PSUM") as pa, \
         tc.tile_pool(name="ps_c", bufs=1, space="PSUM") as pc, \
         tc.tile_pool(name="ps_y", bufs=1, space="PSUM") as py:

        # ---- constant weights ----
        w_inT = wp.tile([DMP, 2 * DP], BF16)
        conv_w = wp.tile([DP, K], F32)
        conv_b = wp.tile([DP, 1], F32)
        w_effT = wp.tile([DP, DP], BF16)
        w_bcT = wp.tile([DP, 2 * N], BF16)
        b_dt = wp.tile([DP, 1], F32)
        d_col = wp.tile([DP, 1], F32)
        w1T = wp.tile([DP, HID], BF16)
        b_c1 = wp.tile([HID, 1], F32)
        w2T = wp.tile([HID, NL], BF16)
        p_sel = wp.tile([DP, DG * 128], BF16)
        ed_sel = wp.tile([128, DG * DP], BF16)
        qb_sel = wp.tile([2 * N, 128], BF16)
        qc_sel = wp.tile([2 * N, 128], BF16)
        a_pack = wp.tile([128, DG], F32)
        for t_, d_ in [(w_inT, w_inT_d), (conv_w, conv_w_d),
                       (conv_b, conv_b_d), (w_effT, w_effT_d),
                       (w_bcT, w_bcT_d), (b_dt, b_dt_d), (d_col, d_col_d),
                       (w1T, w1T_d), (b_c1, b_c1_d), (w2T, w2T_d),
                       (p_sel, p_sel_d), (ed_sel, ed_sel_d),
                       (qb_sel, qb_sel_d), (qc_sel, qc_sel_d),
                       (a_pack, a_pack_d)]:
            nc.sync.dma_start(t_[:], d_[:])

        # ---- per-batch-element persistent state ----
        xi_b = [wp.tile([DP, K - 1 + L], BF16, name=f"xi{i}", tag=f"xi{i}")
                for i in range(BLOC)]
        xc_b = [wp.tile([DP, L], BF16, name=f"xc{i}", tag=f"xc{i}")
                for i in range(BLOC)]
        zs_b = [wp.tile([DP, L], BF16, name=f"zs{i}", tag=f"zs{i}")
                for i in range(BLOC)]
        dl_b = [wp.tile([DP, L], BF16, name=f"dl{i}", tag=f"dl{i}")
                for i in range(BLOC)]
        u_b = [wp.tile([DP, L], BF16, name=f"u{i}", tag=f"u{i}")
               for i in range(BLOC)]
        br_b = [wp.tile([128, L], BF16, name=f"br{i}", tag=f"br{i}")
                for i in range(BLOC)]
        hcar_b = [wp.tile([128, DG], F32, name=f"hc{i}", tag=f"hc{i}")
                  for i in range(BLOC)]
        for t_ in xi_b:
            nc.vector.memset(t_[:, 0:K - 1], 0.0)

        iters = [(ch, b) for ch in range(NCH) for b in range(BLOC)]

        # ======== phase A0: input proj + conv + silus (silu table) ========
        for ch, b in iters:
            t0 = ch * C
            xT = xp.tile([DMP, C], BF16)
            nc.sync.dma_start(xT[:], xT_d[b, :, t0:t0 + C])

            xi_ps = pf.tile([DP, C], F32, tag="f")
            z_ps = pf.tile([DP, C], F32, tag="f")
            nc.tensor.matmul(xi_ps[:], w_inT[:, 0:DP], xT[:],
                             start=True, stop=True)
            nc.tensor.matmul(z_ps[:], w_inT[:, DP:2 * DP], xT[:],
                             start=True, stop=True)
            # z gate: silu in one op
            nc.scalar.activation(zs_b[b][:, t0:t0 + C], z_ps[:], ACTF.Silu)
            # xi to SBUF (Act copy; DVE is busy with the conv FIR)
            nc.scalar.copy(xi_b[b][:, K - 1 + t0:K - 1 + t0 + C], xi_ps[:])
            # causal depthwise conv: 4-tap FIR on DVE (shifted reads)
            xi = xi_b[b]
            acc = sp.tile([DP, C], F32, tag="cv")
            nc.vector.tensor_scalar(acc[:], xi[:, t0:t0 + C],
                                    conv_w[:, 0:1], None, op0=OP.mult)
            for k in range(1, K):
                nxt = sp.tile([DP, C], F32, tag="cv")
                nc.vector.scalar_tensor_tensor(nxt[:], xi[:, t0 + k:t0 + k + C],
                                               conv_w[:, k:k + 1], acc[:],
                                               op0=OP.mult, op1=OP.add)
                acc = nxt
            nc.scalar.activation(xc_b[b][:, t0:t0 + C], acc[:], ACTF.Silu,
                                 bias=conv_b[:])

        # ======== unified phase A1+B (natural_log_exp table) ========
        # Per outer iteration i: first the x_proj/softplus work for i+1
        # (pipelined one ahead), then the u-replication DMAs for i+1, then
        # the packed scan + head for i. The u replication goes through a
        # DRAM round-trip (DRAM APs are linear -> the partition-crossing
        # read is legal); the read-triggers sit on the sync queue before
        # out-DMA(i) so transfers overlap iteration i's compute.
        def a1(j):
            chj, bj = iters[j]
            tj = chj * C
            xc_j = xc_b[bj][:, tj:tj + C]
            dpre_ps = pf.tile([DP, C], F32, tag="f")
            nc.tensor.matmul(dpre_ps[:], w_effT[:], xc_j, start=True,
                             stop=True)
            esp_sb = sp.tile([DP, C], BF16, tag="es")
            nc.scalar.activation(esp_sb[:], dpre_ps[:], ACTF.Exp, bias=b_dt[:])
            nc.scalar.activation(dl_b[bj][:, tj:tj + C], esp_sb[:], ACTF.Ln,
                                 bias=1.0)
            bc_ps = pf.tile([2 * N, C], F32, tag="f")
            nc.tensor.matmul(bc_ps[:], w_bcT[:], xc_j, start=True, stop=True)
            bc_sb = sp.tile([2 * N, C], BF16, tag="bc")
            nc.scalar.copy(bc_sb[:], bc_ps[:])
            bq_ps = pd.tile([128, C], F32, tag="d")
            nc.tensor.matmul(bq_ps[:], qb_sel[:], bc_sb[:], start=True,
                             stop=True)
            nc.scalar.copy(br_b[bj][:, tj:tj + C], bq_ps[:])
            # u = delta * xc -> DRAM scratch (write on the Act HWDGE queue)
            nc.vector.tensor_tensor(u_b[bj][:, tj:tj + C],
                                    dl_b[bj][:, tj:tj + C], xc_j, op=OP.mult)
            nc.scalar.dma_start(
                u_scr[bj, chj].rearrange("p g t -> g p t"),
                u_b[bj][:, tj:tj + C].rearrange("(g p) t -> g p t", p=8))
            return bc_sb

        def issue_urep(i):
            chi, bi = iters[i]
            ur = rp.tile([128, DG * C], BF16, tag="ur")
            usrc = u_scr[bi, chi]
            for n in range(N):
                nc.sync.dma_start(
                    ur[n * 8:(n + 1) * 8, :].rearrange(
                        "p (g t) -> p g t", g=DG), usrc)
            return ur

        bcs = {0: a1(0), 1: a1(1)}
        ureps = {0: issue_urep(0)}
        for it, (ch, b) in enumerate(iters):
            t0 = ch * C
            xc_s = xc_b[b][:, t0:t0 + C]
            if it + 2 < len(iters):
                bcs[it + 2] = a1(it + 2)
            if it + 1 < len(iters):
                ureps[it + 1] = issue_urep(it + 1)
            urep = ureps.pop(it)
            bc_sb = bcs.pop(it)

            # C in PSUM for the whole g-loop: hC becomes a single-SBUF-port
            # op, so gpsimd dBx is never locked out by it
            c_ps = pc.tile([128, C], F32, tag="c")
            nc.tensor.matmul(c_ps[:], qc_sel[:], bc_sb[:], start=True,
                             stop=True)

            dbx = bp.tile([128, DG * C], BF16, tag="dbx")
            h = hp.tile([128, DG * C], BF16, tag="h")
            b_s = br_b[b][:, t0:t0 + C]
            dl_s = dl_b[b][:, t0:t0 + C]
            y_ps = py.tile([DP, C], F32, tag="y")
            for g in range(DG):
                # dBx = u_rep * B  (gpsimd; overlaps the DVE scans)
                eng = nc.gpsimd if g < DBX_GP else nc.vector
                eng.tensor_tensor(dbx[:, g * C:(g + 1) * C],
                                  urep[:, g * C:(g + 1) * C], b_s, op=OP.mult)
                # delta broadcast -> PSUM
                drep_ps = pd.tile([128, C], F32, tag="d")
                nc.tensor.matmul(drep_ps[:], p_sel[:, g * 128:(g + 1) * 128],
                                 dl_s, start=True, stop=True)
                # dA = exp(A * delta) -> PSUM (scan in0 reads PSUM)
                dA_ps = pa.tile([128, C], F32, tag="a")
                nc.scalar.activation(dA_ps[:], drep_ps[:], ACTF.Exp,
                                     scale=a_pack[:, g:g + 1])
                hs = h[:, g * C:(g + 1) * C]
                init = 0.0 if ch == 0 else hcar_b[b][:, g:g + 1]
                nc.vector.tensor_tensor_scan(hs, dA_ps[:],
                                             dbx[:, g * C:(g + 1) * C], init,
                                             op0=OP.mult, op1=OP.add)
                hC = sp.tile([128, C], BF16, tag="hC")
                heng = nc.gpsimd if g < HC_GP else nc.vector
                heng.tensor_tensor(hC[:], hs, c_ps[:], op=OP.mult)
                nc.tensor.matmul(y_ps[:], ed_sel[:, g * DP:(g + 1) * DP],
                                 hC[:], start=(g == 0), stop=(g == DG - 1))
            if ch < NCH - 1:
                nc.vector.tensor_copy(
                    hcar_b[b][:].rearrange("p (g c) -> p g c", c=1),
                    h[:].rearrange("p (g c) -> p g c", g=DG)[:, :, C - 1:C])

            # ---- gate + head ----
            y1 = yp.tile([DP, C], BF16, tag="y1")
            nc.vector.scalar_tensor_tensor(y1[:], xc_s, d_col[:], y_ps[:],
                                           op0=OP.mult, op1=OP.add)
            yg = yp.tile([DP, C], BF16, tag="yg")
            nc.vector.tensor_tensor(yg[:], y1[:], zs_b[b][:, t0:t0 + C],
                                    op=OP.mult)
            g_ps = pf.tile([HID, C], F32, tag="f")
            nc.tensor.matmul(g_ps[:], w1T[:], yg[:], start=True, stop=True)
            g_sb = sp.tile([HID, C], BF16, tag="g")
            nc.scalar.activation(g_sb[:], g_ps[:], ACTF.Relu, bias=b_c1[:])
            lg_ps = pf.tile([128, Q * NL], F32, tag="f")
            for q in range(Q):
                nc.tensor.matmul(lg_ps[:, q * NL:(q + 1) * NL],
                                 g_sb[:, q * 128:(q + 1) * 128], w2T[:],
                                 start=True, stop=True)
            out_sb = sp.tile([128, Q * NL], F32, tag="o")
            nc.vector.tensor_copy(out_sb[:], lg_ps[:])
            nc.sync.dma_start(
                out_d[b, ch],
                out_sb[:].rearrange("p (q c) -> p q c", q=Q))

    nc.compile()
    return nc


def _packed_consts(A):
    p_sel = np.zeros((DP, DG * 128), np.float32)
    ed = np.zeros((128, DG * DP), np.float32)
    qb = np.zeros((2 * N, 128), np.float32)
    qc = np.zeros((2 * N, 128), np.float32)
    a_pack = np.zeros((128, DG), np.float32)
    for n in range(N):
        for ds in range(8):
            r = n * 8 + ds
            qb[n, r] = 1.0
            qc[N + n, r] = 1.0
            for g in range(DG):
                d = g * 8 + ds
                if d < DIN:
                    p_sel[d, g * 128 + r] = 1.0
                    ed[r, g * DP + d] = 1.0
                    a_pack[r, g] = A[d, n]
    bf = ml_dtypes.bfloat16
    return {"p_sel": p_sel.astype(bf), "ed_sel": ed.astype(bf),
            "qb_sel": qb.astype(bf), "qc_sel": qc.astype(bf),
            "a_pack": a_pack}


def _prep_inputs(inputs):
    bf = ml_dtypes.bfloat16
    x = np.asarray(inputs["x"], np.float32)
    W_in = np.asarray(inputs["W_in"], np.float64)
    conv_w = np.asarray(inputs["conv_w"], np.float64)
    conv_b = np.asarray(inputs["conv_b"], np.float64)
    W_xproj = np.asarray(inputs["W_xproj"], np.float64)
    W_dt = np.asarray(inputs["W_dt"], np.float64)
    b_dt = np.asarray(inputs["b_dt"], np.float64)
    A_log = np.asarray(inputs["A_log"], np.float64)
    D = np.asarray(inputs["D"], np.float64)
    W_out = np.asarray(inputs["W_out"], np.float64)
    W_c1 = np.asarray(inputs["W_c1"], np.float64)
    b_c1 = np.asarray(inputs["b_c1"], np.float64)
    W_c2 = np.asarray(inputs["W_c2"], np.float64)

    def padrc(a, rows, cols):
        out = np.zeros((rows, cols), np.float64)
        out[:a.shape[0], :a.shape[1]] = a
        return out

    # x: pad d_model 41->48, cast bf16, transpose to [b, d, t]
    xp = np.zeros((B, L, DMP), np.float32)
    xp[:, :, :DM] = x
    xT = np.ascontiguousarray(xp.transpose(0, 2, 1)).astype(bf)

    w_inT = np.zeros((DMP, 2 * DP), np.float64)
    w_inT[:DM, 0:DIN] = W_in[:DIN].T
    w_inT[:DM, DP:DP + DIN] = W_in[DIN:].T

    f32c = lambda a: np.ascontiguousarray(a, dtype=np.float32)
    bfc = lambda a: np.ascontiguousarray(a.astype(np.float32)).astype(bf)
    shared = {
        "w_inT": bfc(w_inT),
        "conv_w": f32c(padrc(conv_w, DP, K)),
        "conv_b": f32c(padrc(conv_b[:, None], DP, 1)),
        "w_effT": bfc(padrc((W_dt @ W_xproj[:DTR]).T, DP, DP)),
        "w_bcT": bfc(padrc(W_xproj[DTR:].T, DP, 2 * N)),
        "b_dt": f32c(padrc(b_dt[:, None], DP, 1)),
        "d_col": f32c(padrc(D[:, None], DP, 1)),
        "w1T": bfc(padrc((W_c1 @ W_out).T, DP, HID)),
        "b_c1": f32c(b_c1[:, None]),
        "w2T": bfc(W_c2.T),
        **_packed_consts((-np.exp(A_log)).astype(np.float32)),
    }
    in_maps = []
    for c in range(NCORES):
        m = dict(shared)
        m["xT"] = xT[c * BLOC:(c + 1) * BLOC]
        in_maps.append(m)
    return in_maps


def kernel(**inputs):
    return _run(inputs, trace=False)[0]


def kernel_traced(**inputs):
    return _run(inputs, trace=True)


def _run(inputs, trace=False):
    key = "nc"
    if key not in _cache:
        _cache[key] = _build({})
    nc = _cache[key]
    in_maps = _prep_inputs(inputs)
    res = run_bass_kernel_spmd(nc, in_maps, core_ids=list(range(NCORES)),
                               trace=trace)
    b_c2 = np.asarray(inputs["b_c2"], np.float32)
    # device layout [BLOC, NCH, 128, Q, NL] -> [BLOC, L, NL]
    outs = []
    for r in res.results:
        o = r["out"].transpose(0, 1, 3, 2, 4).reshape(BLOC, L, NL)
        outs.append(o)
    out = np.concatenate(outs, axis=0) + b_c2[None, None, :]
    return out, res


# revision 26
# speedup vs baseline: 1.6019x; 1.1024x over previous
"""Trainium2 Bass kernel for nn_Network_61658550501610 (Mamba block + MLP head).

Reference computation (per batch element b, sequence length L=2048):
  xz = x @ W_in.T; xi, z = split(xz)
  xc = silu(causal_depthwise_conv(xi, conv_w) + conv_b)
  x_dbl = xc @ W_xproj.T -> (dt, B, C)
  delta = softplus(dt @ W_dt.T + b_dt)
  h_t = exp(delta*A)*h_{t-1} + delta*B*xc   (selective scan, state [82,16])
  y = (h @ C) + D*xc; y *= silu(z)
  out = y @ W_out.T;  logits = relu(out@W_c1.T+b_c1)@W_c2.T + b_c2

Sharding: data-parallel over batch (B=16 -> 2 per core across 8 cores).

Layout on chip: d_inner (82) on partitions, time on free dim. The scan uses
the DVE tensor_tensor_scan instruction per state index n (16 of them), with
chunk carries through per-partition initial values. B[n,:]/C[n,:] are
broadcast across partitions with TensorE ones-matmuls into PSUM; the sum
over n runs as accumulating identity matmuls on TensorE.
"""
import ml_dtypes
import numpy as np

import concourse.bacc as bacc
import concourse.tile as tile
import concourse.mybir as mybir
from concourse.bass_utils import run_bass_kernel_spmd

F32 = mybir.dt.float32
F32R = mybir.dt.float32r
BF16 = mybir.dt.bfloat16
OP = mybir.AluOpType
ACTF = mybir.ActivationFunctionType
AX = mybir.AxisListType

# problem dims (hardcoded per contract)
B, L, DM = 16, 2048, 41
DIN, N, K = 82, 16, 4          # d_inner, d_state, d_conv
DTR, HID, NL = 3, 64, 10
NCORES = 8
BLOC = B // NCORES             # batch per core

DG = (DIN + 7) // 8            # 11 d-groups of 8 for the packed scan
DP = DG * 8                    # 88 padded d
C = 512                        # time-chunk length
NCH = L // C                   # chunks per batch element
Q = C // 128                   # 128-row subtiles per chunk

_cache = {}


def _build(cfg):
    nc = bacc.Bacc("TRN2", target_bir_lowering=False, debug=False,
                   enable_asserts=False)

    def din(name, shape):
        return nc.dram_tensor(name, list(shape), F32, kind="ExternalInput").ap()

    x_d = din("x", (BLOC, L, DM))
    w_inT_d = nc.dram_tensor("w_inT", [DM, 2 * DIN], F32R,
                             kind="ExternalInput").ap()
    w_effT_d = nc.dram_tensor("w_effT", [DIN, DIN], F32R,
                              kind="ExternalInput").ap()
    w_bcT_d = nc.dram_tensor("w_bcT", [DIN, 2 * N], F32R,
                             kind="ExternalInput").ap()
    conv_w_d = din("conv_w", (DIN, K))
    conv_diag_d = nc.dram_tensor("conv_diag", [DIN, K * DIN], F32R,
                                 kind="ExternalInput").ap()
    conv_b_d = din("conv_b", (DIN, 1))
    conv_bh_d = din("conv_bh", (DIN, 1))
    b_dt_d = din("b_dt", (DIN, 1))
    d_col_d = din("d_col", (DIN, 1))
    w1T_d = nc.dram_tensor("w1T", [DIN, HID], F32R,
                           kind="ExternalInput").ap()
    b_c1_d = din("b_c1", (HID, 1))
    w2T_d = din("w2T", (HID + 1, NL))
    ident_d = din("ident", (128, 128))
    e_sel_d = nc.dram_tensor("e_sel", [2 * N, 2 * N * DIN], BF16,
                             kind="ExternalInput").ap()
    p_sel_d = nc.dram_tensor("p_sel", [DIN, DG * 128], BF16,
                             kind="ExternalInput").ap()
    ed_sel_d = nc.dram_tensor("ed_sel", [128, DG * DP], BF16,
                              kind="ExternalInput").ap()
    qb_sel_d = nc.dram_tensor("qb_sel", [2 * N, 128], BF16,
                              kind="ExternalInput").ap()
    qc_sel_d = nc.dram_tensor("qc_sel", [2 * N, 128], BF16,
                              kind="ExternalInput").ap()
    a_pack_d = din("a_pack", (128, DG))
    out_d = nc.dram_tensor("out", [BLOC, L, NL], F32, kind="ExternalOutput").ap()

    with tile.TileContext(nc) as tc, tc.tile_pool(name="wts", bufs=1) as wp, \
         tc.tile_pool(name="work", bufs=3) as kp, \
         tc.tile_pool(name="seg", bufs=6) as sp, \
         tc.tile_pool(name="hbuf", bufs=2) as hp, \
         tc.tile_pool(name="ps_f", bufs=3, space="PSUM") as pf, \
         tc.tile_pool(name="ps_t", bufs=2, space="PSUM") as pt, \
         tc.tile_pool(name="ps_rep", bufs=2, space="PSUM") as prep, \
         tc.tile_pool(name="ps_y", bufs=1, space="PSUM") as py:

        # ---- constant weights ----
        w_inT = wp.tile([DM, 2 * DIN], F32R)
        w_effT = wp.tile([DIN, DIN], F32R)
        w_bcT = wp.tile([DIN, 2 * N], F32R)
        conv_w = wp.tile([DIN, K], F32)
        conv_diag = wp.tile([DIN, K * DIN], F32R)
        conv_b = wp.tile([DIN, 1], F32)
        conv_bh = wp.tile([DIN, 1], F32)
        b_dt = wp.tile([DIN, 1], F32)
        d_col = wp.tile([DIN, 1], F32)
        w1T = wp.tile([DIN, HID], F32R)
        b_c1 = wp.tile([HID, 1], F32)
        w2T = wp.tile([HID + 1, NL], F32)
        ident = wp.tile([128, 128], F32)
        p_sel = wp.tile([DIN, DG * 128], BF16)
        ed_sel = wp.tile([128, DG * DP], BF16)
        qb_sel = wp.tile([2 * N, 128], BF16)
        qc_sel = wp.tile([2 * N, 128], BF16)
        a_pack = wp.tile([128, DG], F32)
        for t_, d_ in [(w_inT, w_inT_d), (w_effT, w_effT_d), (w_bcT, w_bcT_d),
                       (conv_w, conv_w_d), (conv_diag, conv_diag_d),
                       (conv_b, conv_b_d),
                       (conv_bh, conv_bh_d),
                       (b_dt, b_dt_d), (d_col, d_col_d), (w1T, w1T_d),
                       (w2T, w2T_d), (ident, ident_d),
                       (p_sel, p_sel_d), (ed_sel, ed_sel_d),
                       (qb_sel, qb_sel_d), (qc_sel, qc_sel_d),
                       (a_pack, a_pack_d),
                       (b_c1, b_c1_d)]:
            nc.sync.dma_start(t_[:], d_[:])

        # persistent state, one per batch element (independent streams)
        h_carry_b = [wp.tile([128, DG], F32, name=f"hcar{i}", tag=f"hcar{i}")
                     for i in range(BLOC)]
        halo_b = [wp.tile([DIN, K - 1], F32, name=f"halo{i}", tag=f"halo{i}")
                  for i in range(BLOC)]
        for t_ in halo_b:
            nc.vector.memset(t_[:], 0.0)

        def front(ch, b):
            h_carry = h_carry_b[b]
            halo = halo_b[b]
            t0 = ch * C
            # ---- load x chunk [C, DM] as [128, Q*DM] ----
            x_in = kp.tile([128, Q * DM], F32)
            src = x_d[b, t0:t0 + C, :].rearrange("(q p) d -> p q d", p=128)
            nc.sync.dma_start(x_in[:].rearrange("p (q d) -> p q d", q=Q), src)

            # ---- transpose to xT [DM, C] ----
            xT_ps = pf.tile([DM, C], F32, tag="f")
            for q in range(Q):
                nc.tensor.transpose(
                    xT_ps[:, q * 128:(q + 1) * 128],
                    x_in[:, q * DM:(q + 1) * DM], ident[:])
            xT = kp.tile([DM, C], F32)
            nc.scalar.copy(xT[:].bitcast(F32R), xT_ps[:])

            # ---- input projection ----
            xi_ps = pf.tile([DIN, C], F32, tag="f")
            z_ps = pf.tile([DIN, C], F32, tag="f")
            nc.tensor.matmul(xi_ps[:], w_inT[:, 0:DIN],
                             xT[:].bitcast(F32R), start=True, stop=True)
            nc.tensor.matmul(z_ps[:], w_inT[:, DIN:2 * DIN],
                             xT[:].bitcast(F32R), start=True, stop=True)

            # silu(z) via tanh
            th_z = kp.tile([DIN, C], F32)
            nc.scalar.activation(th_z[:], z_ps[:], ACTF.Tanh, scale=0.5)
            sg_z = kp.tile([DIN, C], F32)
            nc.vector.tensor_scalar(sg_z[:], th_z[:], 0.5, 0.5,
                                    op0=OP.mult, op1=OP.add)
            zs = kp.tile([DIN, C], F32)
            nc.vector.tensor_tensor(zs[:], z_ps[:], sg_z[:], op=OP.mult)

            # ---- causal depthwise conv (K=4) on TensorE ----
            # xi_sb is halo-extended: cols 0..2 = xi[-3:], cols 3..C+2 = xi
            # xc_pre[t] = sum_k diag(w_k) @ xi_sb[:, k+t] (4 accumulating
            # diagonal fp32r matmuls over shifted views)
            xi_sb = kp.tile([DIN, C + K - 1], F32)
            nc.vector.tensor_copy(xi_sb[:, 0:K - 1].bitcast(F32R),
                                  halo[:])
            nc.scalar.copy(xi_sb[:, K - 1:C + K - 1].bitcast(F32R), xi_ps[:])
            if ch < NCH - 1:
                nc.vector.tensor_copy(halo[:], xi_sb[:, C:C + K - 1])
            xc_pre = pf.tile([DIN, C], F32, tag="f")
            for k in range(K):
                nc.tensor.matmul(xc_pre[:],
                                 conv_diag[:, k * DIN:(k + 1) * DIN],
                                 xi_sb[:, k:k + C].bitcast(F32R),
                                 start=(k == 0), stop=(k == K - 1))
            # silu(v) = v * (0.5 + 0.5*tanh(v/2)), v = xc_pre + conv_b
            th = kp.tile([DIN, C], F32)
            nc.scalar.activation(th[:], xc_pre[:], ACTF.Tanh,
                                 bias=conv_bh[:], scale=0.5)
            sg = kp.tile([DIN, C], F32)
            nc.vector.tensor_scalar(sg[:], th[:], 0.5, 0.5,
                                    op0=OP.mult, op1=OP.add)
            xc = kp.tile([DIN, C], F32)
            nc.vector.scalar_tensor_tensor(xc[:].bitcast(F32R), xc_pre[:],
                                           conv_b[:], sg[:],
                                           op0=OP.add, op1=OP.mult)

            # ---- x_proj: delta / B / C ----
            dpre_ps = pf.tile([DIN, C], F32, tag="f")
            nc.tensor.matmul(dpre_ps[:], w_effT[:],
                             xc[:].bitcast(F32R), start=True, stop=True)
            # softplus(v) = ln(exp(v) + 1), v = dpre + b_dt
            e_sp = kp.tile([DIN, C], F32)
            nc.scalar.activation(e_sp[:], dpre_ps[:], ACTF.Exp, bias=b_dt[:])
            delta = kp.tile([DIN, C], BF16)
            nc.scalar.activation(delta[:], e_sp[:], ACTF.Ln, bias=1.0)
            bc_ps = pf.tile([2 * N, C], F32, tag="f")
            nc.tensor.matmul(bc_ps[:], w_bcT[:],
                             xc[:].bitcast(F32R), start=True, stop=True)
            bc_sb = kp.tile([2 * N, C], BF16)
            nc.scalar.copy(bc_sb[:], bc_ps[:])

            # u = delta * xc (bf16, feeds the p_sel replication matmul)
            u = kp.tile([DIN, C], BF16)
            nc.vector.tensor_tensor(u[:], delta[:], xc[:], op=OP.mult)

            return dict(delta=delta, u=u, bc_sb=bc_sb, xc=xc, zs=zs)

        def back(ch, b, st):
            h_carry = h_carry_b[b]
            halo = halo_b[b]
            t0 = ch * C
            delta, u, bc_sb, xc, zs = (st["delta"], st["u"], st["bc_sb"],
                                       st["xc"], st["zs"])
            # ---- packed selective scan: rows = (n, dsub) ----
            bq_ps = pt.tile([128, C], F32, tag="t")
            nc.tensor.matmul(bq_ps[:], qb_sel[:], bc_sb[:],
                             start=True, stop=True)
            b_sb = kp.tile([128, C], BF16)
            nc.scalar.copy(b_sb[:], bq_ps[:])
            cq_ps = pt.tile([128, C], F32, tag="t")
            nc.tensor.matmul(cq_ps[:], qc_sel[:], bc_sb[:],
                             start=True, stop=True)
            c_sb = kp.tile([128, C], BF16)
            nc.scalar.copy(c_sb[:], cq_ps[:])

            h = hp.tile([128, DG * C], BF16, tag="h")
            y_ps = py.tile([DP, C], F32, tag="y")
            for g in range(DG):
                d_ps = prep.tile([128, C], F32, tag="rep")
                nc.tensor.matmul(d_ps[:], p_sel[:, g * 128:(g + 1) * 128],
                                 delta[:], start=True, stop=True)
                u_ps = prep.tile([128, C], F32, tag="rep")
                nc.tensor.matmul(u_ps[:], p_sel[:, g * 128:(g + 1) * 128],
                                 u[:], start=True, stop=True)
                dA = sp.tile([128, C], BF16, tag="dA")
                nc.scalar.activation(dA[:], d_ps[:], ACTF.Exp,
                                     scale=a_pack[:, g:g + 1])
                u_sb = sp.tile([128, C], BF16, tag="u_sb")
                nc.scalar.copy(u_sb[:], u_ps[:])
                dBx = sp.tile([128, C], BF16, tag="dBx")
                nc.vector.tensor_tensor(dBx[:], u_sb[:], b_sb[:], op=OP.mult)
                hs = h[:, g * C:(g + 1) * C]
                init = 0.0 if ch == 0 else h_carry[:, g:g + 1]
                nc.vector.tensor_tensor_scan(hs, dA[:], dBx[:], init,
                                             op0=OP.mult, op1=OP.add)
                hC = sp.tile([128, C], BF16, tag="hC")
                nc.vector.tensor_tensor(hC[:], c_sb[:], hs, op=OP.mult)
                nc.tensor.matmul(y_ps[:], ed_sel[:, g * DP:(g + 1) * DP],
                                 hC[:], start=(g == 0), stop=(g == DG - 1))
            if ch < NCH - 1:
                nc.vector.tensor_copy(
                    h_carry[:].rearrange("p (g c) -> p g c", c=1),
                    h[:].rearrange("p (g c) -> p g c", g=DG)[:, :, C - 1:C])

            # ---- gate + output ----
            y1 = kp.tile([DIN, C], F32)
            nc.vector.scalar_tensor_tensor(y1[:], xc[:], d_col[:],
                                           y_ps[0:DIN, :],
                                           op0=OP.mult, op1=OP.add)
            y_gated = kp.tile([DIN, C], F32)
            nc.vector.tensor_tensor(y_gated[:].bitcast(F32R), y1[:],
                                    zs[:], op=OP.mult)

            g_ps = pt.tile([HID, C], F32, tag="t")
            nc.tensor.matmul(g_ps[:], w1T[:],
                             y_gated[:].bitcast(F32R), start=True, stop=True)
            g_aug = kp.tile([HID + 1, C], F32)
            nc.scalar.activation(g_aug[0:HID, :], g_ps[:], ACTF.Relu,
                                 bias=b_c1[:])
            nc.vector.memset(g_aug[HID:HID + 1, :], 1.0)

            out_sb = kp.tile([128, Q * NL], F32)
            for q in range(Q):
                lg_ps = pt.tile([128, NL], F32, tag="t")
                nc.tensor.matmul(lg_ps[:], g_aug[:, q * 128:(q + 1) * 128],
                                 w2T[:], start=True, stop=True)
                nc.scalar.copy(out_sb[:, q * NL:(q + 1) * NL], lg_ps[:])
            dst = out_d[b, t0:t0 + C, :].rearrange("(q p) c -> p q c", p=128)
            nc.sync.dma_start(
                dst, out_sb[:].rearrange("p (q c) -> p q c", q=Q))


        iters = [(ch, b) for ch in range(NCH) for b in range(BLOC)]
        pend = None
        for j, (ch, b) in enumerate(iters):
            st = front(ch, b)
            if pend is not None:
                back(*pend)
            pend = (ch, b, st)
        back(*pend)

    nc.compile()
    return nc


def _e_sel():
    # e_sel[:, j*DIN:(j+1)*DIN] is [2N, DIN]; row j all-ones, rest zero:
    # lhsT for the TensorE partition-broadcast of bc row j.
    e = np.zeros((2 * N, 2 * N * DIN), np.float32)
    for j in range(2 * N):
        e[j, j * DIN:(j + 1) * DIN] = 1.0
    return e


def _packed_consts(A):
    p_sel = np.zeros((DIN, DG * 128), np.float32)
    ed = np.zeros((128, DG * DP), np.float32)
    qb = np.zeros((2 * N, 128), np.float32)
    qc = np.zeros((2 * N, 128), np.float32)
    a_pack = np.zeros((128, DG), np.float32)
    for n in range(N):
        for ds in range(8):
            r = n * 8 + ds
            qb[n, r] = 1.0
            qc[N + n, r] = 1.0
            for g in range(DG):
                d = g * 8 + ds
                if d < DIN:
                    p_sel[d, g * 128 + r] = 1.0
                    ed[r, g * DP + d] = 1.0
                    a_pack[r, g] = A[d, n]
    bf = ml_dtypes.bfloat16
    return {"p_sel": p_sel.astype(bf), "ed_sel": ed.astype(bf),
            "qb_sel": qb.astype(bf), "qc_sel": qc.astype(bf),
            "a_pack": a_pack}


def _prep_inputs(inputs):
    x = np.ascontiguousarray(inputs["x"], dtype=np.float32)
    W_in = np.asarray(inputs["W_in"], np.float64)
    conv_w = np.asarray(inputs["conv_w"], np.float64)
    conv_b = np.asarray(inputs["conv_b"], np.float64)
    W_xproj = np.asarray(inputs["W_xproj"], np.float64)
    W_dt = np.asarray(inputs["W_dt"], np.float64)
    b_dt = np.asarray(inputs["b_dt"], np.float64)
    A_log = np.asarray(inputs["A_log"], np.float64)
    D = np.asarray(inputs["D"], np.float64)
    W_out = np.asarray(inputs["W_out"], np.float64)
    W_c1 = np.asarray(inputs["W_c1"], np.float64)
    b_c1 = np.asarray(inputs["b_c1"], np.float64)
    W_c2 = np.asarray(inputs["W_c2"], np.float64)
    b_c2 = np.asarray(inputs["b_c2"], np.float64)

    f = lambda a: np.ascontiguousarray(a, dtype=np.float32)
    shared = {
        "w_inT": f(W_in.T),
        "w_effT": f((W_dt @ W_xproj[:DTR]).T),
        "w_bcT": f(W_xproj[DTR:].T),
        "conv_w": f(conv_w),
        "conv_b": f(conv_b[:, None]),
        "conv_diag": np.concatenate(
            [np.diag(conv_w[:, k]) for k in range(K)], axis=1).astype(np.float32),
        "conv_bh": f(conv_b[:, None] * 0.5),
        "b_dt": f(b_dt[:, None]),
        "d_col": f(D[:, None]),
        "w1T": f((W_c1 @ W_out).T),
        "b_c1": f(b_c1[:, None]),
        "w2T": f(np.vstack([W_c2.T, b_c2[None, :]])),
        "ident": np.eye(128, dtype=np.float32),
        "e_sel": _e_sel().astype(ml_dtypes.bfloat16),
        **_packed_consts(f(-np.exp(A_log))),
    }
    in_maps = []
    for c in range(NCORES):
        m = dict(shared)
        m["x"] = x[c * BLOC:(c + 1) * BLOC]
        in_maps.append(m)
    return in_maps


def kernel(**inputs):
    return _run(inputs, trace=False)[0]


def kernel_traced(**inputs):
    return _run(inputs, trace=True)


def _run(inputs, trace=False):
    key = "nc"
    if key not in _cache:
        _cache[key] = _build({})
    nc = _cache[key]
    in_maps = _prep_inputs(inputs)
    res = run_bass_kernel_spmd(nc, in_maps, core_ids=list(range(NCORES)),
                               trace=trace)
    out = np.concatenate([r["out"] for r in res.results], axis=0)
    return out, res


# revision 27
# speedup vs baseline: 1.6671x; 1.0407x over previous
"""Trainium2 Bass kernel for nn_Network_61658550501610 (Mamba block + MLP head).

Reference computation (per batch element b, sequence length L=2048):
  xz = x @ W_in.T; xi, z = split(xz)
  xc = silu(causal_depthwise_conv(xi, conv_w) + conv_b)
  x_dbl = xc @ W_xproj.T -> (dt, B, C)
  delta = softplus(dt @ W_dt.T + b_dt)
  h_t = exp(delta*A)*h_{t-1} + delta*B*xc   (selective scan, state [82,16])
  y = (h @ C) + D*xc; y *= silu(z)
  out = y @ W_out.T;  logits = relu(out@W_c1.T+b_c1)@W_c2.T + b_c2

Sharding: data-parallel over batch (B=16 -> 2 per core across 8 cores).

Layout on chip: d_inner (82) on partitions, time on free dim. The scan uses
the DVE tensor_tensor_scan instruction per state index n (16 of them), with
chunk carries through per-partition initial values. B[n,:]/C[n,:] are
broadcast across partitions with TensorE ones-matmuls into PSUM; the sum
over n runs as accumulating identity matmuls on TensorE.
"""
import ml_dtypes
import numpy as np

import bass_rust as _bass_rust
import concourse.bacc as bacc
import concourse.tile as tile
import concourse.mybir as mybir
from concourse.bass_utils import run_bass_kernel_spmd
from concourse.hw_specs import get_activation_tables


class _Bacc(bacc.Bacc):
    """All activations here use {Exp, Ln, Relu, Copy}, which the combined
    natural_log_exp set covers; blanking the other sets (order kept: set ids
    are positional into act_info.json) pins one resident table -> a single
    ACT_TABLE_LOAD instead of two reloads per chunk."""

    _KEEP = ("natural_log_exp_and_others",)

    def insert_act_table_loads(self):
        has_activation = any(
            isinstance(i, mybir.InstActivation)
            for b in self.main_func.blocks
            for i in b.instructions
        )
        if not has_activation:
            return
        tables = [(name, fns if name in self._KEEP else set())
                  for name, fns in get_activation_tables(self.m.arch).items()]
        _bass_rust.insert_act_table_loads(self, tables)

F32 = mybir.dt.float32
F32R = mybir.dt.float32r
BF16 = mybir.dt.bfloat16
OP = mybir.AluOpType
ACTF = mybir.ActivationFunctionType
AX = mybir.AxisListType

# problem dims (hardcoded per contract)
B, L, DM = 16, 2048, 41
DIN, N, K = 82, 16, 4          # d_inner, d_state, d_conv
DTR, HID, NL = 3, 64, 10
NCORES = 8
BLOC = B // NCORES             # batch per core

DG = (DIN + 7) // 8            # 11 d-groups of 8 for the packed scan
DP = DG * 8                    # 88 padded d
C = 512                        # time-chunk length
NCH = L // C                   # chunks per batch element
Q = C // 128                   # 128-row subtiles per chunk

_cache = {}


def _build(cfg):
    nc = _Bacc("TRN2", target_bir_lowering=False, debug=False,
               enable_asserts=False)

    def din(name, shape):
        return nc.dram_tensor(name, list(shape), F32, kind="ExternalInput").ap()

    x_d = din("x", (BLOC, L, DM))
    w_inT_d = nc.dram_tensor("w_inT", [DM, 2 * DIN], F32R,
                             kind="ExternalInput").ap()
    w_effT_d = nc.dram_tensor("w_effT", [DIN, DIN], F32R,
                              kind="ExternalInput").ap()
    w_bcT_d = nc.dram_tensor("w_bcT", [DIN, 2 * N], F32R,
                             kind="ExternalInput").ap()
    conv_w_d = din("conv_w", (DIN, K))
    conv_diag_d = nc.dram_tensor("conv_diag", [DIN, K * DIN], F32R,
                                 kind="ExternalInput").ap()
    conv_b_d = din("conv_b", (DIN, 1))
    conv_bh_d = din("conv_bh", (DIN, 1))
    b_dt_d = din("b_dt", (DIN, 1))
    d_col_d = din("d_col", (DIN, 1))
    w1T_d = nc.dram_tensor("w1T", [DIN, HID], F32R,
                           kind="ExternalInput").ap()
    b_c1_d = din("b_c1", (HID, 1))
    w2T_d = din("w2T", (HID + 1, NL))
    ident_d = din("ident", (128, 128))
    e_sel_d = nc.dram_tensor("e_sel", [2 * N, 2 * N * DIN], BF16,
                             kind="ExternalInput").ap()
    p_sel_d = nc.dram_tensor("p_sel", [DIN, DG * 128], BF16,
                             kind="ExternalInput").ap()
    ed_sel_d = nc.dram_tensor("ed_sel", [128, DG * DP], BF16,
                              kind="ExternalInput").ap()
    qb_sel_d = nc.dram_tensor("qb_sel", [2 * N, 128], BF16,
                              kind="ExternalInput").ap()
    qc_sel_d = nc.dram_tensor("qc_sel", [2 * N, 128], BF16,
                              kind="ExternalInput").ap()
    a_pack_d = din("a_pack", (128, DG))
    out_d = nc.dram_tensor("out", [BLOC, L, NL], F32, kind="ExternalOutput").ap()

    with tile.TileContext(nc) as tc, tc.tile_pool(name="wts", bufs=1) as wp, \
         tc.tile_pool(name="work", bufs=3) as kp, \
         tc.tile_pool(name="seg", bufs=6) as sp, \
         tc.tile_pool(name="hbuf", bufs=2) as hp, \
         tc.tile_pool(name="ps_f", bufs=3, space="PSUM") as pf, \
         tc.tile_pool(name="ps_t", bufs=2, space="PSUM") as pt, \
         tc.tile_pool(name="ps_rep", bufs=2, space="PSUM") as prep, \
         tc.tile_pool(name="ps_y", bufs=1, space="PSUM") as py:

        # ---- constant weights ----
        w_inT = wp.tile([DM, 2 * DIN], F32R)
        w_effT = wp.tile([DIN, DIN], F32R)
        w_bcT = wp.tile([DIN, 2 * N], F32R)
        conv_w = wp.tile([DIN, K], F32)
        conv_diag = wp.tile([DIN, K * DIN], F32R)
        conv_b = wp.tile([DIN, 1], F32)
        conv_bh = wp.tile([DIN, 1], F32)
        b_dt = wp.tile([DIN, 1], F32)
        d_col = wp.tile([DIN, 1], F32)
        w1T = wp.tile([DIN, HID], F32R)
        b_c1 = wp.tile([HID, 1], F32)
        w2T = wp.tile([HID + 1, NL], F32)
        ident = wp.tile([128, 128], F32)
        p_sel = wp.tile([DIN, DG * 128], BF16)
        ed_sel = wp.tile([128, DG * DP], BF16)
        qb_sel = wp.tile([2 * N, 128], BF16)
        qc_sel = wp.tile([2 * N, 128], BF16)
        a_pack = wp.tile([128, DG], F32)
        for t_, d_ in [(w_inT, w_inT_d), (w_effT, w_effT_d), (w_bcT, w_bcT_d),
                       (conv_w, conv_w_d), (conv_diag, conv_diag_d),
                       (conv_b, conv_b_d),
                       (conv_bh, conv_bh_d),
                       (b_dt, b_dt_d), (d_col, d_col_d), (w1T, w1T_d),
                       (w2T, w2T_d), (ident, ident_d),
                       (p_sel, p_sel_d), (ed_sel, ed_sel_d),
                       (qb_sel, qb_sel_d), (qc_sel, qc_sel_d),
                       (a_pack, a_pack_d),
                       (b_c1, b_c1_d)]:
            nc.sync.dma_start(t_[:], d_[:])

        # persistent state, one per batch element (independent streams)
        h_carry_b = [wp.tile([128, DG], F32, name=f"hcar{i}", tag=f"hcar{i}")
                     for i in range(BLOC)]
        halo_b = [wp.tile([DIN, K - 1], F32, name=f"halo{i}", tag=f"halo{i}")
                  for i in range(BLOC)]
        for t_ in halo_b:
            nc.vector.memset(t_[:], 0.0)

        def front(ch, b):
            h_carry = h_carry_b[b]
            halo = halo_b[b]
            t0 = ch * C
            # ---- load x chunk [C, DM] as [128, Q*DM] ----
            x_in = kp.tile([128, Q * DM], F32)
            src = x_d[b, t0:t0 + C, :].rearrange("(q p) d -> p q d", p=128)
            nc.sync.dma_start(x_in[:].rearrange("p (q d) -> p q d", q=Q), src)

            # ---- transpose to xT [DM, C] ----
            xT_ps = pf.tile([DM, C], F32, tag="f")
            for q in range(Q):
                nc.tensor.transpose(
                    xT_ps[:, q * 128:(q + 1) * 128],
                    x_in[:, q * DM:(q + 1) * DM], ident[:])
            xT = kp.tile([DM, C], F32)
            nc.scalar.copy(xT[:].bitcast(F32R), xT_ps[:])

            # ---- input projection ----
            xi_ps = pf.tile([DIN, C], F32, tag="f")
            z_ps = pf.tile([DIN, C], F32, tag="f")
            nc.tensor.matmul(xi_ps[:], w_inT[:, 0:DIN],
                             xT[:].bitcast(F32R), start=True, stop=True)
            nc.tensor.matmul(z_ps[:], w_inT[:, DIN:2 * DIN],
                             xT[:].bitcast(F32R), start=True, stop=True)

            # silu(z) = z*sigmoid(z); sigmoid(z) = exp(-ln(1+exp(-z)))
            # keeps every Act op on the single ln+exp table set
            e_z = kp.tile([DIN, C], F32)
            nc.scalar.activation(e_z[:], z_ps[:], ACTF.Exp, scale=-1.0)
            sp_z = kp.tile([DIN, C], F32)
            nc.scalar.activation(sp_z[:], e_z[:], ACTF.Ln, bias=1.0)
            sg_z = kp.tile([DIN, C], F32)
            nc.scalar.activation(sg_z[:], sp_z[:], ACTF.Exp, scale=-1.0)
            zs = kp.tile([DIN, C], F32)
            nc.vector.tensor_tensor(zs[:], z_ps[:], sg_z[:], op=OP.mult)

            # ---- causal depthwise conv (K=4) on TensorE ----
            # xi_sb is halo-extended: cols 0..2 = xi[-3:], cols 3..C+2 = xi
            # xc_pre[t] = sum_k diag(w_k) @ xi_sb[:, k+t] (4 accumulating
            # diagonal fp32r matmuls over shifted views)
            xi_sb = kp.tile([DIN, C + K - 1], F32)
            nc.vector.tensor_copy(xi_sb[:, 0:K - 1].bitcast(F32R),
                                  halo[:])
            nc.scalar.copy(xi_sb[:, K - 1:C + K - 1].bitcast(F32R), xi_ps[:])
            if ch < NCH - 1:
                nc.vector.tensor_copy(halo[:], xi_sb[:, C:C + K - 1])
            xc_pre = pf.tile([DIN, C], F32, tag="f")
            for k in range(K):
                nc.tensor.matmul(xc_pre[:],
                                 conv_diag[:, k * DIN:(k + 1) * DIN],
                                 xi_sb[:, k:k + C].bitcast(F32R),
                                 start=(k == 0), stop=(k == K - 1))
            # silu(v) = v*sigmoid(v), v = xc_pre + conv_b;
            # exp(-v) = exp(-xc_pre + conv_bh) with conv_bh = -conv_b
            e_x = kp.tile([DIN, C], F32)
            nc.scalar.activation(e_x[:], xc_pre[:], ACTF.Exp,
                                 bias=conv_bh[:], scale=-1.0)
            sp_x = kp.tile([DIN, C], F32)
            nc.scalar.activation(sp_x[:], e_x[:], ACTF.Ln, bias=1.0)
            sg = kp.tile([DIN, C], F32)
            nc.scalar.activation(sg[:], sp_x[:], ACTF.Exp, scale=-1.0)
            xc = kp.tile([DIN, C], F32)
            nc.vector.scalar_tensor_tensor(xc[:].bitcast(F32R), xc_pre[:],
                                           conv_b[:], sg[:],
                                           op0=OP.add, op1=OP.mult)

            # ---- x_proj: delta / B / C ----
            dpre_ps = pf.tile([DIN, C], F32, tag="f")
            nc.tensor.matmul(dpre_ps[:], w_effT[:],
                             xc[:].bitcast(F32R), start=True, stop=True)
            # softplus(v) = ln(exp(v) + 1), v = dpre + b_dt
            e_sp = kp.tile([DIN, C], F32)
            nc.scalar.activation(e_sp[:], dpre_ps[:], ACTF.Exp, bias=b_dt[:])
            delta = kp.tile([DIN, C], BF16)
            nc.scalar.activation(delta[:], e_sp[:], ACTF.Ln, bias=1.0)
            bc_ps = pf.tile([2 * N, C], F32, tag="f")
            nc.tensor.matmul(bc_ps[:], w_bcT[:],
                             xc[:].bitcast(F32R), start=True, stop=True)
            bc_sb = kp.tile([2 * N, C], BF16)
            nc.scalar.copy(bc_sb[:], bc_ps[:])

            # u = delta * xc (bf16, feeds the p_sel replication matmul)
            u = kp.tile([DIN, C], BF16)
            nc.vector.tensor_tensor(u[:], delta[:], xc[:], op=OP.mult)

            return dict(delta=delta, u=u, bc_sb=bc_sb, xc=xc, zs=zs)

        def back(ch, b, st):
            h_carry = h_carry_b[b]
            halo = halo_b[b]
            t0 = ch * C
            delta, u, bc_sb, xc, zs = (st["delta"], st["u"], st["bc_sb"],
                                       st["xc"], st["zs"])
            # ---- packed selective scan: rows = (n, dsub) ----
            bq_ps = pt.tile([128, C], F32, tag="t")
            nc.tensor.matmul(bq_ps[:], qb_sel[:], bc_sb[:],
                             start=True, stop=True)
            b_sb = kp.tile([128, C], BF16)
            nc.scalar.copy(b_sb[:], bq_ps[:])
            cq_ps = pt.tile([128, C], F32, tag="t")
            nc.tensor.matmul(cq_ps[:], qc_sel[:], bc_sb[:],
                             start=True, stop=True)
            c_sb = kp.tile([128, C], BF16)
            nc.scalar.copy(c_sb[:], cq_ps[:])

            h = hp.tile([128, DG * C], BF16, tag="h")
            y_ps = py.tile([DP, C], F32, tag="y")
            for g in range(DG):
                d_ps = prep.tile([128, C], F32, tag="rep")
                nc.tensor.matmul(d_ps[:], p_sel[:, g * 128:(g + 1) * 128],
                                 delta[:], start=True, stop=True)
                u_ps = prep.tile([128, C], F32, tag="rep")
                nc.tensor.matmul(u_ps[:], p_sel[:, g * 128:(g + 1) * 128],
                                 u[:], start=True, stop=True)
                dA = sp.tile([128, C], BF16, tag="dA")
                nc.scalar.activation(dA[:], d_ps[:], ACTF.Exp,
                                     scale=a_pack[:, g:g + 1])
                u_sb = sp.tile([128, C], BF16, tag="u_sb")
                nc.scalar.copy(u_sb[:], u_ps[:])
                dBx = sp.tile([128, C], BF16, tag="dBx")
                nc.vector.tensor_tensor(dBx[:], u_sb[:], b_sb[:], op=OP.mult)
                hs = h[:, g * C:(g + 1) * C]
                init = 0.0 if ch == 0 else h_carry[:, g:g + 1]
                nc.vector.tensor_tensor_scan(hs, dA[:], dBx[:], init,
                                             op0=OP.mult, op1=OP.add)
                hC = sp.tile([128, C], BF16, tag="hC")
                nc.vector.tensor_tensor(hC[:], c_sb[:], hs, op=OP.mult)
                nc.tensor.matmul(y_ps[:], ed_sel[:, g * DP:(g + 1) * DP],
                                 hC[:], start=(g == 0), stop=(g == DG - 1))
            if ch < NCH - 1:
                nc.vector.tensor_copy(
                    h_carry[:].rearrange("p (g c) -> p g c", c=1),
                    h[:].rearrange("p (g c) -> p g c", g=DG)[:, :, C - 1:C])

            # ---- gate + output ----
            y1 = kp.tile([DIN, C], F32)
            nc.vector.scalar_tensor_tensor(y1[:], xc[:], d_col[:],
                                           y_ps[0:DIN, :],
                                           op0=OP.mult, op1=OP.add)
            y_gated = kp.tile([DIN, C], F32)
            nc.vector.tensor_tensor(y_gated[:].bitcast(F32R), y1[:],
                                    zs[:], op=OP.mult)

            g_ps = pt.tile([HID, C], F32, tag="t")
            nc.tensor.matmul(g_ps[:], w1T[:],
                             y_gated[:].bitcast(F32R), start=True, stop=True)
            g_aug = kp.tile([HID + 1, C], F32)
            nc.scalar.activation(g_aug[0:HID, :], g_ps[:], ACTF.Relu,
                                 bias=b_c1[:])
            nc.vector.memset(g_aug[HID:HID + 1, :], 1.0)

            out_sb = kp.tile([128, Q * NL], F32)
            for q in range(Q):
                lg_ps = pt.tile([128, NL], F32, tag="t")
                nc.tensor.matmul(lg_ps[:], g_aug[:, q * 128:(q + 1) * 128],
                                 w2T[:], start=True, stop=True)
                nc.scalar.copy(out_sb[:, q * NL:(q + 1) * NL], lg_ps[:])
            dst = out_d[b, t0:t0 + C, :].rearrange("(q p) c -> p q c", p=128)
            nc.sync.dma_start(
                dst, out_sb[:].rearrange("p (q c) -> p q c", q=Q))


        iters = [(ch, b) for ch in range(NCH) for b in range(BLOC)]
        pend = None
        for j, (ch, b) in enumerate(iters):
            st = front(ch, b)
            if pend is not None:
                back(*pend)
            pend = (ch, b, st)
        back(*pend)

    nc.compile()
    return nc


def _e_sel():
    # e_sel[:, j*DIN:(j+1)*DIN] is [2N, DIN]; row j all-ones, rest zero:
    # lhsT for the TensorE partition-broadcast of bc row j.
    e = np.zeros((2 * N, 2 * N * DIN), np.float32)
    for j in range(2 * N):
        e[j, j * DIN:(j + 1) * DIN] = 1.0
    return e


def _packed_consts(A):
    p_sel = np.zeros((DIN, DG * 128), np.float32)
    ed = np.zeros((128, DG * DP), np.float32)
    qb = np.zeros((2 * N, 128), np.float32)
    qc = np.zeros((2 * N, 128), np.float32)
    a_pack = np.zeros((128, DG), np.float32)
    for n in range(N):
        for ds in range(8):
            r = n * 8 + ds
            qb[n, r] = 1.0
            qc[N + n, r] = 1.0
            for g in range(DG):
                d = g * 8 + ds
                if d < DIN:
                    p_sel[d, g * 128 + r] = 1.0
                    ed[r, g * DP + d] = 1.0
                    a_pack[r, g] = A[d, n]
    bf = ml_dtypes.bfloat16
    return {"p_sel": p_sel.astype(bf), "ed_sel": ed.astype(bf),
            "qb_sel": qb.astype(bf), "qc_sel": qc.astype(bf),
            "a_pack": a_pack}


def _prep_inputs(inputs):
    x = np.ascontiguousarray(inputs["x"], dtype=np.float32)
    W_in = np.asarray(inputs["W_in"], np.float64)
    conv_w = np.asarray(inputs["conv_w"], np.float64)
    conv_b = np.asarray(inputs["conv_b"], np.float64)
    W_xproj = np.asarray(inputs["W_xproj"], np.float64)
    W_dt = np.asarray(inputs["W_dt"], np.float64)
    b_dt = np.asarray(inputs["b_dt"], np.float64)
    A_log = np.asarray(inputs["A_log"], np.float64)
    D = np.asarray(inputs["D"], np.float64)
    W_out = np.asarray(inputs["W_out"], np.float64)
    W_c1 = np.asarray(inputs["W_c1"], np.float64)
    b_c1 = np.asarray(inputs["b_c1"], np.float64)
    W_c2 = np.asarray(inputs["W_c2"], np.float64)
    b_c2 = np.asarray(inputs["b_c2"], np.float64)

    f = lambda a: np.ascontiguousarray(a, dtype=np.float32)
    shared = {
        "w_inT": f(W_in.T),
        "w_effT": f((W_dt @ W_xproj[:DTR]).T),
        "w_bcT": f(W_xproj[DTR:].T),
        "conv_w": f(conv_w),
        "conv_b": f(conv_b[:, None]),
        "conv_diag": np.concatenate(
            [np.diag(conv_w[:, k]) for k in range(K)], axis=1).astype(np.float32),
        "conv_bh": f(-conv_b[:, None]),
        "b_dt": f(b_dt[:, None]),
        "d_col": f(D[:, None]),
        "w1T": f((W_c1 @ W_out).T),
        "b_c1": f(b_c1[:, None]),
        "w2T": f(np.vstack([W_c2.T, b_c2[None, :]])),
        "ident": np.eye(128, dtype=np.float32),
        "e_sel": _e_sel().astype(ml_dtypes.bfloat16),
        **_packed_consts(f(-np.exp(A_log))),
    }
    in_maps = []
    for c in range(NCORES):
        m = dict(shared)
        m["x"] = x[c * BLOC:(c + 1) * BLOC]
        in_maps.append(m)
    return in_maps


def kernel(**inputs):
    return _run(inputs, trace=False)[0]


def kernel_traced(**inputs):
    return _run(inputs, trace=True)


def _run(inputs, trace=False):
    key = "nc"
    if key not in _cache:
        _cache[key] = _build({})
    nc = _cache[key]
    in_maps = _prep_inputs(inputs)
    res = run_bass_kernel_spmd(nc, in_maps, core_ids=list(range(NCORES)),
                               trace=trace)
    out = np.concatenate([r["out"] for r in res.results], axis=0)
    return out, res


# revision 28
# speedup vs baseline: 1.6750x; 1.0048x over previous
"""Trainium2 Bass kernel for nn_Network_61658550501610 (Mamba block + MLP head).

Reference computation (per batch element b, sequence length L=2048):
  xz = x @ W_in.T; xi, z = split(xz)
  xc = silu(causal_depthwise_conv(xi, conv_w) + conv_b)
  x_dbl = xc @ W_xproj.T -> (dt, B, C)
  delta = softplus(dt @ W_dt.T + b_dt)
  h_t = exp(delta*A)*h_{t-1} + delta*B*xc   (selective scan, state [82,16])
  y = (h @ C) + D*xc; y *= silu(z)
  out = y @ W_out.T;  logits = relu(out@W_c1.T+b_c1)@W_c2.T + b_c2

Sharding: data-parallel over batch (B=16 -> 2 per core across 8 cores).

Layout on chip: d_inner (82) on partitions, time on free dim. The scan uses
the DVE tensor_tensor_scan instruction per state index n (16 of them), with
chunk carries through per-partition initial values. B[n,:]/C[n,:] are
broadcast across partitions with TensorE ones-matmuls into PSUM; the sum
over n runs as accumulating identity matmuls on TensorE.
"""
import ml_dtypes
import numpy as np

import bass_rust as _bass_rust
import concourse.bacc as bacc
import concourse.tile as tile
import concourse.mybir as mybir
from concourse.bass_utils import run_bass_kernel_spmd
from concourse.hw_specs import get_activation_tables


class _Bacc(bacc.Bacc):
    """All activations here use {Exp, Ln, Relu, Copy}, which the combined
    natural_log_exp set covers; blanking the other sets (order kept: set ids
    are positional into act_info.json) pins one resident table -> a single
    ACT_TABLE_LOAD instead of two reloads per chunk."""

    _KEEP = ("natural_log_exp_and_others",)

    def insert_act_table_loads(self):
        has_activation = any(
            isinstance(i, mybir.InstActivation)
            for b in self.main_func.blocks
            for i in b.instructions
        )
        if not has_activation:
            return
        tables = [(name, fns if name in self._KEEP else set())
                  for name, fns in get_activation_tables(self.m.arch).items()]
        _bass_rust.insert_act_table_loads(self, tables)

F32 = mybir.dt.float32
F32R = mybir.dt.float32r
BF16 = mybir.dt.bfloat16
OP = mybir.AluOpType
ACTF = mybir.ActivationFunctionType
AX = mybir.AxisListType

# problem dims (hardcoded per contract)
B, L, DM = 16, 2048, 41
DIN, N, K = 82, 16, 4          # d_inner, d_state, d_conv
DTR, HID, NL = 3, 64, 10
NCORES = 8
BLOC = B // NCORES             # batch per core

DG = (DIN + 7) // 8            # 11 d-groups of 8 for the packed scan
DP = DG * 8                    # 88 padded d
C = 512                        # time-chunk length
NCH = L // C                   # chunks per batch element
Q = C // 128                   # 128-row subtiles per chunk

_cache = {}


def _build(cfg):
    nc = _Bacc("TRN2", target_bir_lowering=False, debug=False,
               enable_asserts=False)

    def din(name, shape):
        return nc.dram_tensor(name, list(shape), F32, kind="ExternalInput").ap()

    x_d = din("x", (BLOC, L, DM))
    w_inT_d = nc.dram_tensor("w_inT", [DM, 2 * DIN], F32R,
                             kind="ExternalInput").ap()
    w_effT_d = nc.dram_tensor("w_effT", [DIN, DIN], F32R,
                              kind="ExternalInput").ap()
    w_bcT_d = nc.dram_tensor("w_bcT", [DIN, 2 * N], F32R,
                             kind="ExternalInput").ap()
    conv_w_d = din("conv_w", (DIN, K))
    conv_diag_d = nc.dram_tensor("conv_diag", [DIN, K * DIN], F32R,
                                 kind="ExternalInput").ap()
    conv_b_d = din("conv_b", (DIN, 1))
    conv_bh_d = din("conv_bh", (DIN, 1))
    b_dt_d = din("b_dt", (DIN, 1))
    d_col_d = din("d_col", (DIN, 1))
    w1T_d = nc.dram_tensor("w1T", [DIN, HID], F32R,
                           kind="ExternalInput").ap()
    b_c1_d = din("b_c1", (HID, 1))
    w2T_d = din("w2T", (HID, NL))
    ident_d = din("ident", (128, 128))
    e_sel_d = nc.dram_tensor("e_sel", [2 * N, 2 * N * DIN], BF16,
                             kind="ExternalInput").ap()
    p_sel_d = nc.dram_tensor("p_sel", [DIN, DG * 128], BF16,
                             kind="ExternalInput").ap()
    ed_sel_d = nc.dram_tensor("ed_sel", [128, DG * DP], BF16,
                              kind="ExternalInput").ap()
    qb_sel_d = nc.dram_tensor("qb_sel", [2 * N, 128], BF16,
                              kind="ExternalInput").ap()
    qc_sel_d = nc.dram_tensor("qc_sel", [2 * N, 128], BF16,
                              kind="ExternalInput").ap()
    a_pack_d = din("a_pack", (128, DG))
    out_d = nc.dram_tensor("out", [BLOC, L, NL], F32, kind="ExternalOutput").ap()

    with tile.TileContext(nc) as tc, tc.tile_pool(name="wts", bufs=1) as wp, \
         tc.tile_pool(name="work", bufs=3) as kp, \
         tc.tile_pool(name="seg", bufs=6) as sp, \
         tc.tile_pool(name="hbuf", bufs=2) as hp, \
         tc.tile_pool(name="ps_f", bufs=3, space="PSUM") as pf, \
         tc.tile_pool(name="ps_t", bufs=2, space="PSUM") as pt, \
         tc.tile_pool(name="ps_rep", bufs=2, space="PSUM") as prep, \
         tc.tile_pool(name="ps_y", bufs=1, space="PSUM") as py:

        # ---- constant weights ----
        w_inT = wp.tile([DM, 2 * DIN], F32R)
        w_effT = wp.tile([DIN, DIN], F32R)
        w_bcT = wp.tile([DIN, 2 * N], F32R)
        conv_w = wp.tile([DIN, K], F32)
        conv_diag = wp.tile([DIN, K * DIN], F32R)
        conv_b = wp.tile([DIN, 1], F32)
        conv_bh = wp.tile([DIN, 1], F32)
        b_dt = wp.tile([DIN, 1], F32)
        d_col = wp.tile([DIN, 1], F32)
        w1T = wp.tile([DIN, HID], F32R)
        b_c1 = wp.tile([HID, 1], F32)
        w2T = wp.tile([HID, NL], F32)
        ident = wp.tile([128, 128], F32)
        p_sel = wp.tile([DIN, DG * 128], BF16)
        ed_sel = wp.tile([128, DG * DP], BF16)
        qb_sel = wp.tile([2 * N, 128], BF16)
        qc_sel = wp.tile([2 * N, 128], BF16)
        a_pack = wp.tile([128, DG], F32)
        for t_, d_ in [(w_inT, w_inT_d), (w_effT, w_effT_d), (w_bcT, w_bcT_d),
                       (conv_w, conv_w_d), (conv_diag, conv_diag_d),
                       (conv_b, conv_b_d),
                       (conv_bh, conv_bh_d),
                       (b_dt, b_dt_d), (d_col, d_col_d), (w1T, w1T_d),
                       (w2T, w2T_d), (ident, ident_d),
                       (p_sel, p_sel_d), (ed_sel, ed_sel_d),
                       (qb_sel, qb_sel_d), (qc_sel, qc_sel_d),
                       (a_pack, a_pack_d),
                       (b_c1, b_c1_d)]:
            nc.sync.dma_start(t_[:], d_[:])

        # persistent state, one per batch element (independent streams)
        h_carry_b = [wp.tile([128, DG], F32, name=f"hcar{i}", tag=f"hcar{i}")
                     for i in range(BLOC)]
        halo_b = [wp.tile([DIN, K - 1], F32, name=f"halo{i}", tag=f"halo{i}")
                  for i in range(BLOC)]
        for t_ in halo_b:
            nc.vector.memset(t_[:], 0.0)

        def front(ch, b):
            h_carry = h_carry_b[b]
            halo = halo_b[b]
            t0 = ch * C
            # ---- load x chunk [C, DM] as [128, Q*DM] ----
            x_in = kp.tile([128, Q * DM], F32)
            src = x_d[b, t0:t0 + C, :].rearrange("(q p) d -> p q d", p=128)
            nc.sync.dma_start(x_in[:].rearrange("p (q d) -> p q d", q=Q), src)

            # ---- transpose to xT [DM, C] ----
            xT_ps = pf.tile([DM, C], F32, tag="f")
            for q in range(Q):
                nc.tensor.transpose(
                    xT_ps[:, q * 128:(q + 1) * 128],
                    x_in[:, q * DM:(q + 1) * DM], ident[:])
            xT = kp.tile([DM, C], F32)
            nc.scalar.copy(xT[:].bitcast(F32R), xT_ps[:])

            # ---- input projection ----
            xi_ps = pf.tile([DIN, C], F32, tag="f")
            z_ps = pf.tile([DIN, C], F32, tag="f")
            nc.tensor.matmul(xi_ps[:], w_inT[:, 0:DIN],
                             xT[:].bitcast(F32R), start=True, stop=True)
            nc.tensor.matmul(z_ps[:], w_inT[:, DIN:2 * DIN],
                             xT[:].bitcast(F32R), start=True, stop=True)

            # silu(z) = z*sigmoid(z); sigmoid(z) = exp(-ln(1+exp(-z)))
            # keeps every Act op on the single ln+exp table set
            e_z = kp.tile([DIN, C], F32)
            nc.scalar.activation(e_z[:], z_ps[:], ACTF.Exp, scale=-1.0)
            sp_z = kp.tile([DIN, C], F32)
            nc.scalar.activation(sp_z[:], e_z[:], ACTF.Ln, bias=1.0)
            sg_z = kp.tile([DIN, C], F32)
            nc.scalar.activation(sg_z[:], sp_z[:], ACTF.Exp, scale=-1.0)
            zs = kp.tile([DIN, C], F32)
            nc.vector.tensor_tensor(zs[:], z_ps[:], sg_z[:], op=OP.mult)

            # ---- causal depthwise conv (K=4) on TensorE ----
            # xi_sb is halo-extended: cols 0..2 = xi[-3:], cols 3..C+2 = xi
            # xc_pre[t] = sum_k diag(w_k) @ xi_sb[:, k+t] (4 accumulating
            # diagonal fp32r matmuls over shifted views)
            xi_sb = kp.tile([DIN, C + K - 1], F32)
            nc.vector.tensor_copy(xi_sb[:, 0:K - 1].bitcast(F32R),
                                  halo[:])
            nc.scalar.copy(xi_sb[:, K - 1:C + K - 1].bitcast(F32R), xi_ps[:])
            if ch < NCH - 1:
                nc.vector.tensor_copy(halo[:], xi_sb[:, C:C + K - 1])
            xc_pre = pf.tile([DIN, C], F32, tag="f")
            for k in range(K):
                nc.tensor.matmul(xc_pre[:],
                                 conv_diag[:, k * DIN:(k + 1) * DIN],
                                 xi_sb[:, k:k + C].bitcast(F32R),
                                 start=(k == 0), stop=(k == K - 1))
            # silu(v) = v*sigmoid(v), v = xc_pre + conv_b;
            # exp(-v) = exp(-xc_pre + conv_bh) with conv_bh = -conv_b
            e_x = kp.tile([DIN, C], F32)
            nc.scalar.activation(e_x[:], xc_pre[:], ACTF.Exp,
                                 bias=conv_bh[:], scale=-1.0)
            sp_x = kp.tile([DIN, C], F32)
            nc.scalar.activation(sp_x[:], e_x[:], ACTF.Ln, bias=1.0)
            sg = kp.tile([DIN, C], F32)
            nc.scalar.activation(sg[:], sp_x[:], ACTF.Exp, scale=-1.0)
            xc = kp.tile([DIN, C], F32)
            nc.vector.scalar_tensor_tensor(xc[:].bitcast(F32R), xc_pre[:],
                                           conv_b[:], sg[:],
                                           op0=OP.add, op1=OP.mult)

            # ---- x_proj: delta / B / C ----
            dpre_ps = pf.tile([DIN, C], F32, tag="f")
            nc.tensor.matmul(dpre_ps[:], w_effT[:],
                             xc[:].bitcast(F32R), start=True, stop=True)
            # softplus(v) = ln(exp(v) + 1), v = dpre + b_dt
            e_sp = kp.tile([DIN, C], F32)
            nc.scalar.activation(e_sp[:], dpre_ps[:], ACTF.Exp, bias=b_dt[:])
            delta = kp.tile([DIN, C], BF16)
            nc.scalar.activation(delta[:], e_sp[:], ACTF.Ln, bias=1.0)
            bc_ps = pf.tile([2 * N, C], F32, tag="f")
            nc.tensor.matmul(bc_ps[:], w_bcT[:],
                             xc[:].bitcast(F32R), start=True, stop=True)
            bc_sb = kp.tile([2 * N, C], BF16)
            nc.scalar.copy(bc_sb[:], bc_ps[:])

            # u = delta * xc (bf16, feeds the p_sel replication matmul)
            u = kp.tile([DIN, C], BF16)
            nc.vector.tensor_tensor(u[:], delta[:], xc[:], op=OP.mult)

            return dict(delta=delta, u=u, bc_sb=bc_sb, xc=xc, zs=zs)

        def back(ch, b, st):
            h_carry = h_carry_b[b]
            halo = halo_b[b]
            t0 = ch * C
            delta, u, bc_sb, xc, zs = (st["delta"], st["u"], st["bc_sb"],
                                       st["xc"], st["zs"])
            # ---- packed selective scan: rows = (n, dsub) ----
            bq_ps = pt.tile([128, C], F32, tag="t")
            nc.tensor.matmul(bq_ps[:], qb_sel[:], bc_sb[:],
                             start=True, stop=True)
            b_sb = kp.tile([128, C], BF16)
            nc.scalar.copy(b_sb[:], bq_ps[:])
            cq_ps = pt.tile([128, C], F32, tag="t")
            nc.tensor.matmul(cq_ps[:], qc_sel[:], bc_sb[:],
                             start=True, stop=True)
            c_sb = kp.tile([128, C], BF16)
            nc.scalar.copy(c_sb[:], cq_ps[:])

            h = hp.tile([128, DG * C], BF16, tag="h")
            y_ps = py.tile([DP, C], F32, tag="y")
            for g in range(DG):
                d_ps = prep.tile([128, C], F32, tag="rep")
                nc.tensor.matmul(d_ps[:], p_sel[:, g * 128:(g + 1) * 128],
                                 delta[:], start=True, stop=True)
                u_ps = prep.tile([128, C], F32, tag="rep")
                nc.tensor.matmul(u_ps[:], p_sel[:, g * 128:(g + 1) * 128],
                                 u[:], start=True, stop=True)
                dA = sp.tile([128, C], BF16, tag="dA")
                nc.scalar.activation(dA[:], d_ps[:], ACTF.Exp,
                                     scale=a_pack[:, g:g + 1])
                u_sb = sp.tile([128, C], BF16, tag="u_sb")
                nc.scalar.copy(u_sb[:], u_ps[:])
                dBx = sp.tile([128, C], BF16, tag="dBx")
                nc.vector.tensor_tensor(dBx[:], u_sb[:], b_sb[:], op=OP.mult)
                hs = h[:, g * C:(g + 1) * C]
                init = 0.0 if ch == 0 else h_carry[:, g:g + 1]
                nc.vector.tensor_tensor_scan(hs, dA[:], dBx[:], init,
                                             op0=OP.mult, op1=OP.add)
                hC = sp.tile([128, C], BF16, tag="hC")
                nc.vector.tensor_tensor(hC[:], c_sb[:], hs, op=OP.mult)
                nc.tensor.matmul(y_ps[:], ed_sel[:, g * DP:(g + 1) * DP],
                                 hC[:], start=(g == 0), stop=(g == DG - 1))
            if ch < NCH - 1:
                nc.vector.tensor_copy(
                    h_carry[:].rearrange("p (g c) -> p g c", c=1),
                    h[:].rearrange("p (g c) -> p g c", g=DG)[:, :, C - 1:C])

            # ---- gate + output ----
            y1 = kp.tile([DIN, C], F32)
            nc.vector.scalar_tensor_tensor(y1[:], xc[:], d_col[:],
                                           y_ps[0:DIN, :],
                                           op0=OP.mult, op1=OP.add)
            y_gated = kp.tile([DIN, C], F32)
            nc.vector.tensor_tensor(y_gated[:].bitcast(F32R), y1[:],
                                    zs[:], op=OP.mult)

            g_ps = pt.tile([HID, C], F32, tag="t")
            nc.tensor.matmul(g_ps[:], w1T[:],
                             y_gated[:].bitcast(F32R), start=True, stop=True)
            g_aug = kp.tile([HID, C], F32)
            nc.scalar.activation(g_aug[:], g_ps[:], ACTF.Relu,
                                 bias=b_c1[:])

            out_sb = kp.tile([128, Q * NL], F32)
            for q in range(Q):
                lg_ps = pt.tile([128, NL], F32, tag="t")
                nc.tensor.matmul(lg_ps[:], g_aug[:, q * 128:(q + 1) * 128],
                                 w2T[:], start=True, stop=True)
                nc.scalar.copy(out_sb[:, q * NL:(q + 1) * NL], lg_ps[:])
            dst = out_d[b, t0:t0 + C, :].rearrange("(q p) c -> p q c", p=128)
            nc.sync.dma_start(
                dst, out_sb[:].rearrange("p (q c) -> p q c", q=Q))


        iters = [(ch, b) for ch in range(NCH) for b in range(BLOC)]
        pend = None
        for j, (ch, b) in enumerate(iters):
            st = front(ch, b)
            if pend is not None:
                back(*pend)
            pend = (ch, b, st)
        back(*pend)

    nc.compile()
    return nc


def _e_sel():
    # e_sel[:, j*DIN:(j+1)*DIN] is [2N, DIN]; row j all-ones, rest zero:
    # lhsT for the TensorE partition-broadcast of bc row j.
    e = np.zeros((2 * N, 2 * N * DIN), np.float32)
    for j in range(2 * N):
        e[j, j * DIN:(j + 1) * DIN] = 1.0
    return e


def _packed_consts(A):
    p_sel = np.zeros((DIN, DG * 128), np.float32)
    ed = np.zeros((128, DG * DP), np.float32)
    qb = np.zeros((2 * N, 128), np.float32)
    qc = np.zeros((2 * N, 128), np.float32)
    a_pack = np.zeros((128, DG), np.float32)
    for n in range(N):
        for ds in range(8):
            r = n * 8 + ds
            qb[n, r] = 1.0
            qc[N + n, r] = 1.0
            for g in range(DG):
                d = g * 8 + ds
                if d < DIN:
                    p_sel[d, g * 128 + r] = 1.0
                    ed[r, g * DP + d] = 1.0
                    a_pack[r, g] = A[d, n]
    bf = ml_dtypes.bfloat16
    return {"p_sel": p_sel.astype(bf), "ed_sel": ed.astype(bf),
            "qb_sel": qb.astype(bf), "qc_sel": qc.astype(bf),
            "a_pack": a_pack}


def _prep_inputs(inputs):
    x = np.ascontiguousarray(inputs["x"], dtype=np.float32)
    W_in = np.asarray(inputs["W_in"], np.float64)
    conv_w = np.asarray(inputs["conv_w"], np.float64)
    conv_b = np.asarray(inputs["conv_b"], np.float64)
    W_xproj = np.asarray(inputs["W_xproj"], np.float64)
    W_dt = np.asarray(inputs["W_dt"], np.float64)
    b_dt = np.asarray(inputs["b_dt"], np.float64)
    A_log = np.asarray(inputs["A_log"], np.float64)
    D = np.asarray(inputs["D"], np.float64)
    W_out = np.asarray(inputs["W_out"], np.float64)
    W_c1 = np.asarray(inputs["W_c1"], np.float64)
    b_c1 = np.asarray(inputs["b_c1"], np.float64)
    W_c2 = np.asarray(inputs["W_c2"], np.float64)
    b_c2 = np.asarray(inputs["b_c2"], np.float64)

    f = lambda a: np.ascontiguousarray(a, dtype=np.float32)
    shared = {
        "w_inT": f(W_in.T),
        "w_effT": f((W_dt @ W_xproj[:DTR]).T),
        "w_bcT": f(W_xproj[DTR:].T),
        "conv_w": f(conv_w),
        "conv_b": f(conv_b[:, None]),
        "conv_diag": np.concatenate(
            [np.diag(conv_w[:, k]) for k in range(K)], axis=1).astype(np.float32),
        "conv_bh": f(-conv_b[:, None]),
        "b_dt": f(b_dt[:, None]),
        "d_col": f(D[:, None]),
        "w1T": f((W_c1 @ W_out).T),
        "b_c1": f(b_c1[:, None]),
        "w2T": f(W_c2.T),
        "ident": np.eye(128, dtype=np.float32),
        "e_sel": _e_sel().astype(ml_dtypes.bfloat16),
        **_packed_consts(f(-np.exp(A_log))),
    }
    in_maps = []
    for c in range(NCORES):
        m = dict(shared)
        m["x"] = x[c * BLOC:(c + 1) * BLOC]
        in_maps.append(m)
    return in_maps


def kernel(**inputs):
    return _run(inputs, trace=False)[0]


def kernel_traced(**inputs):
    return _run(inputs, trace=True)


def _run(inputs, trace=False):
    key = "nc"
    if key not in _cache:
        _cache[key] = _build({})
    nc = _cache[key]
    in_maps = _prep_inputs(inputs)
    res = run_bass_kernel_spmd(nc, in_maps, core_ids=list(range(NCORES)),
                               trace=trace)
    out = np.concatenate([r["out"] for r in res.results], axis=0)
    out = out + np.asarray(inputs["b_c2"], np.float32)[None, None, :]
    return out, res
